# revision 19
# baseline (speedup 1.0000x reference)
"""BrahmanTransformer Trainium2 kernel.

Sharding: data-parallel over batch (32 -> 4 sequences per core x 8 cores),
full 6-layer transformer per core in one Bass/Tile program; float32r matmuls.

Device dataflow (per core, d-major residual):
- LayerNorm: gamma/beta folded into weights host-side; mean subtraction folded
  via column-centered weights; on-device LN is xs = x * rsqrt(var+eps) only.
- Attention: case one-hot (9) + const row (1) appended to q, gathered
  case-bias columns + verb/pad row appended to k -> single K=74 score matmul.
  scoresT layout [key, query]; causality via partial-width matmuls; softmax
  without max-subtraction (scores are O(1)); denominators via a ones column
  appended to V (token-major) and the AV matmul.
- FFN: fc-halves streamed from HBM; psum-accumulated ffn2; gelu on ACT.
- All biases exact: ACT-copy bias (q/k), K=1 matmuls (v/o/ffn2), gelu bias.
"""
import os
import numpy as np

B, L, D, H, NL, F, C = 32, 512, 512, 8, 6, 2048, 9
HD = D // H
NCORES = 8
BLOC = B // NCORES            # 4 sequences per core
SCALE = float(np.sqrt(HD))
EPS = 1e-5
NLB = int(os.environ.get("KB_LAYERS", str(NL)))  # build-depth (debug aid)
DEBUG = bool(int(os.environ.get("KB_DEBUG", "0")))

DC = D // 128     # 4 d-chunks
FC = F // 128     # 16 f-chunks
FH = FC // 2      # 8 f-chunks per streamed half
NP = H // 2       # 4 head pairs

_PROGRAM = None


def _build_program():
    import concourse.bacc as bacc
    import concourse.mybir as mybir
    import concourse.tile as tile
    from concourse.masks import make_identity

    F32 = mybir.dt.float32
    F32R = mybir.dt.float32r
    AF = mybir.ActivationFunctionType
    AL = mybir.AluOpType

    nc = bacc.Bacc("TRN2", target_bir_lowering=False, debug=False)

    def inp(name, shape):
        return nc.dram_tensor(name, shape, F32, kind="ExternalInput").ap()

    xT = inp("xT", [D, BLOC * L])
    wq = inp("wq", [NLB, D, D])
    wk = inp("wk", [NLB, D, D])
    wv = inp("wv", [NLB, D, D])
    wo = inp("wo", [NLB, D, D])
    w1 = inp("w1", [NLB, D, F])
    w2 = inp("w2", [NLB, F, D])
    bqh = inp("bqh", [NLB, HD, H])
    bkh = inp("bkh", [NLB, HD, H])
    bvr = inp("bvr", [NLB, D])
    bor = inp("bor", [NLB, D])
    b1h = inp("b1h", [NLB, 128, FC])
    b2r = inp("b2r", [NLB, D])
    qe = inp("qe", [BLOC, 10, L])
    ke = inp("ke", [NLB, H, BLOC, 10, L])
    tri = inp("tri", [128, 128])
    fng = inp("fng", [128, DC])
    fnb = inp("fnb", [128, DC])
    out = nc.dram_tensor("out", [BLOC, L, D], F32, kind="ExternalOutput").ap()
    dbg = {}
    if DEBUG:
        for nm, shp in [
            ("d_xs", [128, DC, L]), ("d_v", [128, DC, H, HD + 1]),
            ("d_qx", [80, L]), ("d_kx", [80, L]), ("d_wt", [DC, 128, L]),
            ("d_at", [128, L]), ("d_x1", [128, DC, L]), ("d_x2", [128, DC, L]),
            ("d_sc", [DC, 128, L]), ("d_po", [HD + 1, L]), ("d_rb", [HD, L]),
        ]:
            dbg[nm] = nc.dram_tensor(nm, shp, F32, kind="ExternalOutput").ap()

    xT_r = xT.rearrange("(dc p) t -> p dc t", p=128)

    from contextlib import ExitStack

    with tile.TileContext(nc) as tc:
        with ExitStack() as _st:
            _e = _st.enter_context
            cp = _e(tc.tile_pool(name="const", bufs=1))
            wp = _e(tc.tile_pool(name="wpool", bufs=1))
            wf = _e(tc.tile_pool(name="wff", bufs=1))
            xbp = _e(tc.tile_pool(name="xb", bufs=2))
            sqp = _e(tc.tile_pool(name="sq", bufs=1))
            xsp = _e(tc.tile_pool(name="xsp", bufs=1))
            stp = _e(tc.tile_pool(name="stats", bufs=2))
            rs4p = _e(tc.tile_pool(name="rs4", bufs=4))
            qkp = _e(tc.tile_pool(name="qk", bufs=3))
            vep = _e(tc.tile_pool(name="vex", bufs=1))
            wtp = _e(tc.tile_pool(name="wtp", bufs=3))
            atp = _e(tc.tile_pool(name="atn", bufs=2))
            h1p = _e(tc.tile_pool(name="h1g", bufs=3))
            osb = _e(tc.tile_pool(name="osb", bufs=2))
            dram = _e(tc.tile_pool(name="dram", bufs=1, space="DRAM"))
            ps_a = _e(tc.tile_pool(name="ps_a", bufs=3, space="PSUM"))
            ps_av = _e(tc.tile_pool(name="ps_av", bufs=1, space="PSUM"))
            ps_acc = _e(tc.tile_pool(name="ps_acc", bufs=4, space="PSUM"))
            # ---- constants
            onesf = cp.tile([1, L], F32, tag="onesf")
            nc.vector.memset(onesf[:], 1.0)
            ones_row = cp.tile([1, L], F32R, tag="ones_row")
            nc.vector.tensor_copy(ones_row[:], onesf[:])
            redf = cp.tile([128, 128], F32, tag="redf")
            nc.vector.memset(redf[:], 1.0 / D)
            red = cp.tile([128, 128], F32R, tag="red")
            nc.vector.tensor_copy(red[:], redf[:])
            eps_t = cp.tile([128, 1], F32, tag="eps")
            nc.vector.memset(eps_t[:], EPS)
            ones_col = cp.tile([128, 1], F32, tag="ones_col")
            nc.vector.memset(ones_col[:], 1.0)
            ident = cp.tile([128, 128], F32, tag="ident")
            make_identity(nc, ident[:])
            tri_sb = cp.tile([128, 128], F32, tag="tri")
            nc.sync.dma_start(tri_sb[:], tri)
            fng_sb = cp.tile([128, DC], F32, tag="fng")
            nc.sync.dma_start(fng_sb[:], fng)
            fnb_sb = cp.tile([128, DC], F32, tag="fnb")
            nc.sync.dma_start(fnb_sb[:], fnb)

            # residual stream working copies in DRAM:
            # xw = running residual; xa = post-attention snapshot (LN2 input)
            xw = dram.tile([BLOC, 128, DC, L], F32, tag="xw")
            xa = dram.tile([BLOC, 128, DC, L], F32, tag="xa")

            def load_x(b, layer, src=None):
                x_b = xbp.tile([128, DC, L], F32R, tag="xb")
                if src is None and layer == 0:
                    nc.sync.dma_start(
                        x_b[:], xT_r[:, :, b * L:(b + 1) * L].bitcast(F32R)
                    )
                else:
                    nc.sync.dma_start(x_b[:], (xw if src is None else src)[b].bitcast(F32R))
                return x_b

            def ln_stats(x_b, rs_out, mu_out=None):
                """rs_out[:] = rsqrt(var(x)+eps) (+ mean into mu_out)."""
                xsq = sqp.tile([128, DC, L], F32R, tag="xsq")
                nc.scalar.activation(out=xsq[:], in_=x_b[:], func=AF.Square, scale=1.0)
                ps_mu = ps_a.tile([128, L], F32, tag="seq")
                ps_sq = ps_a.tile([128, L], F32, tag="seq")
                for dc in range(DC):
                    nc.tensor.matmul(ps_mu[:], red[:], x_b[:, dc, :],
                                     start=(dc == 0), stop=(dc == DC - 1))
                for dc in range(DC):
                    nc.tensor.matmul(ps_sq[:], red[:], xsq[:, dc, :],
                                     start=(dc == 0), stop=(dc == DC - 1))
                musq = stp.tile([128, L], F32, tag="musq")
                nc.scalar.activation(out=musq[:], in_=ps_mu[:], func=AF.Square, scale=1.0)
                nc.vector.tensor_sub(musq[:], ps_sq[:], musq[:])
                nc.scalar.activation(out=musq[:], in_=musq[:], func=AF.Sqrt,
                                     bias=eps_t[:], scale=1.0)
                scr = stp.tile([128, L], F32, tag="scr")
                nc.vector.reciprocal_approx_accurate(rs_out, musq[:], scr[:])
                if mu_out is not None:
                    nc.vector.tensor_copy(mu_out, ps_mu[:])

            def scaled(x_b, rs):
                xs = xsp.tile([128, DC, L], F32R, tag="xs")
                nc.vector.tensor_mul(
                    xs[:], x_b[:], rs.unsqueeze(1).broadcast_to([128, DC, L])
                )
                return xs

            for i in range(NLB):
                wq_sb = wp.tile([128, DC, D], F32R, tag="wq")
                nc.sync.dma_start(wq_sb[:], wq[i].rearrange("(c p) o -> p c o", p=128).bitcast(F32R))
                wk_sb = wp.tile([128, DC, D], F32R, tag="wk")
                nc.sync.dma_start(wk_sb[:], wk[i].rearrange("(c p) o -> p c o", p=128).bitcast(F32R))
                wv_sb = wp.tile([128, DC, D], F32R, tag="wv")
                nc.sync.dma_start(wv_sb[:], wv[i].rearrange("(c p) o -> p c o", p=128).bitcast(F32R))
                wo_sb = wp.tile([128, DC, D], F32R, tag="wo")
                nc.sync.dma_start(wo_sb[:], wo[i].rearrange("(c p) o -> p c o", p=128).bitcast(F32R))
                bqh_sb = wp.tile([HD, H], F32, tag="bqh")
                nc.sync.dma_start(bqh_sb[:], bqh[i])
                bkh_sb = wp.tile([HD, H], F32, tag="bkh")
                nc.sync.dma_start(bkh_sb[:], bkh[i])
                bvr_sb = wp.tile([1, D], F32R, tag="bvr")
                nc.sync.dma_start(bvr_sb[:], bvr[i].unsqueeze(0).bitcast(F32R))
                bor_sb = wp.tile([1, D], F32R, tag="bor")
                nc.sync.dma_start(bor_sb[:], bor[i].unsqueeze(0).bitcast(F32R))
                b2r_sb = wp.tile([1, D], F32R, tag="b2r")
                nc.sync.dma_start(b2r_sb[:], b2r[i].unsqueeze(0).bitcast(F32R))
                b1h_sb = wp.tile([128, FC], F32, tag="b1h")
                nc.sync.dma_start(b1h_sb[:], b1h[i])

                # ---------------- attention phase (all b) ----------------
                for b in range(BLOC):
                    x_b = load_x(b, i)
                    rs1 = stp.tile([128, L], F32, tag="rs1")
                    ln_stats(x_b, rs1[:])
                    xs = scaled(x_b, rs1[:])
                    if DEBUG and i == 0 and b == 0:
                        nc.sync.dma_start(dbg["d_xs"], xs[:].bitcast(F32))

                    # v projection, token-major, + ones column for denominators
                    vext = vep.tile([128, DC, H, HD + 1], F32R, tag="vext")
                    nc.vector.tensor_copy(
                        vext[:, :, :, HD:HD + 1],
                        ones_col[:].unsqueeze(1).unsqueeze(1).broadcast_to([128, DC, H, 1]),
                    )
                    for t in range(DC):
                        ps_v = ps_a.tile([128, D], F32, tag="seq")
                        for dc in range(DC):
                            nc.tensor.matmul(
                                ps_v[:], xs[:, dc, t * 128:(t + 1) * 128],
                                wv_sb[:, dc, :], start=(dc == 0), stop=False,
                            )
                        nc.tensor.matmul(ps_v[:], ones_row[0:1, 0:128], bvr_sb[:],
                                         start=False, stop=True)
                        nc.vector.tensor_copy(
                            vext[:, t, :, 0:HD],
                            ps_v[:].rearrange("p (h e) -> p h e", h=H),
                        )
                    if DEBUG and i == 0 and b == 0:
                        nc.sync.dma_start(dbg["d_v"], vext[:].bitcast(F32))

                    for pair in range(NP):
                        h0, h1 = 2 * pair, 2 * pair + 1
                        ps_q = ps_a.tile([128, L], F32, tag="seq")
                        ps_k = ps_a.tile([128, L], F32, tag="seq")
                        for dc in range(DC):
                            nc.tensor.matmul(
                                ps_q[:], wq_sb[:, dc, pair * 128:(pair + 1) * 128],
                                xs[:, dc, :], start=(dc == 0), stop=(dc == DC - 1),
                            )
                        for dc in range(DC):
                            nc.tensor.matmul(
                                ps_k[:], wk_sb[:, dc, pair * 128:(pair + 1) * 128],
                                xs[:, dc, :], start=(dc == 0), stop=(dc == DC - 1),
                            )
                        qx, kx = {}, {}
                        for hh in (h0, h1):
                            off = 64 * (hh % 2)
                            qx[hh] = qkp.tile([80, L], F32R, tag="qx", name=f"qx{hh}")
                            nc.vector.tensor_scalar_add(
                                out=qx[hh][0:HD, :], in0=ps_q[off:off + HD, :],
                                scalar1=bqh_sb[:, hh:hh + 1],
                            )
                            nc.sync.dma_start(qx[hh][HD:HD + 10, :], qe[b].bitcast(F32R))
                            kx[hh] = qkp.tile([80, L], F32R, tag="kx", name=f"kx{hh}")
                            nc.vector.tensor_scalar_add(
                                out=kx[hh][0:HD, :], in0=ps_k[off:off + HD, :],
                                scalar1=bkh_sb[:, hh:hh + 1],
                            )
                            nc.sync.dma_start(kx[hh][HD:HD + 10, :],
                                              ke[i, hh, b].bitcast(F32R))
                        if DEBUG and i == 0 and b == 0 and pair == 0:
                            nc.sync.dma_start(dbg["d_qx"], qx[h0][:].bitcast(F32))
                            nc.sync.dma_start(dbg["d_kx"], kx[h0][:].bitcast(F32))

                        attnT = atp.tile([128, L], F32R, tag="attnT")
                        for hh in (h0, h1):
                            wts = []
                            ps_o = ps_av.tile([HD + 1, L], F32, tag="av")
                            for cs in range(DC):
                                n0 = cs * 128
                                ps_s = ps_a.tile([128, L], F32, tag="seq")
                                nc.tensor.matmul(
                                    ps_s[:, 0:L - n0],
                                    kx[hh][0:74, n0:n0 + 128],
                                    qx[hh][0:74, n0:L],
                                    start=True, stop=True,
                                )
                                nc.vector.tensor_add(ps_s[:, 0:128], ps_s[:, 0:128],
                                                     tri_sb[:])
                                if DEBUG and i == 0 and b == 0 and hh == 0:
                                    _scd = stp.tile([128, L], F32, tag="scd", name=f"scd{cs}")
                                    nc.vector.tensor_copy(_scd[:, 0:L - n0], ps_s[:, 0:L - n0])
                                    nc.sync.dma_start(dbg["d_sc"][cs, :, 0:L - n0], _scd[:, 0:L - n0])
                                wt = wtp.tile([128, L], F32R, tag="wt", name=f"wt{cs}")
                                nc.scalar.activation(
                                    out=wt[:, 0:L - n0], in_=ps_s[:, 0:L - n0],
                                    func=AF.Exp, scale=1.0,
                                )
                                if DEBUG and i == 0 and b == 0 and hh == 0:
                                    nc.sync.dma_start(dbg["d_wt"][cs], wt[:].bitcast(F32))
                                wts.append(wt)
                            for cs in range(DC):
                                n0 = cs * 128
                                nc.tensor.matmul(
                                    ps_o[:, n0:L], vext[:, cs, hh, :],
                                    wts[cs][:, 0:L - n0],
                                    start=(cs == 0), stop=(cs == DC - 1),
                                )
                            if DEBUG and i == 0 and b == 0 and hh == 0:
                                _pod = atp.tile([HD + 1, L], F32, tag="pod")
                                nc.vector.tensor_copy(_pod[:], ps_o[:])
                                nc.sync.dma_start(dbg["d_po"], _pod[:])
                            den = stp.tile([1, L], F32, tag="den")
                            nc.vector.tensor_copy(den[:], ps_o[HD:HD + 1, :])
                            rcp = stp.tile([1, L], F32, tag="rcp")
                            rcs = stp.tile([1, L], F32, tag="rcs")
                            nc.vector.reciprocal_approx_accurate(
                                rcp[:], den[:], rcs[:]
                            )
                            rb = stp.tile([HD, L], F32, tag="rb")
                            nc.gpsimd.partition_broadcast(rb[:], rcp[:])
                            if DEBUG and i == 0 and b == 0 and hh == 0:
                                nc.sync.dma_start(dbg["d_rb"], rb[:])
                            off = 64 * (hh % 2)
                            nc.vector.tensor_mul(attnT[off:off + HD, :],
                                                 ps_o[0:HD, :], rb[:])
                        if DEBUG and i == 0 and b == 0 and pair == 0:
                            nc.sync.dma_start(dbg["d_at"], attnT[:].bitcast(F32))

                        for oc in range(DC):
                            if pair == 0:
                                ps_x = ps_acc.tile([128, L], F32, tag="acc", name=f"psx{oc}")
                                if oc == 0:
                                    ps_xs = []
                                ps_xs.append(ps_x)
                            nc.tensor.matmul(
                                ps_xs[oc][:], wo_sb[:, pair, oc * 128:(oc + 1) * 128],
                                attnT[:], start=(pair == 0), stop=False,
                            )
                    for oc in range(DC):
                        nc.tensor.matmul(
                            ps_xs[oc][:], bor_sb[0:1, oc * 128:(oc + 1) * 128],
                            ones_row[:], start=False, stop=True,
                        )
                        nc.vector.tensor_add(x_b[:, oc, :], ps_xs[oc][:], x_b[:, oc, :])

                    if DEBUG and i == 0 and b == 0:
                        nc.sync.dma_start(dbg["d_x1"], x_b[:].bitcast(F32))
                    # LN2 stats on post-attention x (rs kept for both ffn halves)
                    rsb = rs4p.tile([128, L], F32, tag="rsb")
                    ln_stats(x_b, rsb[:])
                    if b == 0:
                        rs_list = []
                    rs_list.append(rsb)
                    nc.sync.dma_start(xa[b], x_b[:].bitcast(F32))

                # ---------------- FFN phase (two streamed halves) ----------------
                for half in range(2):
                    w1h = wf.tile([128, DC, FH * 128], F32R, tag="w1h")
                    nc.sync.dma_start(
                        w1h[:],
                        w1[i, :, half * FH * 128:(half + 1) * FH * 128]
                        .rearrange("(c p) o -> p c o", p=128).bitcast(F32R),
                    )
                    w2h = wf.tile([128, FH, D], F32R, tag="w2h")
                    nc.sync.dma_start(
                        w2h[:],
                        w2[i, half * FH * 128:(half + 1) * FH * 128, :]
                        .rearrange("(c p) o -> p c o", p=128).bitcast(F32R),
                    )
                    for b in range(BLOC):
                        x_b = load_x(b, 1, src=xa)
                        xs2 = scaled(x_b, rs_list[b][:])
                        if half == 1:
                            x_b = load_x(b, 1, src=xw)
                        ps_f = [ps_acc.tile([128, L], F32, tag="acc", name=f"psf{_oc}")
                                for _oc in range(DC)]
                        for fc in range(FH):
                            gfc = half * FH + fc
                            ps_h = ps_a.tile([128, L], F32, tag="seq")
                            for dc in range(DC):
                                nc.tensor.matmul(
                                    ps_h[:], w1h[:, dc, fc * 128:(fc + 1) * 128],
                                    xs2[:, dc, :], start=(dc == 0), stop=(dc == DC - 1),
                                )
                            h1g = h1p.tile([128, L], F32R, tag="h1g")
                            nc.scalar.activation(
                                out=h1g[:], in_=ps_h[:], func=AF.Gelu,
                                bias=b1h_sb[:, gfc:gfc + 1], scale=1.0,
                            )
                            for oc in range(DC):
                                nc.tensor.matmul(
                                    ps_f[oc][:], w2h[:, fc, oc * 128:(oc + 1) * 128],
                                    h1g[:], start=(fc == 0),
                                    stop=(half == 0 and fc == FH - 1),
                                )
                        for oc in range(DC):
                            if half == 1:
                                nc.tensor.matmul(
                                    ps_f[oc][:],
                                    b2r_sb[0:1, oc * 128:(oc + 1) * 128],
                                    ones_row[:], start=False, stop=True,
                                )
                            nc.vector.tensor_add(x_b[:, oc, :], ps_f[oc][:],
                                                 x_b[:, oc, :])
                        if DEBUG and i == 0 and b == 0 and half == 1:
                            nc.sync.dma_start(dbg["d_x2"], x_b[:].bitcast(F32))
                        nc.sync.dma_start(xw[b], x_b[:].bitcast(F32))

            # ---------------- final layernorm + transpose ----------------
            for b in range(BLOC):
                x_b = load_x(b, NLB)
                rs1 = stp.tile([128, L], F32, tag="rs1")
                mu = stp.tile([128, L], F32, tag="mu")
                ln_stats(x_b, rs1[:], mu_out=mu[:])
                xc = xsp.tile([128, DC, L], F32, tag="xs")
                nc.vector.tensor_sub(
                    xc[:], x_b[:], mu[:].unsqueeze(1).broadcast_to([128, DC, L])
                )
                xf = sqp.tile([128, DC, L], F32, tag="xsq")
                nc.vector.tensor_mul(
                    xf[:], xc[:], rs1[:].unsqueeze(1).broadcast_to([128, DC, L])
                )
                for dc in range(DC):
                    nc.vector.tensor_scalar(
                        out=xf[:, dc, :], in0=xf[:, dc, :],
                        scalar1=fng_sb[:, dc:dc + 1], scalar2=fnb_sb[:, dc:dc + 1],
                        op0=AL.mult, op1=AL.add,
                    )
                for t in range(DC):
                    o_sb = osb.tile([128, D], F32, tag="osb")
                    for dc in range(DC):
                        ps_t = ps_a.tile([128, 128], F32, tag="seq")
                        nc.tensor.transpose(
                            ps_t[:], xf[:, dc, t * 128:(t + 1) * 128], ident[:]
                        )
                        nc.vector.tensor_copy(o_sb[:, dc * 128:(dc + 1) * 128], ps_t[:])
                    nc.sync.dma_start(out[b, t * 128:(t + 1) * 128, :], o_sb[:])

    nc.compile()
    return nc


def _center_cols(W):
    return W - W.mean(axis=0, keepdims=True)


def _preprocess(inputs):
    """Host-side folding; returns per-core in_maps."""
    f32 = np.float32
    g = {k: np.asarray(v) for k, v in inputs.items()}
    Wq, Wk, Wv, Wo = g["Wq"], g["Wk"], g["Wv"], g["Wo"]
    W1, W2 = g["W1"], g["W2"]
    g1, b1n = g["ln1_g"], g["ln1_b"]
    g2, b2n = g["ln2_g"], g["ln2_b"]

    wq_e = np.stack([_center_cols(g1[i][:, None] * Wq[i]) / SCALE for i in range(NL)]).astype(f32)
    bq_e = np.stack([(g["bq"][i] + b1n[i] @ Wq[i]) / SCALE for i in range(NL)]).astype(f32)
    wk_e = np.stack([_center_cols(g1[i][:, None] * Wk[i]) for i in range(NL)]).astype(f32)
    bk_e = np.stack([g["bk"][i] + b1n[i] @ Wk[i] for i in range(NL)]).astype(f32)
    wv_e = np.stack([_center_cols(g1[i][:, None] * Wv[i]) for i in range(NL)]).astype(f32)
    bv_e = np.stack([g["bv"][i] + b1n[i] @ Wv[i] for i in range(NL)]).astype(f32)
    w1_e = np.stack([_center_cols(g2[i][:, None] * W1[i]) for i in range(NL)]).astype(f32)
    b1_e = np.stack([g["b1"][i] + b2n[i] @ W1[i] for i in range(NL)]).astype(f32)

    ci = g["case_ids"].astype(np.int64)
    am = g["attention_mask"].astype(f32)
    verb = (ci == 8).astype(f32)
    qe = np.zeros((B, 10, L), f32)
    for c in range(C):
        qe[:, c, :] = (ci == c)
    qe[:, 9, :] = 1.0
    cb = g["case_bias"].astype(f32)
    vb = g["verb_bias"].astype(f32)
    ke = np.zeros((NL, H, B, 10, L), f32)
    for i in range(NL):
        for h in range(H):
            ke[i, h, :, 0:C, :] = np.transpose(cb[i, h][:, ci], (1, 0, 2))
            ke[i, h, :, 9, :] = vb[i, h] * verb - 10000.0 * (1.0 - am)

    tri = np.where(
        np.arange(128)[:, None] > np.arange(128)[None, :], f32(-10000.0), f32(0.0)
    ).astype(f32)

    common = {
        "wq": wq_e[:NLB], "wk": wk_e[:NLB], "wv": wv_e[:NLB],
        "wo": np.ascontiguousarray(Wo.astype(f32)[:NLB]),
        "w1": w1_e[:NLB], "w2": np.ascontiguousarray(W2.astype(f32)[:NLB]),
        "bqh": np.ascontiguousarray(bq_e.reshape(NL, H, HD).transpose(0, 2, 1))[:NLB],
        "bkh": np.ascontiguousarray(bk_e.reshape(NL, H, HD).transpose(0, 2, 1))[:NLB],
        "bvr": bv_e[:NLB], "bor": np.ascontiguousarray(g["bo"].astype(f32)[:NLB]),
        "b1h": np.ascontiguousarray(b1_e.reshape(NL, FC, 128).transpose(0, 2, 1))[:NLB],
        "b2r": np.ascontiguousarray(g["b2"].astype(f32)[:NLB]),
        "tri": tri,
        "fng": np.ascontiguousarray(g["fn_g"].astype(f32).reshape(DC, 128).T),
        "fnb": np.ascontiguousarray(g["fn_b"].astype(f32).reshape(DC, 128).T),
    }
    x = g["x"].astype(f32)
    in_maps = []
    for core in range(NCORES):
        sl = slice(core * BLOC, (core + 1) * BLOC)
        m = dict(common)
        m["xT"] = np.ascontiguousarray(x[sl].reshape(BLOC * L, D).T)
        m["qe"] = np.ascontiguousarray(qe[sl])
        m["ke"] = np.ascontiguousarray(ke[:NLB, :, sl])
        in_maps.append(m)
    return in_maps


def _get_program():
    global _PROGRAM
    if _PROGRAM is None:
        _PROGRAM = _build_program()
    return _PROGRAM


def kernel(**inputs) -> np.ndarray:
    from concourse.bass_utils import run_bass_kernel_spmd

    nc = _get_program()
    in_maps = _preprocess(inputs)
    res = run_bass_kernel_spmd(nc, in_maps, list(range(NCORES)))
    return np.concatenate(
        [res.results[c]["out"] for c in range(NCORES)], axis=0
    ).astype(np.float32)


# revision 20
# speedup vs baseline: 1.0498x; 1.0498x over previous
"""BrahmanTransformer Trainium2 kernel.

Sharding: data-parallel over batch (32 -> 4 sequences per core x 8 cores),
full 6-layer transformer per core in one Bass/Tile program; float32r matmuls.

Device dataflow (per core, d-major residual):
- LayerNorm: gamma/beta folded into weights host-side; mean subtraction folded
  via column-centered weights; on-device LN is xs = x * rsqrt(var+eps) only.
- Attention: case one-hot (9) + const row (1) appended to q, gathered
  case-bias columns + verb/pad row appended to k -> single K=74 score matmul.
  scoresT layout [key, query]; causality via partial-width matmuls; softmax
  without max-subtraction (scores are O(1)); denominators via a ones column
  appended to V (token-major) and the AV matmul.
- FFN: fc-halves streamed from HBM; psum-accumulated ffn2; gelu on ACT.
- All biases exact: ACT-copy bias (q/k), K=1 matmuls (v/o/ffn2), gelu bias.
"""
import os
import numpy as np

B, L, D, H, NL, F, C = 32, 512, 512, 8, 6, 2048, 9
HD = D // H
NCORES = 8
BLOC = B // NCORES            # 4 sequences per core
SCALE = float(np.sqrt(HD))
EPS = 1e-5
NLB = int(os.environ.get("KB_LAYERS", str(NL)))  # build-depth (debug aid)
DEBUG = bool(int(os.environ.get("KB_DEBUG", "0")))

DC = D // 128     # 4 d-chunks
FC = F // 128     # 16 f-chunks
FH = FC // 2      # 8 f-chunks per streamed half
NP = H // 2       # 4 head pairs

_PROGRAM = None


def _build_program():
    import concourse.bacc as bacc
    import concourse.mybir as mybir
    import concourse.tile as tile
    from concourse.masks import make_identity

    F32 = mybir.dt.float32
    F32R = mybir.dt.float32r
    AF = mybir.ActivationFunctionType
    AL = mybir.AluOpType

    nc = bacc.Bacc("TRN2", target_bir_lowering=False, debug=False)

    BF16 = mybir.dt.bfloat16

    def inp(name, shape, dt=F32):
        return nc.dram_tensor(name, shape, dt, kind="ExternalInput").ap()

    xT = inp("xT", [D, BLOC * L])
    wq = inp("wq", [NLB, D, D], BF16)
    wk = inp("wk", [NLB, D, D], BF16)
    wv = inp("wv", [NLB, D, D], BF16)
    wo = inp("wo", [NLB, D, D], BF16)
    w1 = inp("w1", [NLB, D, F], BF16)
    w2 = inp("w2", [NLB, F, D], BF16)
    bqh = inp("bqh", [NLB, HD, H])
    bkh = inp("bkh", [NLB, HD, H])
    bvr = inp("bvr", [NLB, D])
    bor = inp("bor", [NLB, D])
    b1h = inp("b1h", [NLB, 128, FC])
    b2r = inp("b2r", [NLB, D])
    qe = inp("qe", [BLOC, 10, L])
    ke = inp("ke", [NLB, H, BLOC, 10, L])
    tri = inp("tri", [128, 128])
    fng = inp("fng", [128, DC])
    fnb = inp("fnb", [128, DC])
    out = nc.dram_tensor("out", [BLOC, L, D], F32, kind="ExternalOutput").ap()
    dbg = {}
    if DEBUG:
        for nm, shp in [
            ("d_xs", [128, DC, L]), ("d_v", [128, DC, H, HD + 1]),
            ("d_qx", [80, L]), ("d_kx", [80, L]), ("d_wt", [DC, 128, L]),
            ("d_at", [128, L]), ("d_x1", [128, DC, L]), ("d_x2", [128, DC, L]),
            ("d_sc", [DC, 128, L]), ("d_po", [HD + 1, L]), ("d_rb", [HD, L]),
        ]:
            dbg[nm] = nc.dram_tensor(nm, shp, F32, kind="ExternalOutput").ap()

    xT_r = xT.rearrange("(dc p) t -> p dc t", p=128)

    from contextlib import ExitStack

    with tile.TileContext(nc) as tc:
        with ExitStack() as _st:
            _e = _st.enter_context
            cp = _e(tc.tile_pool(name="const", bufs=1))
            wp = _e(tc.tile_pool(name="wpool", bufs=1))
            wf = _e(tc.tile_pool(name="wff", bufs=1))
            xbp = _e(tc.tile_pool(name="xb", bufs=2))
            sqp = _e(tc.tile_pool(name="sq", bufs=1))
            xsp = _e(tc.tile_pool(name="xsp", bufs=1))
            stp = _e(tc.tile_pool(name="stats", bufs=2))
            rs4p = _e(tc.tile_pool(name="rs4", bufs=4))
            qkp = _e(tc.tile_pool(name="qk", bufs=3))
            vep = _e(tc.tile_pool(name="vex", bufs=1))
            wtp = _e(tc.tile_pool(name="wtp", bufs=3))
            atp = _e(tc.tile_pool(name="atn", bufs=2))
            h1p = _e(tc.tile_pool(name="h1g", bufs=3))
            osb = _e(tc.tile_pool(name="osb", bufs=2))
            dram = _e(tc.tile_pool(name="dram", bufs=1, space="DRAM"))
            ps_a = _e(tc.tile_pool(name="ps_a", bufs=3, space="PSUM"))
            ps_av = _e(tc.tile_pool(name="ps_av", bufs=1, space="PSUM"))
            ps_acc = _e(tc.tile_pool(name="ps_acc", bufs=4, space="PSUM"))
            # ---- constants
            onesf = cp.tile([1, L], F32, tag="onesf")
            nc.vector.memset(onesf[:], 1.0)
            ones_row = cp.tile([1, L], F32R, tag="ones_row")
            nc.vector.tensor_copy(ones_row[:], onesf[:])
            redf = cp.tile([128, 128], F32, tag="redf")
            nc.vector.memset(redf[:], 1.0 / D)
            red = cp.tile([128, 128], F32R, tag="red")
            nc.vector.tensor_copy(red[:], redf[:])
            eps_t = cp.tile([128, 1], F32, tag="eps")
            nc.vector.memset(eps_t[:], EPS)
            ones_col = cp.tile([128, 1], F32, tag="ones_col")
            nc.vector.memset(ones_col[:], 1.0)
            ident = cp.tile([128, 128], F32, tag="ident")
            make_identity(nc, ident[:])
            tri_sb = cp.tile([128, 128], F32, tag="tri")
            nc.sync.dma_start(tri_sb[:], tri)
            fng_sb = cp.tile([128, DC], F32, tag="fng")
            nc.sync.dma_start(fng_sb[:], fng)
            fnb_sb = cp.tile([128, DC], F32, tag="fnb")
            nc.sync.dma_start(fnb_sb[:], fnb)

            # residual stream working copies in DRAM:
            # xw = running residual; xa = post-attention snapshot (LN2 input)
            xw = dram.tile([BLOC, 128, DC, L], F32, tag="xw")
            xa = dram.tile([BLOC, 128, DC, L], F32, tag="xa")

            def load_x(b, layer, src=None):
                x_b = xbp.tile([128, DC, L], F32R, tag="xb")
                if src is None and layer == 0:
                    nc.sync.dma_start(
                        x_b[:], xT_r[:, :, b * L:(b + 1) * L].bitcast(F32R)
                    )
                else:
                    nc.sync.dma_start(x_b[:], (xw if src is None else src)[b].bitcast(F32R))
                return x_b

            def ln_stats(x_b, rs_out, mu_out=None):
                """rs_out[:] = rsqrt(var(x)+eps) (+ mean into mu_out)."""
                xsq = sqp.tile([128, DC, L], F32R, tag="xsq")
                nc.scalar.activation(out=xsq[:], in_=x_b[:], func=AF.Square, scale=1.0)
                ps_mu = ps_a.tile([128, L], F32, tag="seq")
                ps_sq = ps_a.tile([128, L], F32, tag="seq")
                for dc in range(DC):
                    nc.tensor.matmul(ps_mu[:], red[:], x_b[:, dc, :],
                                     start=(dc == 0), stop=(dc == DC - 1))
                for dc in range(DC):
                    nc.tensor.matmul(ps_sq[:], red[:], xsq[:, dc, :],
                                     start=(dc == 0), stop=(dc == DC - 1))
                musq = stp.tile([128, L], F32, tag="musq")
                nc.scalar.activation(out=musq[:], in_=ps_mu[:], func=AF.Square, scale=1.0)
                nc.vector.tensor_sub(musq[:], ps_sq[:], musq[:])
                nc.scalar.activation(out=musq[:], in_=musq[:], func=AF.Sqrt,
                                     bias=eps_t[:], scale=1.0)
                scr = stp.tile([128, L], F32, tag="scr")
                nc.vector.reciprocal_approx_accurate(rs_out, musq[:], scr[:])
                if mu_out is not None:
                    nc.vector.tensor_copy(mu_out, ps_mu[:])

            def scaled(x_b, rs):
                xs = xsp.tile([128, DC, L], BF16, tag="xs")
                nc.vector.tensor_mul(
                    xs[:], x_b[:], rs.unsqueeze(1).broadcast_to([128, DC, L])
                )
                return xs

            for i in range(NLB):
                wq_sb = wp.tile([128, DC, D], BF16, tag="wq")
                nc.sync.dma_start(wq_sb[:], wq[i].rearrange("(c p) o -> p c o", p=128))
                wk_sb = wp.tile([128, DC, D], BF16, tag="wk")
                nc.sync.dma_start(wk_sb[:], wk[i].rearrange("(c p) o -> p c o", p=128))
                wv_sb = wp.tile([128, DC, D], BF16, tag="wv")
                nc.sync.dma_start(wv_sb[:], wv[i].rearrange("(c p) o -> p c o", p=128))
                wo_sb = wp.tile([128, DC, D], BF16, tag="wo")
                nc.sync.dma_start(wo_sb[:], wo[i].rearrange("(c p) o -> p c o", p=128))
                bqh_sb = wp.tile([HD, H], F32, tag="bqh")
                nc.sync.dma_start(bqh_sb[:], bqh[i])
                bkh_sb = wp.tile([HD, H], F32, tag="bkh")
                nc.sync.dma_start(bkh_sb[:], bkh[i])
                bvr_sb = wp.tile([1, D], F32R, tag="bvr")
                nc.sync.dma_start(bvr_sb[:], bvr[i].unsqueeze(0).bitcast(F32R))
                bor_sb = wp.tile([1, D], F32R, tag="bor")
                nc.sync.dma_start(bor_sb[:], bor[i].unsqueeze(0).bitcast(F32R))
                b2r_sb = wp.tile([1, D], F32R, tag="b2r")
                nc.sync.dma_start(b2r_sb[:], b2r[i].unsqueeze(0).bitcast(F32R))
                b1h_sb = wp.tile([128, FC], F32, tag="b1h")
                nc.sync.dma_start(b1h_sb[:], b1h[i])

                # ---------------- attention phase (all b) ----------------
                for b in range(BLOC):
                    x_b = load_x(b, i)
                    rs1 = stp.tile([128, L], F32, tag="rs1")
                    ln_stats(x_b, rs1[:])
                    xs = scaled(x_b, rs1[:])
                    if DEBUG and i == 0 and b == 0:
                        nc.sync.dma_start(dbg["d_xs"], xs[:].bitcast(F32))

                    # v projection, token-major, + ones column for denominators
                    vext = vep.tile([128, DC, H, HD + 1], F32R, tag="vext")
                    nc.vector.tensor_copy(
                        vext[:, :, :, HD:HD + 1],
                        ones_col[:].unsqueeze(1).unsqueeze(1).broadcast_to([128, DC, H, 1]),
                    )
                    for t in range(DC):
                        ps_v = ps_a.tile([128, D], F32, tag="seq")
                        for dc in range(DC):
                            nc.tensor.matmul(
                                ps_v[:], xs[:, dc, t * 128:(t + 1) * 128],
                                wv_sb[:, dc, :], start=(dc == 0), stop=False,
                            )
                        nc.tensor.matmul(ps_v[:], ones_row[0:1, 0:128], bvr_sb[:],
                                         start=False, stop=True)
                        nc.vector.tensor_copy(
                            vext[:, t, :, 0:HD],
                            ps_v[:].rearrange("p (h e) -> p h e", h=H),
                        )
                    if DEBUG and i == 0 and b == 0:
                        nc.sync.dma_start(dbg["d_v"], vext[:].bitcast(F32))

                    for pair in range(NP):
                        h0, h1 = 2 * pair, 2 * pair + 1
                        ps_q = ps_a.tile([128, L], F32, tag="seq")
                        ps_k = ps_a.tile([128, L], F32, tag="seq")
                        for dc in range(DC):
                            nc.tensor.matmul(
                                ps_q[:], wq_sb[:, dc, pair * 128:(pair + 1) * 128],
                                xs[:, dc, :], start=(dc == 0), stop=(dc == DC - 1),
                            )
                        for dc in range(DC):
                            nc.tensor.matmul(
                                ps_k[:], wk_sb[:, dc, pair * 128:(pair + 1) * 128],
                                xs[:, dc, :], start=(dc == 0), stop=(dc == DC - 1),
                            )
                        qx, kx = {}, {}
                        for hh in (h0, h1):
                            off = 64 * (hh % 2)
                            qx[hh] = qkp.tile([80, L], F32R, tag="qx", name=f"qx{hh}")
                            nc.vector.tensor_scalar_add(
                                out=qx[hh][0:HD, :], in0=ps_q[off:off + HD, :],
                                scalar1=bqh_sb[:, hh:hh + 1],
                            )
                            nc.sync.dma_start(qx[hh][HD:HD + 10, :], qe[b].bitcast(F32R))
                            kx[hh] = qkp.tile([80, L], F32R, tag="kx", name=f"kx{hh}")
                            nc.vector.tensor_scalar_add(
                                out=kx[hh][0:HD, :], in0=ps_k[off:off + HD, :],
                                scalar1=bkh_sb[:, hh:hh + 1],
                            )
                            nc.sync.dma_start(kx[hh][HD:HD + 10, :],
                                              ke[i, hh, b].bitcast(F32R))
                        if DEBUG and i == 0 and b == 0 and pair == 0:
                            nc.sync.dma_start(dbg["d_qx"], qx[h0][:].bitcast(F32))
                            nc.sync.dma_start(dbg["d_kx"], kx[h0][:].bitcast(F32))

                        attnT = atp.tile([128, L], BF16, tag="attnT")
                        for hh in (h0, h1):
                            wts = []
                            ps_o = ps_av.tile([HD + 1, L], F32, tag="av")
                            for cs in range(DC):
                                n0 = cs * 128
                                ps_s = ps_a.tile([128, L], F32, tag="seq")
                                nc.tensor.matmul(
                                    ps_s[:, 0:L - n0],
                                    kx[hh][0:74, n0:n0 + 128],
                                    qx[hh][0:74, n0:L],
                                    start=True, stop=True,
                                )
                                nc.vector.tensor_add(ps_s[:, 0:128], ps_s[:, 0:128],
                                                     tri_sb[:])
                                if DEBUG and i == 0 and b == 0 and hh == 0:
                                    _scd = stp.tile([128, L], F32, tag="scd", name=f"scd{cs}")
                                    nc.vector.tensor_copy(_scd[:, 0:L - n0], ps_s[:, 0:L - n0])
                                    nc.sync.dma_start(dbg["d_sc"][cs, :, 0:L - n0], _scd[:, 0:L - n0])
                                wt = wtp.tile([128, L], F32R, tag="wt", name=f"wt{cs}")
                                nc.scalar.activation(
                                    out=wt[:, 0:L - n0], in_=ps_s[:, 0:L - n0],
                                    func=AF.Exp, scale=1.0,
                                )
                                if DEBUG and i == 0 and b == 0 and hh == 0:
                                    nc.sync.dma_start(dbg["d_wt"][cs], wt[:].bitcast(F32))
                                wts.append(wt)
                            for cs in range(DC):
                                n0 = cs * 128
                                nc.tensor.matmul(
                                    ps_o[:, n0:L], vext[:, cs, hh, :],
                                    wts[cs][:, 0:L - n0],
                                    start=(cs == 0), stop=(cs == DC - 1),
                                )
                            if DEBUG and i == 0 and b == 0 and hh == 0:
                                _pod = atp.tile([HD + 1, L], F32, tag="pod")
                                nc.vector.tensor_copy(_pod[:], ps_o[:])
                                nc.sync.dma_start(dbg["d_po"], _pod[:])
                            den = stp.tile([1, L], F32, tag="den")
                            nc.vector.tensor_copy(den[:], ps_o[HD:HD + 1, :])
                            rcp = stp.tile([1, L], F32, tag="rcp")
                            rcs = stp.tile([1, L], F32, tag="rcs")
                            nc.vector.reciprocal_approx_accurate(
                                rcp[:], den[:], rcs[:]
                            )
                            rb = stp.tile([HD, L], F32, tag="rb")
                            nc.gpsimd.partition_broadcast(rb[:], rcp[:])
                            if DEBUG and i == 0 and b == 0 and hh == 0:
                                nc.sync.dma_start(dbg["d_rb"], rb[:])
                            off = 64 * (hh % 2)
                            nc.vector.tensor_mul(attnT[off:off + HD, :],
                                                 ps_o[0:HD, :], rb[:])
                        if DEBUG and i == 0 and b == 0 and pair == 0:
                            nc.sync.dma_start(dbg["d_at"], attnT[:].bitcast(F32))

                        for oc in range(DC):
                            if pair == 0:
                                ps_x = ps_acc.tile([128, L], F32, tag="acc", name=f"psx{oc}")
                                if oc == 0:
                                    ps_xs = []
                                ps_xs.append(ps_x)
                            nc.tensor.matmul(
                                ps_xs[oc][:], wo_sb[:, pair, oc * 128:(oc + 1) * 128],
                                attnT[:], start=(pair == 0), stop=False,
                            )
                    for oc in range(DC):
                        nc.tensor.matmul(
                            ps_xs[oc][:], bor_sb[0:1, oc * 128:(oc + 1) * 128],
                            ones_row[:], start=False, stop=True,
                        )
                        nc.vector.tensor_add(x_b[:, oc, :], ps_xs[oc][:], x_b[:, oc, :])

                    if DEBUG and i == 0 and b == 0:
                        nc.sync.dma_start(dbg["d_x1"], x_b[:].bitcast(F32))
                    # LN2 stats on post-attention x (rs kept for both ffn halves)
                    rsb = rs4p.tile([128, L], F32, tag="rsb")
                    ln_stats(x_b, rsb[:])
                    if b == 0:
                        rs_list = []
                    rs_list.append(rsb)
                    nc.sync.dma_start(xa[b], x_b[:].bitcast(F32))

                # ---------------- FFN phase (two streamed halves) ----------------
                for half in range(2):
                    w1h = wf.tile([128, DC, FH * 128], BF16, tag="w1h")
                    nc.sync.dma_start(
                        w1h[:],
                        w1[i, :, half * FH * 128:(half + 1) * FH * 128]
                        .rearrange("(c p) o -> p c o", p=128),
                    )
                    w2h = wf.tile([128, FH, D], BF16, tag="w2h")
                    nc.sync.dma_start(
                        w2h[:],
                        w2[i, half * FH * 128:(half + 1) * FH * 128, :]
                        .rearrange("(c p) o -> p c o", p=128),
                    )
                    for b in range(BLOC):
                        x_b = load_x(b, 1, src=xa)
                        xs2 = scaled(x_b, rs_list[b][:])
                        if half == 1:
                            x_b = load_x(b, 1, src=xw)
                        ps_f = [ps_acc.tile([128, L], F32, tag="acc", name=f"psf{_oc}")
                                for _oc in range(DC)]
                        for fc in range(FH):
                            gfc = half * FH + fc
                            ps_h = ps_a.tile([128, L], F32, tag="seq")
                            for dc in range(DC):
                                nc.tensor.matmul(
                                    ps_h[:], w1h[:, dc, fc * 128:(fc + 1) * 128],
                                    xs2[:, dc, :], start=(dc == 0), stop=(dc == DC - 1),
                                )
                            h1g = h1p.tile([128, L], BF16, tag="h1g")
                            nc.scalar.activation(
                                out=h1g[:], in_=ps_h[:], func=AF.Gelu,
                                bias=b1h_sb[:, gfc:gfc + 1], scale=1.0,
                            )
                            for oc in range(DC):
                                nc.tensor.matmul(
                                    ps_f[oc][:], w2h[:, fc, oc * 128:(oc + 1) * 128],
                                    h1g[:], start=(fc == 0),
                                    stop=(half == 0 and fc == FH - 1),
                                )
                        for oc in range(DC):
                            if half == 1:
                                nc.tensor.matmul(
                                    ps_f[oc][:],
                                    b2r_sb[0:1, oc * 128:(oc + 1) * 128],
                                    ones_row[:], start=False, stop=True,
                                )
                            nc.vector.tensor_add(x_b[:, oc, :], ps_f[oc][:],
                                                 x_b[:, oc, :])
                        if DEBUG and i == 0 and b == 0 and half == 1:
                            nc.sync.dma_start(dbg["d_x2"], x_b[:].bitcast(F32))
                        nc.sync.dma_start(xw[b], x_b[:].bitcast(F32))

            # ---------------- final layernorm + transpose ----------------
            for b in range(BLOC):
                x_b = load_x(b, NLB)
                rs1 = stp.tile([128, L], F32, tag="rs1")
                mu = stp.tile([128, L], F32, tag="mu")
                ln_stats(x_b, rs1[:], mu_out=mu[:])
                xc = xsp.tile([128, DC, L], F32, tag="xs")
                nc.vector.tensor_sub(
                    xc[:], x_b[:], mu[:].unsqueeze(1).broadcast_to([128, DC, L])
                )
                xf = sqp.tile([128, DC, L], F32, tag="xsq")
                nc.vector.tensor_mul(
                    xf[:], xc[:], rs1[:].unsqueeze(1).broadcast_to([128, DC, L])
                )
                for dc in range(DC):
                    nc.vector.tensor_scalar(
                        out=xf[:, dc, :], in0=xf[:, dc, :],
                        scalar1=fng_sb[:, dc:dc + 1], scalar2=fnb_sb[:, dc:dc + 1],
                        op0=AL.mult, op1=AL.add,
                    )
                for t in range(DC):
                    o_sb = osb.tile([128, D], F32, tag="osb")
                    for dc in range(DC):
                        ps_t = ps_a.tile([128, 128], F32, tag="seq")
                        nc.tensor.transpose(
                            ps_t[:], xf[:, dc, t * 128:(t + 1) * 128], ident[:]
                        )
                        nc.vector.tensor_copy(o_sb[:, dc * 128:(dc + 1) * 128], ps_t[:])
                    nc.sync.dma_start(out[b, t * 128:(t + 1) * 128, :], o_sb[:])

    nc.compile()
    return nc


def _center_cols(W):
    return W - W.mean(axis=0, keepdims=True)


def _preprocess(inputs):
    """Host-side folding; returns per-core in_maps."""
    f32 = np.float32
    g = {k: np.asarray(v) for k, v in inputs.items()}
    Wq, Wk, Wv, Wo = g["Wq"], g["Wk"], g["Wv"], g["Wo"]
    W1, W2 = g["W1"], g["W2"]
    g1, b1n = g["ln1_g"], g["ln1_b"]
    g2, b2n = g["ln2_g"], g["ln2_b"]

    wq_e = np.stack([_center_cols(g1[i][:, None] * Wq[i]) / SCALE for i in range(NL)]).astype(f32)
    bq_e = np.stack([(g["bq"][i] + b1n[i] @ Wq[i]) / SCALE for i in range(NL)]).astype(f32)
    wk_e = np.stack([_center_cols(g1[i][:, None] * Wk[i]) for i in range(NL)]).astype(f32)
    bk_e = np.stack([g["bk"][i] + b1n[i] @ Wk[i] for i in range(NL)]).astype(f32)
    wv_e = np.stack([_center_cols(g1[i][:, None] * Wv[i]) for i in range(NL)]).astype(f32)
    bv_e = np.stack([g["bv"][i] + b1n[i] @ Wv[i] for i in range(NL)]).astype(f32)
    w1_e = np.stack([_center_cols(g2[i][:, None] * W1[i]) for i in range(NL)]).astype(f32)
    b1_e = np.stack([g["b1"][i] + b2n[i] @ W1[i] for i in range(NL)]).astype(f32)

    ci = g["case_ids"].astype(np.int64)
    am = g["attention_mask"].astype(f32)
    verb = (ci == 8).astype(f32)
    qe = np.zeros((B, 10, L), f32)
    for c in range(C):
        qe[:, c, :] = (ci == c)
    qe[:, 9, :] = 1.0
    cb = g["case_bias"].astype(f32)
    vb = g["verb_bias"].astype(f32)
    ke = np.zeros((NL, H, B, 10, L), f32)
    for i in range(NL):
        for h in range(H):
            ke[i, h, :, 0:C, :] = np.transpose(cb[i, h][:, ci], (1, 0, 2))
            ke[i, h, :, 9, :] = vb[i, h] * verb - 10000.0 * (1.0 - am)

    tri = np.where(
        np.arange(128)[:, None] > np.arange(128)[None, :], f32(-10000.0), f32(0.0)
    ).astype(f32)

    import ml_dtypes
    bf16 = ml_dtypes.bfloat16
    common = {
        "wq": wq_e[:NLB].astype(bf16), "wk": wk_e[:NLB].astype(bf16),
        "wv": wv_e[:NLB].astype(bf16),
        "wo": np.ascontiguousarray(Wo.astype(f32)[:NLB]).astype(bf16),
        "w1": w1_e[:NLB].astype(bf16),
        "w2": np.ascontiguousarray(W2.astype(f32)[:NLB]).astype(bf16),
        "bqh": np.ascontiguousarray(bq_e.reshape(NL, H, HD).transpose(0, 2, 1))[:NLB],
        "bkh": np.ascontiguousarray(bk_e.reshape(NL, H, HD).transpose(0, 2, 1))[:NLB],
        "bvr": bv_e[:NLB], "bor": np.ascontiguousarray(g["bo"].astype(f32)[:NLB]),
        "b1h": np.ascontiguousarray(b1_e.reshape(NL, FC, 128).transpose(0, 2, 1))[:NLB],
        "b2r": np.ascontiguousarray(g["b2"].astype(f32)[:NLB]),
        "tri": tri,
        "fng": np.ascontiguousarray(g["fn_g"].astype(f32).reshape(DC, 128).T),
        "fnb": np.ascontiguousarray(g["fn_b"].astype(f32).reshape(DC, 128).T),
    }
    x = g["x"].astype(f32)
    in_maps = []
    for core in range(NCORES):
        sl = slice(core * BLOC, (core + 1) * BLOC)
        m = dict(common)
        m["xT"] = np.ascontiguousarray(x[sl].reshape(BLOC * L, D).T)
        m["qe"] = np.ascontiguousarray(qe[sl])
        m["ke"] = np.ascontiguousarray(ke[:NLB, :, sl])
        in_maps.append(m)
    return in_maps


def _get_program():
    global _PROGRAM
    if _PROGRAM is None:
        _PROGRAM = _build_program()
    return _PROGRAM


def kernel(**inputs) -> np.ndarray:
    from concourse.bass_utils import run_bass_kernel_spmd

    nc = _get_program()
    in_maps = _preprocess(inputs)
    res = run_bass_kernel_spmd(nc, in_maps, list(range(NCORES)))
    return np.concatenate(
        [res.results[c]["out"] for c in range(NCORES)], axis=0
    ).astype(np.float32)


# revision 24
# speedup vs baseline: 1.0499x; 1.0001x over previous
"""BrahmanTransformer Trainium2 kernel.

Sharding: data-parallel over batch (32 -> 4 sequences per core x 8 cores),
full 6-layer transformer per core in one Bass/Tile program; float32r matmuls.

Device dataflow (per core, d-major residual):
- LayerNorm: gamma/beta folded into weights host-side; mean subtraction folded
  via column-centered weights; on-device LN is xs = x * rsqrt(var+eps) only.
- Attention: case one-hot (9) + const row (1) appended to q, gathered
  case-bias columns + verb/pad row appended to k -> single K=74 score matmul.
  scoresT layout [key, query]; causality via partial-width matmuls; softmax
  without max-subtraction (scores are O(1)); denominators via a ones column
  appended to V (token-major) and the AV matmul.
- FFN: fc-halves streamed from HBM; psum-accumulated ffn2; gelu on ACT.
- All biases exact: ACT-copy bias (q/k), K=1 matmuls (v/o/ffn2), gelu bias.
"""
import os
import numpy as np

B, L, D, H, NL, F, C = 32, 512, 512, 8, 6, 2048, 9
HD = D // H
NCORES = 8
BLOC = B // NCORES            # 4 sequences per core
SCALE = float(np.sqrt(HD))
EPS = 1e-5
NLB = int(os.environ.get("KB_LAYERS", str(NL)))  # build-depth (debug aid)
DEBUG = bool(int(os.environ.get("KB_DEBUG", "0")))

DC = D // 128     # 4 d-chunks
FC = F // 128     # 16 f-chunks
FH = FC // 2      # 8 f-chunks per streamed half
NP = H // 2       # 4 head pairs

_PROGRAM = None


def _build_program():
    import concourse.bacc as bacc
    import concourse.mybir as mybir
    import concourse.tile as tile
    from concourse.masks import make_identity

    F32 = mybir.dt.float32
    F32R = mybir.dt.float32r
    AF = mybir.ActivationFunctionType
    AL = mybir.AluOpType

    nc = bacc.Bacc("TRN2", target_bir_lowering=False, debug=False)

    BF16 = mybir.dt.bfloat16

    def inp(name, shape, dt=F32):
        return nc.dram_tensor(name, shape, dt, kind="ExternalInput").ap()

    xT = inp("xT", [D, BLOC * L])
    wq = inp("wq", [NLB, D, D], BF16)
    wk = inp("wk", [NLB, D, D], BF16)
    wv = inp("wv", [NLB, D, D], BF16)
    wo = inp("wo", [NLB, D, D], BF16)
    w1 = inp("w1", [NLB, D, F], BF16)
    w2 = inp("w2", [NLB, F, D], BF16)
    bqh = inp("bqh", [NLB, HD, H])
    bkh = inp("bkh", [NLB, HD, H])
    bvr = inp("bvr", [NLB, D])
    bor = inp("bor", [NLB, D])
    b1h = inp("b1h", [NLB, 128, FC])
    b2r = inp("b2r", [NLB, D])
    qe = inp("qe", [BLOC, 10, L])
    ke = inp("ke", [NLB, H, BLOC, 10, L])
    tri = inp("tri", [128, 128])
    fng = inp("fng", [128, DC])
    fnb = inp("fnb", [128, DC])
    out = nc.dram_tensor("out", [BLOC, L, D], F32, kind="ExternalOutput").ap()
    dbg = {}
    if DEBUG:
        BF16_ = mybir.dt.bfloat16
        for nm, shp, dt_ in [
            ("d_xs", [128, DC, L], BF16_), ("d_v", [128, DC, H, HD + 1], BF16_),
            ("d_qx", [80, L], BF16_), ("d_kx", [80, L], BF16_),
            ("d_wt", [DC, 128, L], BF16_),
            ("d_at", [128, L], BF16_), ("d_x1", [128, DC, L], F32),
            ("d_x2", [128, DC, L], F32),
            ("d_sc", [DC, 128, L], F32), ("d_po", [HD + 1, L], F32),
            ("d_rb", [HD, L], F32),
        ]:
            dbg[nm] = nc.dram_tensor(nm, shp, dt_, kind="ExternalOutput").ap()

    xT_r = xT.rearrange("(dc p) t -> p dc t", p=128)

    from contextlib import ExitStack

    with tile.TileContext(nc) as tc:
        with ExitStack() as _st:
            _e = _st.enter_context
            cp = _e(tc.tile_pool(name="const", bufs=1))
            wp = _e(tc.tile_pool(name="wpool", bufs=1))
            wf = _e(tc.tile_pool(name="wff", bufs=1))
            xbp = _e(tc.tile_pool(name="xb", bufs=2))
            sqp = _e(tc.tile_pool(name="sq", bufs=1))
            xsp = _e(tc.tile_pool(name="xsp", bufs=1))
            stp = _e(tc.tile_pool(name="stats", bufs=2))
            rs4p = _e(tc.tile_pool(name="rs4", bufs=4))
            qkp = _e(tc.tile_pool(name="qk", bufs=3))
            vep = _e(tc.tile_pool(name="vex", bufs=1))
            wtp = _e(tc.tile_pool(name="wtp", bufs=3))
            atp = _e(tc.tile_pool(name="atn", bufs=2))
            h1p = _e(tc.tile_pool(name="h1g", bufs=3))
            osb = _e(tc.tile_pool(name="osb", bufs=2))
            dram = _e(tc.tile_pool(name="dram", bufs=1, space="DRAM"))
            ps_a = _e(tc.tile_pool(name="ps_a", bufs=3, space="PSUM"))
            ps_av = _e(tc.tile_pool(name="ps_av", bufs=1, space="PSUM"))
            ps_acc = _e(tc.tile_pool(name="ps_acc", bufs=4, space="PSUM"))
            # ---- constants
            onesf = cp.tile([1, L], F32, tag="onesf")
            nc.vector.memset(onesf[:], 1.0)
            ones_row = cp.tile([1, L], F32R, tag="ones_row")
            nc.vector.tensor_copy(ones_row[:], onesf[:])
            redf = cp.tile([128, 128], F32, tag="redf")
            nc.vector.memset(redf[:], 1.0 / D)
            red = cp.tile([128, 128], F32R, tag="red")
            nc.vector.tensor_copy(red[:], redf[:])
            eps_t = cp.tile([128, 1], F32, tag="eps")
            nc.vector.memset(eps_t[:], EPS)
            ones_col = cp.tile([128, 1], F32, tag="ones_col")
            nc.vector.memset(ones_col[:], 1.0)
            ident = cp.tile([128, 128], F32, tag="ident")
            make_identity(nc, ident[:])
            tri_sb = cp.tile([128, 128], F32, tag="tri")
            nc.sync.dma_start(tri_sb[:], tri)
            fng_sb = cp.tile([128, DC], F32, tag="fng")
            nc.sync.dma_start(fng_sb[:], fng)
            fnb_sb = cp.tile([128, DC], F32, tag="fnb")
            nc.sync.dma_start(fnb_sb[:], fnb)

            # residual stream working copies in DRAM:
            # xw = running residual; xa = post-attention snapshot (LN2 input)
            xw = dram.tile([BLOC, 128, DC, L], F32, tag="xw")
            xa = dram.tile([BLOC, 128, DC, L], F32, tag="xa")

            def load_x(b, layer, src=None):
                x_b = xbp.tile([128, DC, L], F32R, tag="xb")
                if src is None and layer == 0:
                    nc.sync.dma_start(
                        x_b[:], xT_r[:, :, b * L:(b + 1) * L].bitcast(F32R)
                    )
                else:
                    nc.sync.dma_start(x_b[:], (xw if src is None else src)[b].bitcast(F32R))
                return x_b

            def ln_stats(x_b, rs_out, mu_out=None):
                """rs_out[:] = rsqrt(var(x)+eps) (+ mean into mu_out)."""
                xsq = sqp.tile([128, DC, L], F32R, tag="xsq")
                nc.scalar.activation(out=xsq[:], in_=x_b[:], func=AF.Square, scale=1.0)
                ps_mu = ps_a.tile([128, L], F32, tag="seq")
                ps_sq = ps_a.tile([128, L], F32, tag="seq")
                for dc in range(DC):
                    nc.tensor.matmul(ps_mu[:], red[:], x_b[:, dc, :],
                                     start=(dc == 0), stop=(dc == DC - 1))
                for dc in range(DC):
                    nc.tensor.matmul(ps_sq[:], red[:], xsq[:, dc, :],
                                     start=(dc == 0), stop=(dc == DC - 1))
                musq = stp.tile([128, L], F32, tag="musq")
                nc.scalar.activation(out=musq[:], in_=ps_mu[:], func=AF.Square, scale=1.0)
                nc.vector.tensor_sub(musq[:], ps_sq[:], musq[:])
                nc.scalar.activation(out=musq[:], in_=musq[:], func=AF.Sqrt,
                                     bias=eps_t[:], scale=1.0)
                scr = stp.tile([128, L], F32, tag="scr")
                nc.vector.reciprocal_approx_accurate(rs_out, musq[:], scr[:])
                if mu_out is not None:
                    nc.vector.tensor_copy(mu_out, ps_mu[:])

            def scaled(x_b, rs):
                xs = xsp.tile([128, DC, L], BF16, tag="xs")
                nc.vector.tensor_mul(
                    xs[:], x_b[:], rs.unsqueeze(1).broadcast_to([128, DC, L])
                )
                return xs

            for i in range(NLB):
                wq_sb = wp.tile([128, DC, D], BF16, tag="wq")
                nc.sync.dma_start(wq_sb[:], wq[i].rearrange("(c p) o -> p c o", p=128))
                wk_sb = wp.tile([128, DC, D], BF16, tag="wk")
                nc.sync.dma_start(wk_sb[:], wk[i].rearrange("(c p) o -> p c o", p=128))
                wv_sb = wp.tile([128, DC, D], BF16, tag="wv")
                nc.sync.dma_start(wv_sb[:], wv[i].rearrange("(c p) o -> p c o", p=128))
                wo_sb = wp.tile([128, DC, D], BF16, tag="wo")
                nc.sync.dma_start(wo_sb[:], wo[i].rearrange("(c p) o -> p c o", p=128))
                bqh_sb = wp.tile([HD, H], F32, tag="bqh")
                nc.sync.dma_start(bqh_sb[:], bqh[i])
                bkh_sb = wp.tile([HD, H], F32, tag="bkh")
                nc.sync.dma_start(bkh_sb[:], bkh[i])
                bvr_sb = wp.tile([1, D], F32R, tag="bvr")
                nc.sync.dma_start(bvr_sb[:], bvr[i].unsqueeze(0).bitcast(F32R))
                bor_sb = wp.tile([1, D], F32R, tag="bor")
                nc.sync.dma_start(bor_sb[:], bor[i].unsqueeze(0).bitcast(F32R))
                b2r_sb = wp.tile([1, D], F32R, tag="b2r")
                nc.sync.dma_start(b2r_sb[:], b2r[i].unsqueeze(0).bitcast(F32R))
                b1h_sb = wp.tile([128, FC], F32, tag="b1h")
                nc.sync.dma_start(b1h_sb[:], b1h[i])

                # ---------------- attention phase (all b) ----------------
                for b in range(BLOC):
                    x_b = load_x(b, i)
                    rs1 = stp.tile([128, L], F32, tag="rs1")
                    ln_stats(x_b, rs1[:])
                    xs = scaled(x_b, rs1[:])
                    if DEBUG and i == 0 and b == 0:
                        nc.sync.dma_start(dbg["d_xs"], xs[:])

                    # v projection, token-major, + ones column for denominators
                    vext = vep.tile([128, DC, H, HD + 1], F32R, tag="vext")
                    nc.vector.tensor_copy(
                        vext[:, :, :, HD:HD + 1],
                        ones_col[:].unsqueeze(1).unsqueeze(1).broadcast_to([128, DC, H, 1]),
                    )
                    for t in range(DC):
                        ps_v = ps_a.tile([128, D], F32, tag="seq")
                        for dc in range(DC):
                            nc.tensor.matmul(
                                ps_v[:], xs[:, dc, t * 128:(t + 1) * 128],
                                wv_sb[:, dc, :], start=(dc == 0), stop=False,
                            )
                        nc.tensor.matmul(ps_v[:], ones_row[0:1, 0:128], bvr_sb[:],
                                         start=False, stop=True)
                        nc.vector.tensor_copy(
                            vext[:, t, :, 0:HD],
                            ps_v[:].rearrange("p (h e) -> p h e", h=H),
                        )
                    if DEBUG and i == 0 and b == 0:
                        nc.sync.dma_start(dbg["d_v"], vext[:])

                    for pair in range(NP):
                        h0, h1 = 2 * pair, 2 * pair + 1
                        ps_q = ps_a.tile([128, L], F32, tag="seq")
                        ps_k = ps_a.tile([128, L], F32, tag="seq")
                        for dc in range(DC):
                            nc.tensor.matmul(
                                ps_q[:], wq_sb[:, dc, pair * 128:(pair + 1) * 128],
                                xs[:, dc, :], start=(dc == 0), stop=(dc == DC - 1),
                            )
                        for dc in range(DC):
                            nc.tensor.matmul(
                                ps_k[:], wk_sb[:, dc, pair * 128:(pair + 1) * 128],
                                xs[:, dc, :], start=(dc == 0), stop=(dc == DC - 1),
                            )
                        qx, kx = {}, {}
                        for hh in (h0, h1):
                            off = 64 * (hh % 2)
                            qx[hh] = qkp.tile([80, L], F32R, tag="qx", name=f"qx{hh}")
                            nc.vector.tensor_scalar_add(
                                out=qx[hh][0:HD, :], in0=ps_q[off:off + HD, :],
                                scalar1=bqh_sb[:, hh:hh + 1],
                            )
                            nc.sync.dma_start(qx[hh][HD:HD + 10, :], qe[b].bitcast(F32R))
                            kx[hh] = qkp.tile([80, L], F32R, tag="kx", name=f"kx{hh}")
                            nc.vector.tensor_scalar_add(
                                out=kx[hh][0:HD, :], in0=ps_k[off:off + HD, :],
                                scalar1=bkh_sb[:, hh:hh + 1],
                            )
                            nc.sync.dma_start(kx[hh][HD:HD + 10, :], ke[i, hh, b].bitcast(F32R))
                        if DEBUG and i == 0 and b == 0 and pair == 0:
                            nc.sync.dma_start(dbg["d_qx"], qx[h0][:])
                            nc.sync.dma_start(dbg["d_kx"], kx[h0][:])

                        attnT = atp.tile([128, L], BF16, tag="attnT")
                        for hh in (h0, h1):
                            wts = []
                            ps_o = ps_av.tile([HD + 1, L], F32, tag="av")
                            for cs in range(DC):
                                n0 = cs * 128
                                ps_s = ps_a.tile([128, L], F32, tag="seq")
                                nc.tensor.matmul(
                                    ps_s[:, 0:L - n0],
                                    kx[hh][0:74, n0:n0 + 128],
                                    qx[hh][0:74, n0:L],
                                    start=True, stop=True,
                                )
                                nc.vector.tensor_add(ps_s[:, 0:128], ps_s[:, 0:128],
                                                     tri_sb[:])
                                if DEBUG and i == 0 and b == 0 and hh == 0:
                                    _scd = stp.tile([128, L], F32, tag="scd", name=f"scd{cs}")
                                    nc.vector.tensor_copy(_scd[:, 0:L - n0], ps_s[:, 0:L - n0])
                                    nc.sync.dma_start(dbg["d_sc"][cs, :, 0:L - n0], _scd[:, 0:L - n0])
                                wt = wtp.tile([128, L], F32R, tag="wt", name=f"wt{cs}")
                                nc.scalar.activation(
                                    out=wt[:, 0:L - n0], in_=ps_s[:, 0:L - n0],
                                    func=AF.Exp, scale=1.0,
                                )
                                if DEBUG and i == 0 and b == 0 and hh == 0:
                                    nc.sync.dma_start(dbg["d_wt"][cs], wt[:])
                                wts.append(wt)
                            for cs in range(DC):
                                n0 = cs * 128
                                nc.tensor.matmul(
                                    ps_o[:, n0:L], vext[:, cs, hh, :],
                                    wts[cs][:, 0:L - n0],
                                    start=(cs == 0), stop=(cs == DC - 1),
                                )
                            if DEBUG and i == 0 and b == 0 and hh == 0:
                                _pod = atp.tile([HD + 1, L], F32, tag="pod")
                                nc.vector.tensor_copy(_pod[:], ps_o[:])
                                nc.sync.dma_start(dbg["d_po"], _pod[:])
                            den = stp.tile([1, L], F32, tag="den")
                            nc.vector.tensor_copy(den[:], ps_o[HD:HD + 1, :])
                            rcp = stp.tile([1, L], F32, tag="rcp")
                            rcs = stp.tile([1, L], F32, tag="rcs")
                            nc.vector.reciprocal_approx_accurate(
                                rcp[:], den[:], rcs[:]
                            )
                            rb = stp.tile([HD, L], F32, tag="rb")
                            nc.gpsimd.partition_broadcast(rb[:], rcp[:])
                            if DEBUG and i == 0 and b == 0 and hh == 0:
                                nc.sync.dma_start(dbg["d_rb"], rb[:])
                            off = 64 * (hh % 2)
                            nc.vector.tensor_mul(attnT[off:off + HD, :],
                                                 ps_o[0:HD, :], rb[:])
                        if DEBUG and i == 0 and b == 0 and pair == 0:
                            nc.sync.dma_start(dbg["d_at"], attnT[:])

                        for oc in range(DC):
                            if pair == 0:
                                ps_x = ps_acc.tile([128, L], F32, tag="acc", name=f"psx{oc}")
                                if oc == 0:
                                    ps_xs = []
                                ps_xs.append(ps_x)
                            nc.tensor.matmul(
                                ps_xs[oc][:], wo_sb[:, pair, oc * 128:(oc + 1) * 128],
                                attnT[:], start=(pair == 0), stop=False,
                            )
                    for oc in range(DC):
                        nc.tensor.matmul(
                            ps_xs[oc][:], bor_sb[0:1, oc * 128:(oc + 1) * 128],
                            ones_row[:], start=False, stop=True,
                        )
                        nc.vector.tensor_add(x_b[:, oc, :], ps_xs[oc][:], x_b[:, oc, :])

                    if DEBUG and i == 0 and b == 0:
                        nc.sync.dma_start(dbg["d_x1"], x_b[:].bitcast(F32))
                    # LN2 stats on post-attention x (rs kept for both ffn halves)
                    rsb = rs4p.tile([128, L], F32, tag="rsb")
                    ln_stats(x_b, rsb[:])
                    if b == 0:
                        rs_list = []
                    rs_list.append(rsb)
                    nc.sync.dma_start(xa[b], x_b[:].bitcast(F32))

                # ---------------- FFN phase (two streamed halves) ----------------
                for half in range(2):
                    w1h = wf.tile([128, DC, FH * 128], BF16, tag="w1h")
                    nc.sync.dma_start(
                        w1h[:],
                        w1[i, :, half * FH * 128:(half + 1) * FH * 128]
                        .rearrange("(c p) o -> p c o", p=128),
                    )
                    w2h = wf.tile([128, FH, D], BF16, tag="w2h")
                    nc.sync.dma_start(
                        w2h[:],
                        w2[i, half * FH * 128:(half + 1) * FH * 128, :]
                        .rearrange("(c p) o -> p c o", p=128),
                    )
                    for b in range(BLOC):
                        x_b = load_x(b, 1, src=xa)
                        xs2 = scaled(x_b, rs_list[b][:])
                        if half == 1:
                            x_b = load_x(b, 1, src=xw)
                        ps_f = [ps_acc.tile([128, L], F32, tag="acc", name=f"psf{_oc}")
                                for _oc in range(DC)]
                        for fc in range(FH):
                            gfc = half * FH + fc
                            ps_h = ps_a.tile([128, L], F32, tag="seq")
                            for dc in range(DC):
                                nc.tensor.matmul(
                                    ps_h[:], w1h[:, dc, fc * 128:(fc + 1) * 128],
                                    xs2[:, dc, :], start=(dc == 0), stop=(dc == DC - 1),
                                )
                            h1g = h1p.tile([128, L], BF16, tag="h1g")
                            nc.scalar.activation(
                                out=h1g[:], in_=ps_h[:], func=AF.Gelu,
                                bias=b1h_sb[:, gfc:gfc + 1], scale=1.0,
                            )
                            for oc in range(DC):
                                nc.tensor.matmul(
                                    ps_f[oc][:], w2h[:, fc, oc * 128:(oc + 1) * 128],
                                    h1g[:], start=(fc == 0),
                                    stop=(half == 0 and fc == FH - 1),
                                )
                        for oc in range(DC):
                            if half == 1:
                                nc.tensor.matmul(
                                    ps_f[oc][:],
                                    b2r_sb[0:1, oc * 128:(oc + 1) * 128],
                                    ones_row[:], start=False, stop=True,
                                )
                            nc.vector.tensor_add(x_b[:, oc, :], ps_f[oc][:],
                                                 x_b[:, oc, :])
                        if DEBUG and i == 0 and b == 0 and half == 1:
                            nc.sync.dma_start(dbg["d_x2"], x_b[:].bitcast(F32))
                        nc.sync.dma_start(xw[b], x_b[:].bitcast(F32))

            # ---------------- final layernorm + transpose ----------------
            for b in range(BLOC):
                x_b = load_x(b, NLB)
                rs1 = stp.tile([128, L], F32, tag="rs1")
                mu = stp.tile([128, L], F32, tag="mu")
                ln_stats(x_b, rs1[:], mu_out=mu[:])
                xc = xsp.tile([128, DC, L], F32, tag="xs")
                nc.vector.tensor_sub(
                    xc[:], x_b[:], mu[:].unsqueeze(1).broadcast_to([128, DC, L])
                )
                xf = sqp.tile([128, DC, L], F32, tag="xsq")
                nc.vector.tensor_mul(
                    xf[:], xc[:], rs1[:].unsqueeze(1).broadcast_to([128, DC, L])
                )
                for dc in range(DC):
                    nc.vector.tensor_scalar(
                        out=xf[:, dc, :], in0=xf[:, dc, :],
                        scalar1=fng_sb[:, dc:dc + 1], scalar2=fnb_sb[:, dc:dc + 1],
                        op0=AL.mult, op1=AL.add,
                    )
                for t in range(DC):
                    o_sb = osb.tile([128, D], F32, tag="osb")
                    for dc in range(DC):
                        ps_t = ps_a.tile([128, 128], F32, tag="seq")
                        nc.tensor.transpose(
                            ps_t[:], xf[:, dc, t * 128:(t + 1) * 128], ident[:]
                        )
                        nc.vector.tensor_copy(o_sb[:, dc * 128:(dc + 1) * 128], ps_t[:])
                    nc.sync.dma_start(out[b, t * 128:(t + 1) * 128, :], o_sb[:])

    nc.compile()
    return nc


def _center_cols(W):
    return W - W.mean(axis=0, keepdims=True)


def _preprocess(inputs):
    """Host-side folding; returns per-core in_maps."""
    f32 = np.float32
    g = {k: np.asarray(v) for k, v in inputs.items()}
    Wq, Wk, Wv, Wo = g["Wq"], g["Wk"], g["Wv"], g["Wo"]
    W1, W2 = g["W1"], g["W2"]
    g1, b1n = g["ln1_g"], g["ln1_b"]
    g2, b2n = g["ln2_g"], g["ln2_b"]

    wq_e = np.stack([_center_cols(g1[i][:, None] * Wq[i]) / SCALE for i in range(NL)]).astype(f32)
    bq_e = np.stack([(g["bq"][i] + b1n[i] @ Wq[i]) / SCALE for i in range(NL)]).astype(f32)
    wk_e = np.stack([_center_cols(g1[i][:, None] * Wk[i]) for i in range(NL)]).astype(f32)
    bk_e = np.stack([g["bk"][i] + b1n[i] @ Wk[i] for i in range(NL)]).astype(f32)
    wv_e = np.stack([_center_cols(g1[i][:, None] * Wv[i]) for i in range(NL)]).astype(f32)
    bv_e = np.stack([g["bv"][i] + b1n[i] @ Wv[i] for i in range(NL)]).astype(f32)
    w1_e = np.stack([_center_cols(g2[i][:, None] * W1[i]) for i in range(NL)]).astype(f32)
    b1_e = np.stack([g["b1"][i] + b2n[i] @ W1[i] for i in range(NL)]).astype(f32)

    ci = g["case_ids"].astype(np.int64)
    am = g["attention_mask"].astype(f32)
    verb = (ci == 8).astype(f32)
    qe = np.zeros((B, 10, L), f32)
    for c in range(C):
        qe[:, c, :] = (ci == c)
    qe[:, 9, :] = 1.0
    cb = g["case_bias"].astype(f32)
    vb = g["verb_bias"].astype(f32)
    ke = np.zeros((NL, H, B, 10, L), f32)
    for i in range(NL):
        for h in range(H):
            ke[i, h, :, 0:C, :] = np.transpose(cb[i, h][:, ci], (1, 0, 2))
            ke[i, h, :, 9, :] = vb[i, h] * verb - 10000.0 * (1.0 - am)

    tri = np.where(
        np.arange(128)[:, None] > np.arange(128)[None, :], f32(-10000.0), f32(0.0)
    ).astype(f32)

    import ml_dtypes
    bf16 = ml_dtypes.bfloat16
    common = {
        "wq": wq_e[:NLB].astype(bf16), "wk": wk_e[:NLB].astype(bf16),
        "wv": wv_e[:NLB].astype(bf16),
        "wo": np.ascontiguousarray(Wo.astype(f32)[:NLB]).astype(bf16),
        "w1": w1_e[:NLB].astype(bf16),
        "w2": np.ascontiguousarray(W2.astype(f32)[:NLB]).astype(bf16),
        "bqh": np.ascontiguousarray(bq_e.reshape(NL, H, HD).transpose(0, 2, 1))[:NLB],
        "bkh": np.ascontiguousarray(bk_e.reshape(NL, H, HD).transpose(0, 2, 1))[:NLB],
        "bvr": bv_e[:NLB], "bor": np.ascontiguousarray(g["bo"].astype(f32)[:NLB]),
        "b1h": np.ascontiguousarray(b1_e.reshape(NL, FC, 128).transpose(0, 2, 1))[:NLB],
        "b2r": np.ascontiguousarray(g["b2"].astype(f32)[:NLB]),
        "tri": tri,
        "fng": np.ascontiguousarray(g["fn_g"].astype(f32).reshape(DC, 128).T),
        "fnb": np.ascontiguousarray(g["fn_b"].astype(f32).reshape(DC, 128).T),
    }
    x = g["x"].astype(f32)
    in_maps = []
    for core in range(NCORES):
        sl = slice(core * BLOC, (core + 1) * BLOC)
        m = dict(common)
        m["xT"] = np.ascontiguousarray(x[sl].reshape(BLOC * L, D).T)
        m["qe"] = np.ascontiguousarray(qe[sl])
        m["ke"] = np.ascontiguousarray(ke[:NLB, :, sl])
        in_maps.append(m)
    return in_maps


def _get_program():
    global _PROGRAM
    if _PROGRAM is None:
        _PROGRAM = _build_program()
    return _PROGRAM


def kernel(**inputs) -> np.ndarray:
    from concourse.bass_utils import run_bass_kernel_spmd

    nc = _get_program()
    in_maps = _preprocess(inputs)
    res = run_bass_kernel_spmd(nc, in_maps, list(range(NCORES)))
    return np.concatenate(
        [res.results[c]["out"] for c in range(NCORES)], axis=0
    ).astype(np.float32)


# revision 25
# speedup vs baseline: 1.0736x; 1.0226x over previous
"""BrahmanTransformer Trainium2 kernel.

Sharding: data-parallel over batch (32 -> 4 sequences per core x 8 cores),
full 6-layer transformer per core in one Bass/Tile program; float32r matmuls.

Device dataflow (per core, d-major residual):
- LayerNorm: gamma/beta folded into weights host-side; mean subtraction folded
  via column-centered weights; on-device LN is xs = x * rsqrt(var+eps) only.
- Attention: case one-hot (9) + const row (1) appended to q, gathered
  case-bias columns + verb/pad row appended to k -> single K=74 score matmul.
  scoresT layout [key, query]; causality via partial-width matmuls; softmax
  without max-subtraction (scores are O(1)); denominators via a ones column
  appended to V (token-major) and the AV matmul.
- FFN: fc-halves streamed from HBM; psum-accumulated ffn2; gelu on ACT.
- All biases exact: ACT-copy bias (q/k), K=1 matmuls (v/o/ffn2), gelu bias.
"""
import os
import numpy as np

B, L, D, H, NL, F, C = 32, 512, 512, 8, 6, 2048, 9
HD = D // H
NCORES = 8
BLOC = B // NCORES            # 4 sequences per core
SCALE = float(np.sqrt(HD))
EPS = 1e-5
NLB = int(os.environ.get("KB_LAYERS", str(NL)))  # build-depth (debug aid)
DEBUG = bool(int(os.environ.get("KB_DEBUG", "0")))

DC = D // 128     # 4 d-chunks
FC = F // 128     # 16 f-chunks
FH = FC // 2      # 8 f-chunks per streamed half
NP = H // 2       # 4 head pairs

_PROGRAM = None


def _build_program():
    import concourse.bacc as bacc
    import concourse.mybir as mybir
    import concourse.tile as tile
    from concourse.masks import make_identity

    F32 = mybir.dt.float32
    F32R = mybir.dt.float32r
    AF = mybir.ActivationFunctionType
    AL = mybir.AluOpType

    nc = bacc.Bacc("TRN2", target_bir_lowering=False, debug=False)

    BF16 = mybir.dt.bfloat16

    def inp(name, shape, dt=F32):
        return nc.dram_tensor(name, shape, dt, kind="ExternalInput").ap()

    xT = inp("xT", [D, BLOC * L])
    wq = inp("wq", [NLB, D, D], BF16)
    wk = inp("wk", [NLB, D, D], BF16)
    wv = inp("wv", [NLB, D, D], BF16)
    wo = inp("wo", [NLB, D, D], BF16)
    w1 = inp("w1", [NLB, D, F], BF16)
    w2 = inp("w2", [NLB, F, D], BF16)
    bqh = inp("bqh", [NLB, HD, H])
    bkh = inp("bkh", [NLB, HD, H])
    bvr = inp("bvr", [NLB, D])
    bor = inp("bor", [NLB, D])
    b1h = inp("b1h", [NLB, 128, FC])
    b2r = inp("b2r", [NLB, D])
    qe = inp("qe", [BLOC, 10, L])
    ke = inp("ke", [NLB, H, BLOC, 10, L])
    tri = inp("tri", [128, 128])
    fng = inp("fng", [128, DC])
    fnb = inp("fnb", [128, DC])
    out = nc.dram_tensor("out", [BLOC, L, D], F32, kind="ExternalOutput").ap()
    dbg = {}
    if DEBUG:
        BF16_ = mybir.dt.bfloat16
        for nm, shp, dt_ in [
            ("d_xs", [128, DC, L], BF16_), ("d_v", [128, DC, H, HD + 1], BF16_),
            ("d_qx", [80, L], BF16_), ("d_kx", [80, L], BF16_),
            ("d_wt", [DC, 128, L], BF16_),
            ("d_at", [128, L], BF16_), ("d_x1", [128, DC, L], F32),
            ("d_x2", [128, DC, L], F32),
            ("d_sc", [DC, 128, L], F32), ("d_po", [HD + 1, L], F32),
            ("d_rb", [HD, L], F32),
        ]:
            dbg[nm] = nc.dram_tensor(nm, shp, dt_, kind="ExternalOutput").ap()

    xT_r = xT.rearrange("(dc p) t -> p dc t", p=128)

    from contextlib import ExitStack

    with tile.TileContext(nc) as tc:
        with ExitStack() as _st:
            _e = _st.enter_context
            cp = _e(tc.tile_pool(name="const", bufs=1))
            wp = _e(tc.tile_pool(name="wpool", bufs=1))
            wf = _e(tc.tile_pool(name="wff", bufs=2))
            xbp = _e(tc.tile_pool(name="xb", bufs=2))
            sqp = _e(tc.tile_pool(name="sq", bufs=1))
            xsp = _e(tc.tile_pool(name="xsp", bufs=2))
            stp = _e(tc.tile_pool(name="stats", bufs=2))
            st1 = _e(tc.tile_pool(name="stats1", bufs=1))
            rs4p = _e(tc.tile_pool(name="rs4", bufs=4))
            qkp = _e(tc.tile_pool(name="qk", bufs=4))
            vep = _e(tc.tile_pool(name="vex", bufs=1))
            wtp = _e(tc.tile_pool(name="wtp", bufs=4))
            atp = _e(tc.tile_pool(name="atn", bufs=2))
            h1p = _e(tc.tile_pool(name="h1g", bufs=3))
            osb = _e(tc.tile_pool(name="osb", bufs=2))
            dram = _e(tc.tile_pool(name="dram", bufs=1, space="DRAM"))
            ps_a = _e(tc.tile_pool(name="ps_a", bufs=3, space="PSUM"))
            ps_av = _e(tc.tile_pool(name="ps_av", bufs=1, space="PSUM"))
            ps_acc = _e(tc.tile_pool(name="ps_acc", bufs=4, space="PSUM"))
            # ---- constants
            onesf = cp.tile([1, L], F32, tag="onesf")
            nc.vector.memset(onesf[:], 1.0)
            ones_row = cp.tile([1, L], F32R, tag="ones_row")
            nc.vector.tensor_copy(ones_row[:], onesf[:])
            redf = cp.tile([128, 128], F32, tag="redf")
            nc.vector.memset(redf[:], 1.0 / D)
            red = cp.tile([128, 128], F32R, tag="red")
            nc.vector.tensor_copy(red[:], redf[:])
            eps_t = cp.tile([128, 1], F32, tag="eps")
            nc.vector.memset(eps_t[:], EPS)
            ones_col = cp.tile([128, 1], F32, tag="ones_col")
            nc.vector.memset(ones_col[:], 1.0)
            ident = cp.tile([128, 128], F32, tag="ident")
            make_identity(nc, ident[:])
            tri_sb = cp.tile([128, 128], F32, tag="tri")
            nc.sync.dma_start(tri_sb[:], tri)
            fng_sb = cp.tile([128, DC], F32, tag="fng")
            nc.sync.dma_start(fng_sb[:], fng)
            fnb_sb = cp.tile([128, DC], F32, tag="fnb")
            nc.sync.dma_start(fnb_sb[:], fnb)

            # residual stream working copies in DRAM:
            # xw = running residual; xa = post-attention snapshot (LN2 input)
            xw = dram.tile([BLOC, 128, DC, L], F32, tag="xw")
            xa = dram.tile([BLOC, 128, DC, L], F32, tag="xa")

            def load_x(b, layer, src=None):
                x_b = xbp.tile([128, DC, L], F32R, tag="xb")
                if src is None and layer == 0:
                    nc.sync.dma_start(
                        x_b[:], xT_r[:, :, b * L:(b + 1) * L].bitcast(F32R)
                    )
                else:
                    nc.sync.dma_start(x_b[:], (xw if src is None else src)[b].bitcast(F32R))
                return x_b

            def ln_stats(x_b, rs_out, mu_out=None):
                """rs_out[:] = rsqrt(var(x)+eps) (+ mean into mu_out)."""
                xsq = sqp.tile([128, DC, L], F32R, tag="xsq")
                nc.scalar.activation(out=xsq[:], in_=x_b[:], func=AF.Square, scale=1.0)
                ps_mu = ps_a.tile([128, L], F32, tag="seq")
                ps_sq = ps_a.tile([128, L], F32, tag="seq")
                for dc in range(DC):
                    nc.tensor.matmul(ps_mu[:], red[:], x_b[:, dc, :],
                                     start=(dc == 0), stop=(dc == DC - 1))
                for dc in range(DC):
                    nc.tensor.matmul(ps_sq[:], red[:], xsq[:, dc, :],
                                     start=(dc == 0), stop=(dc == DC - 1))
                musq = stp.tile([128, L], F32, tag="musq")
                nc.scalar.activation(out=musq[:], in_=ps_mu[:], func=AF.Square, scale=1.0)
                nc.vector.tensor_sub(musq[:], ps_sq[:], musq[:])
                nc.scalar.activation(out=musq[:], in_=musq[:], func=AF.Sqrt,
                                     bias=eps_t[:], scale=1.0)
                scr = stp.tile([128, L], F32, tag="scr")
                nc.vector.reciprocal_approx_accurate(rs_out, musq[:], scr[:])
                if mu_out is not None:
                    nc.vector.tensor_copy(mu_out, ps_mu[:])

            def scaled(x_b, rs):
                xs = xsp.tile([128, DC, L], BF16, tag="xs")
                nc.vector.tensor_mul(
                    xs[:], x_b[:], rs.unsqueeze(1).broadcast_to([128, DC, L])
                )
                return xs

            for i in range(NLB):
                wq_sb = wp.tile([128, DC, D], BF16, tag="wq")
                nc.sync.dma_start(wq_sb[:], wq[i].rearrange("(c p) o -> p c o", p=128))
                wk_sb = wp.tile([128, DC, D], BF16, tag="wk")
                nc.sync.dma_start(wk_sb[:], wk[i].rearrange("(c p) o -> p c o", p=128))
                wv_sb = wp.tile([128, DC, D], BF16, tag="wv")
                nc.sync.dma_start(wv_sb[:], wv[i].rearrange("(c p) o -> p c o", p=128))
                wo_sb = wp.tile([128, DC, D], BF16, tag="wo")
                nc.sync.dma_start(wo_sb[:], wo[i].rearrange("(c p) o -> p c o", p=128))
                bqh_sb = wp.tile([HD, H], F32, tag="bqh")
                nc.sync.dma_start(bqh_sb[:], bqh[i])
                bkh_sb = wp.tile([HD, H], F32, tag="bkh")
                nc.sync.dma_start(bkh_sb[:], bkh[i])
                bvr_sb = wp.tile([1, D], F32R, tag="bvr")
                nc.sync.dma_start(bvr_sb[:], bvr[i].unsqueeze(0).bitcast(F32R))
                bor_sb = wp.tile([1, D], F32R, tag="bor")
                nc.sync.dma_start(bor_sb[:], bor[i].unsqueeze(0).bitcast(F32R))
                b2r_sb = wp.tile([1, D], F32R, tag="b2r")
                nc.sync.dma_start(b2r_sb[:], b2r[i].unsqueeze(0).bitcast(F32R))
                b1h_sb = wp.tile([128, FC], F32, tag="b1h")
                nc.sync.dma_start(b1h_sb[:], b1h[i])

                # ---------------- attention phase (all b) ----------------
                for b in range(BLOC):
                    x_b = load_x(b, i)
                    rs1 = stp.tile([128, L], F32, tag="rs1")
                    ln_stats(x_b, rs1[:])
                    xs = scaled(x_b, rs1[:])
                    if DEBUG and i == 0 and b == 0:
                        nc.sync.dma_start(dbg["d_xs"], xs[:])

                    # v projection, token-major, + ones column for denominators
                    vext = vep.tile([128, DC, H, HD + 1], F32R, tag="vext")
                    nc.vector.tensor_copy(
                        vext[:, :, :, HD:HD + 1],
                        ones_col[:].unsqueeze(1).unsqueeze(1).broadcast_to([128, DC, H, 1]),
                    )
                    for t in range(DC):
                        ps_v = ps_a.tile([128, D], F32, tag="seq")
                        for dc in range(DC):
                            nc.tensor.matmul(
                                ps_v[:], xs[:, dc, t * 128:(t + 1) * 128],
                                wv_sb[:, dc, :], start=(dc == 0), stop=False,
                            )
                        nc.tensor.matmul(ps_v[:], ones_row[0:1, 0:128], bvr_sb[:],
                                         start=False, stop=True)
                        nc.vector.tensor_copy(
                            vext[:, t, :, 0:HD],
                            ps_v[:].rearrange("p (h e) -> p h e", h=H),
                        )
                    if DEBUG and i == 0 and b == 0:
                        nc.sync.dma_start(dbg["d_v"], vext[:])

                    for pair in range(NP):
                        h0, h1 = 2 * pair, 2 * pair + 1
                        ps_q = ps_a.tile([128, L], F32, tag="seq")
                        ps_k = ps_a.tile([128, L], F32, tag="seq")
                        for dc in range(DC):
                            nc.tensor.matmul(
                                ps_q[:], wq_sb[:, dc, pair * 128:(pair + 1) * 128],
                                xs[:, dc, :], start=(dc == 0), stop=(dc == DC - 1),
                            )
                        for dc in range(DC):
                            nc.tensor.matmul(
                                ps_k[:], wk_sb[:, dc, pair * 128:(pair + 1) * 128],
                                xs[:, dc, :], start=(dc == 0), stop=(dc == DC - 1),
                            )
                        qx, kx = {}, {}
                        for hh in (h0, h1):
                            off = 64 * (hh % 2)
                            qx[hh] = qkp.tile([80, L], F32R, tag="qx", name=f"qx{hh}")
                            nc.vector.tensor_scalar_add(
                                out=qx[hh][0:HD, :], in0=ps_q[off:off + HD, :],
                                scalar1=bqh_sb[:, hh:hh + 1],
                            )
                            nc.sync.dma_start(qx[hh][HD:HD + 10, :], qe[b].bitcast(F32R))
                            kx[hh] = qkp.tile([80, L], F32R, tag="kx", name=f"kx{hh}")
                            nc.vector.tensor_scalar_add(
                                out=kx[hh][0:HD, :], in0=ps_k[off:off + HD, :],
                                scalar1=bkh_sb[:, hh:hh + 1],
                            )
                            nc.sync.dma_start(kx[hh][HD:HD + 10, :], ke[i, hh, b].bitcast(F32R))
                        if DEBUG and i == 0 and b == 0 and pair == 0:
                            nc.sync.dma_start(dbg["d_qx"], qx[h0][:])
                            nc.sync.dma_start(dbg["d_kx"], kx[h0][:])

                        attnT = atp.tile([128, L], BF16, tag="attnT")
                        for hh in (h0, h1):
                            wts = []
                            ps_o = ps_av.tile([HD + 1, L], F32, tag="av")
                            for cs in range(DC):
                                n0 = cs * 128
                                ps_s = ps_a.tile([128, L], F32, tag="seq")
                                nc.tensor.matmul(
                                    ps_s[:, 0:L - n0],
                                    kx[hh][0:74, n0:n0 + 128],
                                    qx[hh][0:74, n0:L],
                                    start=True, stop=True,
                                )
                                nc.vector.tensor_add(ps_s[:, 0:128], ps_s[:, 0:128],
                                                     tri_sb[:])
                                if DEBUG and i == 0 and b == 0 and hh == 0:
                                    _scd = stp.tile([128, L], F32, tag="scd", name=f"scd{cs}")
                                    nc.vector.tensor_copy(_scd[:, 0:L - n0], ps_s[:, 0:L - n0])
                                    nc.sync.dma_start(dbg["d_sc"][cs, :, 0:L - n0], _scd[:, 0:L - n0])
                                wt = wtp.tile([128, L], F32R, tag="wt", name=f"wt{cs}")
                                nc.scalar.activation(
                                    out=wt[:, 0:L - n0], in_=ps_s[:, 0:L - n0],
                                    func=AF.Exp, scale=1.0,
                                )
                                if DEBUG and i == 0 and b == 0 and hh == 0:
                                    nc.sync.dma_start(dbg["d_wt"][cs], wt[:])
                                wts.append(wt)
                            for cs in range(DC):
                                n0 = cs * 128
                                nc.tensor.matmul(
                                    ps_o[:, n0:L], vext[:, cs, hh, :],
                                    wts[cs][:, 0:L - n0],
                                    start=(cs == 0), stop=(cs == DC - 1),
                                )
                            if DEBUG and i == 0 and b == 0 and hh == 0:
                                _pod = atp.tile([HD + 1, L], F32, tag="pod")
                                nc.vector.tensor_copy(_pod[:], ps_o[:])
                                nc.sync.dma_start(dbg["d_po"], _pod[:])
                            den = st1.tile([1, L], F32, tag="den")
                            nc.vector.tensor_copy(den[:], ps_o[HD:HD + 1, :])
                            rcp = st1.tile([1, L], F32, tag="rcp")
                            rcs = st1.tile([1, L], F32, tag="rcs")
                            nc.vector.reciprocal_approx_accurate(
                                rcp[:], den[:], rcs[:]
                            )
                            rb = stp.tile([HD, L], F32, tag="rb")
                            nc.gpsimd.partition_broadcast(rb[:], rcp[:])
                            if DEBUG and i == 0 and b == 0 and hh == 0:
                                nc.sync.dma_start(dbg["d_rb"], rb[:])
                            off = 64 * (hh % 2)
                            nc.vector.tensor_mul(attnT[off:off + HD, :],
                                                 ps_o[0:HD, :], rb[:])
                        if DEBUG and i == 0 and b == 0 and pair == 0:
                            nc.sync.dma_start(dbg["d_at"], attnT[:])

                        for oc in range(DC):
                            if pair == 0:
                                ps_x = ps_acc.tile([128, L], F32, tag="acc", name=f"psx{oc}")
                                if oc == 0:
                                    ps_xs = []
                                ps_xs.append(ps_x)
                            nc.tensor.matmul(
                                ps_xs[oc][:], wo_sb[:, pair, oc * 128:(oc + 1) * 128],
                                attnT[:], start=(pair == 0), stop=False,
                            )
                    for oc in range(DC):
                        nc.tensor.matmul(
                            ps_xs[oc][:], bor_sb[0:1, oc * 128:(oc + 1) * 128],
                            ones_row[:], start=False, stop=True,
                        )
                        nc.vector.tensor_add(x_b[:, oc, :], ps_xs[oc][:], x_b[:, oc, :])

                    if DEBUG and i == 0 and b == 0:
                        nc.sync.dma_start(dbg["d_x1"], x_b[:].bitcast(F32))
                    # LN2 stats on post-attention x (rs kept for both ffn halves)
                    rsb = rs4p.tile([128, L], F32, tag="rsb")
                    ln_stats(x_b, rsb[:])
                    if b == 0:
                        rs_list = []
                    rs_list.append(rsb)
                    nc.sync.dma_start(xa[b], x_b[:].bitcast(F32))

                # ---------------- FFN phase (two streamed halves) ----------------
                for half in range(2):
                    w1h = wf.tile([128, DC, FH * 128], BF16, tag="w1h")
                    nc.sync.dma_start(
                        w1h[:],
                        w1[i, :, half * FH * 128:(half + 1) * FH * 128]
                        .rearrange("(c p) o -> p c o", p=128),
                    )
                    w2h = wf.tile([128, FH, D], BF16, tag="w2h")
                    nc.sync.dma_start(
                        w2h[:],
                        w2[i, half * FH * 128:(half + 1) * FH * 128, :]
                        .rearrange("(c p) o -> p c o", p=128),
                    )
                    for b in range(BLOC):
                        x_b = load_x(b, 1, src=xa)
                        xs2 = scaled(x_b, rs_list[b][:])
                        if half == 1:
                            x_b = load_x(b, 1, src=xw)
                        ps_f = [ps_acc.tile([128, L], F32, tag="acc", name=f"psf{_oc}")
                                for _oc in range(DC)]
                        for fc in range(FH):
                            gfc = half * FH + fc
                            ps_h = ps_a.tile([128, L], F32, tag="seq")
                            for dc in range(DC):
                                nc.tensor.matmul(
                                    ps_h[:], w1h[:, dc, fc * 128:(fc + 1) * 128],
                                    xs2[:, dc, :], start=(dc == 0), stop=(dc == DC - 1),
                                )
                            h1g = h1p.tile([128, L], BF16, tag="h1g")
                            nc.scalar.activation(
                                out=h1g[:], in_=ps_h[:], func=AF.Gelu,
                                bias=b1h_sb[:, gfc:gfc + 1], scale=1.0,
                            )
                            for oc in range(DC):
                                nc.tensor.matmul(
                                    ps_f[oc][:], w2h[:, fc, oc * 128:(oc + 1) * 128],
                                    h1g[:], start=(fc == 0),
                                    stop=(half == 0 and fc == FH - 1),
                                )
                        for oc in range(DC):
                            if half == 1:
                                nc.tensor.matmul(
                                    ps_f[oc][:],
                                    b2r_sb[0:1, oc * 128:(oc + 1) * 128],
                                    ones_row[:], start=False, stop=True,
                                )
                            nc.vector.tensor_add(x_b[:, oc, :], ps_f[oc][:],
                                                 x_b[:, oc, :])
                        if DEBUG and i == 0 and b == 0 and half == 1:
                            nc.sync.dma_start(dbg["d_x2"], x_b[:].bitcast(F32))
                        nc.sync.dma_start(xw[b], x_b[:].bitcast(F32))

            # ---------------- final layernorm + transpose ----------------
            for b in range(BLOC):
                x_b = load_x(b, NLB)
                rs1 = stp.tile([128, L], F32, tag="rs1")
                mu = stp.tile([128, L], F32, tag="mu")
                ln_stats(x_b, rs1[:], mu_out=mu[:])
                xc = xsp.tile([128, DC, L], F32, tag="xs")
                nc.vector.tensor_sub(
                    xc[:], x_b[:], mu[:].unsqueeze(1).broadcast_to([128, DC, L])
                )
                xf = sqp.tile([128, DC, L], F32, tag="xsq")
                nc.vector.tensor_mul(
                    xf[:], xc[:], rs1[:].unsqueeze(1).broadcast_to([128, DC, L])
                )
                for dc in range(DC):
                    nc.vector.tensor_scalar(
                        out=xf[:, dc, :], in0=xf[:, dc, :],
                        scalar1=fng_sb[:, dc:dc + 1], scalar2=fnb_sb[:, dc:dc + 1],
                        op0=AL.mult, op1=AL.add,
                    )
                for t in range(DC):
                    o_sb = osb.tile([128, D], F32, tag="osb")
                    for dc in range(DC):
                        ps_t = ps_a.tile([128, 128], F32, tag="seq")
                        nc.tensor.transpose(
                            ps_t[:], xf[:, dc, t * 128:(t + 1) * 128], ident[:]
                        )
                        nc.vector.tensor_copy(o_sb[:, dc * 128:(dc + 1) * 128], ps_t[:])
                    nc.sync.dma_start(out[b, t * 128:(t + 1) * 128, :], o_sb[:])

    nc.compile()
    return nc


def _center_cols(W):
    return W - W.mean(axis=0, keepdims=True)


def _preprocess(inputs):
    """Host-side folding; returns per-core in_maps."""
    f32 = np.float32
    g = {k: np.asarray(v) for k, v in inputs.items()}
    Wq, Wk, Wv, Wo = g["Wq"], g["Wk"], g["Wv"], g["Wo"]
    W1, W2 = g["W1"], g["W2"]
    g1, b1n = g["ln1_g"], g["ln1_b"]
    g2, b2n = g["ln2_g"], g["ln2_b"]

    wq_e = np.stack([_center_cols(g1[i][:, None] * Wq[i]) / SCALE for i in range(NL)]).astype(f32)
    bq_e = np.stack([(g["bq"][i] + b1n[i] @ Wq[i]) / SCALE for i in range(NL)]).astype(f32)
    wk_e = np.stack([_center_cols(g1[i][:, None] * Wk[i]) for i in range(NL)]).astype(f32)
    bk_e = np.stack([g["bk"][i] + b1n[i] @ Wk[i] for i in range(NL)]).astype(f32)
    wv_e = np.stack([_center_cols(g1[i][:, None] * Wv[i]) for i in range(NL)]).astype(f32)
    bv_e = np.stack([g["bv"][i] + b1n[i] @ Wv[i] for i in range(NL)]).astype(f32)
    w1_e = np.stack([_center_cols(g2[i][:, None] * W1[i]) for i in range(NL)]).astype(f32)
    b1_e = np.stack([g["b1"][i] + b2n[i] @ W1[i] for i in range(NL)]).astype(f32)

    ci = g["case_ids"].astype(np.int64)
    am = g["attention_mask"].astype(f32)
    verb = (ci == 8).astype(f32)
    qe = np.zeros((B, 10, L), f32)
    for c in range(C):
        qe[:, c, :] = (ci == c)
    qe[:, 9, :] = 1.0
    cb = g["case_bias"].astype(f32)
    vb = g["verb_bias"].astype(f32)
    ke = np.zeros((NL, H, B, 10, L), f32)
    for i in range(NL):
        for h in range(H):
            ke[i, h, :, 0:C, :] = np.transpose(cb[i, h][:, ci], (1, 0, 2))
            ke[i, h, :, 9, :] = vb[i, h] * verb - 10000.0 * (1.0 - am)

    tri = np.where(
        np.arange(128)[:, None] > np.arange(128)[None, :], f32(-10000.0), f32(0.0)
    ).astype(f32)

    import ml_dtypes
    bf16 = ml_dtypes.bfloat16
    common = {
        "wq": wq_e[:NLB].astype(bf16), "wk": wk_e[:NLB].astype(bf16),
        "wv": wv_e[:NLB].astype(bf16),
        "wo": np.ascontiguousarray(Wo.astype(f32)[:NLB]).astype(bf16),
        "w1": w1_e[:NLB].astype(bf16),
        "w2": np.ascontiguousarray(W2.astype(f32)[:NLB]).astype(bf16),
        "bqh": np.ascontiguousarray(bq_e.reshape(NL, H, HD).transpose(0, 2, 1))[:NLB],
        "bkh": np.ascontiguousarray(bk_e.reshape(NL, H, HD).transpose(0, 2, 1))[:NLB],
        "bvr": bv_e[:NLB], "bor": np.ascontiguousarray(g["bo"].astype(f32)[:NLB]),
        "b1h": np.ascontiguousarray(b1_e.reshape(NL, FC, 128).transpose(0, 2, 1))[:NLB],
        "b2r": np.ascontiguousarray(g["b2"].astype(f32)[:NLB]),
        "tri": tri,
        "fng": np.ascontiguousarray(g["fn_g"].astype(f32).reshape(DC, 128).T),
        "fnb": np.ascontiguousarray(g["fn_b"].astype(f32).reshape(DC, 128).T),
    }
    x = g["x"].astype(f32)
    in_maps = []
    for core in range(NCORES):
        sl = slice(core * BLOC, (core + 1) * BLOC)
        m = dict(common)
        m["xT"] = np.ascontiguousarray(x[sl].reshape(BLOC * L, D).T)
        m["qe"] = np.ascontiguousarray(qe[sl])
        m["ke"] = np.ascontiguousarray(ke[:NLB, :, sl])
        in_maps.append(m)
    return in_maps


def _get_program():
    global _PROGRAM
    if _PROGRAM is None:
        _PROGRAM = _build_program()
    return _PROGRAM


def kernel(**inputs) -> np.ndarray:
    from concourse.bass_utils import run_bass_kernel_spmd

    nc = _get_program()
    in_maps = _preprocess(inputs)
    res = run_bass_kernel_spmd(nc, in_maps, list(range(NCORES)))
    return np.concatenate(
        [res.results[c]["out"] for c in range(NCORES)], axis=0
    ).astype(np.float32)


# revision 26
# speedup vs baseline: 1.0798x; 1.0058x over previous
"""BrahmanTransformer Trainium2 kernel.

Sharding: data-parallel over batch (32 -> 4 sequences per core x 8 cores),
full 6-layer transformer per core in one Bass/Tile program; float32r matmuls.

Device dataflow (per core, d-major residual):
- LayerNorm: gamma/beta folded into weights host-side; mean subtraction folded
  via column-centered weights; on-device LN is xs = x * rsqrt(var+eps) only.
- Attention: case one-hot (9) + const row (1) appended to q, gathered
  case-bias columns + verb/pad row appended to k -> single K=74 score matmul.
  scoresT layout [key, query]; causality via partial-width matmuls; softmax
  without max-subtraction (scores are O(1)); denominators via a ones column
  appended to V (token-major) and the AV matmul.
- FFN: fc-halves streamed from HBM; psum-accumulated ffn2; gelu on ACT.
- All biases exact: ACT-copy bias (q/k), K=1 matmuls (v/o/ffn2), gelu bias.
"""
import os
import numpy as np

B, L, D, H, NL, F, C = 32, 512, 512, 8, 6, 2048, 9
HD = D // H
NCORES = 8
BLOC = B // NCORES            # 4 sequences per core
SCALE = float(np.sqrt(HD))
EPS = 1e-5
NLB = int(os.environ.get("KB_LAYERS", str(NL)))  # build-depth (debug aid)
DEBUG = bool(int(os.environ.get("KB_DEBUG", "0")))

DC = D // 128     # 4 d-chunks
FC = F // 128     # 16 f-chunks
FH = FC // 2      # 8 f-chunks per streamed half
NP = H // 2       # 4 head pairs

_PROGRAM = None


def _build_program():
    import concourse.bacc as bacc
    import concourse.mybir as mybir
    import concourse.tile as tile
    from concourse.masks import make_identity

    F32 = mybir.dt.float32
    F32R = mybir.dt.float32r
    AF = mybir.ActivationFunctionType
    AL = mybir.AluOpType

    nc = bacc.Bacc("TRN2", target_bir_lowering=False, debug=False)

    BF16 = mybir.dt.bfloat16

    def inp(name, shape, dt=F32):
        return nc.dram_tensor(name, shape, dt, kind="ExternalInput").ap()

    xT = inp("xT", [D, BLOC * L])
    wq = inp("wq", [NLB, D, D], BF16)
    wk = inp("wk", [NLB, D, D], BF16)
    wv = inp("wv", [NLB, D, D], BF16)
    wo = inp("wo", [NLB, D, D], BF16)
    w1 = inp("w1", [NLB, D, F], BF16)
    w2 = inp("w2", [NLB, F, D], BF16)
    bqh = inp("bqh", [NLB, HD, H])
    bkh = inp("bkh", [NLB, HD, H])
    bvr = inp("bvr", [NLB, D])
    bor = inp("bor", [NLB, D])
    b1h = inp("b1h", [NLB, 128, FC])
    b2r = inp("b2r", [NLB, D])
    qe = inp("qe", [BLOC, 10, L])
    ke = inp("ke", [NLB, H, BLOC, 10, L])
    tri = inp("tri", [128, 128])
    fng = inp("fng", [128, DC])
    fnb = inp("fnb", [128, DC])
    out = nc.dram_tensor("out", [BLOC, L, D], F32, kind="ExternalOutput").ap()
    dbg = {}
    if DEBUG:
        BF16_ = mybir.dt.bfloat16
        for nm, shp, dt_ in [
            ("d_xs", [128, DC, L], BF16_), ("d_v", [128, DC, H, HD + 1], BF16_),
            ("d_qx", [80, L], BF16_), ("d_kx", [80, L], BF16_),
            ("d_wt", [DC, 128, L], BF16_),
            ("d_at", [128, L], BF16_), ("d_x1", [128, DC, L], F32),
            ("d_x2", [128, DC, L], F32),
            ("d_sc", [DC, 128, L], F32), ("d_po", [HD + 1, L], F32),
            ("d_rb", [HD, L], F32),
        ]:
            dbg[nm] = nc.dram_tensor(nm, shp, dt_, kind="ExternalOutput").ap()

    xT_r = xT.rearrange("(dc p) t -> p dc t", p=128)

    from contextlib import ExitStack

    with tile.TileContext(nc) as tc:
        with ExitStack() as _st:
            _e = _st.enter_context
            cp = _e(tc.tile_pool(name="const", bufs=1))
            wp = _e(tc.tile_pool(name="wpool", bufs=1))
            wf = _e(tc.tile_pool(name="wff", bufs=2))
            xbp = _e(tc.tile_pool(name="xb", bufs=2))
            sqp = _e(tc.tile_pool(name="sq", bufs=2))
            xsp = _e(tc.tile_pool(name="xsp", bufs=2))
            stp = _e(tc.tile_pool(name="stats", bufs=2))
            st1 = _e(tc.tile_pool(name="stats1", bufs=1))
            rs4p = _e(tc.tile_pool(name="rs4", bufs=4))
            qkp = _e(tc.tile_pool(name="qk", bufs=4))
            vep = _e(tc.tile_pool(name="vex", bufs=2))
            wtp = _e(tc.tile_pool(name="wtp", bufs=4))
            atp = _e(tc.tile_pool(name="atn", bufs=4))
            h1p = _e(tc.tile_pool(name="h1g", bufs=4))
            osb = _e(tc.tile_pool(name="osb", bufs=2))
            dram = _e(tc.tile_pool(name="dram", bufs=1, space="DRAM"))
            ps_a = _e(tc.tile_pool(name="ps_a", bufs=3, space="PSUM"))
            ps_av = _e(tc.tile_pool(name="ps_av", bufs=1, space="PSUM"))
            ps_acc = _e(tc.tile_pool(name="ps_acc", bufs=4, space="PSUM"))
            # ---- constants
            onesf = cp.tile([1, L], F32, tag="onesf")
            nc.vector.memset(onesf[:], 1.0)
            ones_row = cp.tile([1, L], F32R, tag="ones_row")
            nc.vector.tensor_copy(ones_row[:], onesf[:])
            redf = cp.tile([128, 128], F32, tag="redf")
            nc.vector.memset(redf[:], 1.0 / D)
            red = cp.tile([128, 128], F32R, tag="red")
            nc.vector.tensor_copy(red[:], redf[:])
            eps_t = cp.tile([128, 1], F32, tag="eps")
            nc.vector.memset(eps_t[:], EPS)
            ones_col = cp.tile([128, 1], F32, tag="ones_col")
            nc.vector.memset(ones_col[:], 1.0)
            ident = cp.tile([128, 128], F32, tag="ident")
            make_identity(nc, ident[:])
            tri_sb = cp.tile([128, 128], F32, tag="tri")
            nc.sync.dma_start(tri_sb[:], tri)
            fng_sb = cp.tile([128, DC], F32, tag="fng")
            nc.sync.dma_start(fng_sb[:], fng)
            fnb_sb = cp.tile([128, DC], F32, tag="fnb")
            nc.sync.dma_start(fnb_sb[:], fnb)

            # residual stream working copies in DRAM:
            # xw = running residual; xa = post-attention snapshot (LN2 input)
            xw = dram.tile([BLOC, 128, DC, L], F32, tag="xw")
            xa = dram.tile([BLOC, 128, DC, L], F32, tag="xa")

            def load_x(b, layer, src=None):
                x_b = xbp.tile([128, DC, L], F32R, tag="xb")
                if src is None and layer == 0:
                    nc.sync.dma_start(
                        x_b[:], xT_r[:, :, b * L:(b + 1) * L].bitcast(F32R)
                    )
                else:
                    nc.sync.dma_start(x_b[:], (xw if src is None else src)[b].bitcast(F32R))
                return x_b

            def ln_stats(x_b, rs_out, mu_out=None):
                """rs_out[:] = rsqrt(var(x)+eps) (+ mean into mu_out)."""
                xsq = sqp.tile([128, DC, L], F32R, tag="xsq")
                nc.scalar.activation(out=xsq[:], in_=x_b[:], func=AF.Square, scale=1.0)
                ps_mu = ps_a.tile([128, L], F32, tag="seq")
                ps_sq = ps_a.tile([128, L], F32, tag="seq")
                for dc in range(DC):
                    nc.tensor.matmul(ps_mu[:], red[:], x_b[:, dc, :],
                                     start=(dc == 0), stop=(dc == DC - 1))
                for dc in range(DC):
                    nc.tensor.matmul(ps_sq[:], red[:], xsq[:, dc, :],
                                     start=(dc == 0), stop=(dc == DC - 1))
                musq = stp.tile([128, L], F32, tag="musq")
                nc.scalar.activation(out=musq[:], in_=ps_mu[:], func=AF.Square, scale=1.0)
                nc.vector.tensor_sub(musq[:], ps_sq[:], musq[:])
                nc.scalar.activation(out=musq[:], in_=musq[:], func=AF.Sqrt,
                                     bias=eps_t[:], scale=1.0)
                scr = stp.tile([128, L], F32, tag="scr")
                nc.vector.reciprocal_approx_accurate(rs_out, musq[:], scr[:])
                if mu_out is not None:
                    nc.vector.tensor_copy(mu_out, ps_mu[:])

            def scaled(x_b, rs):
                xs = xsp.tile([128, DC, L], BF16, tag="xs")
                nc.vector.tensor_mul(
                    xs[:], x_b[:], rs.unsqueeze(1).broadcast_to([128, DC, L])
                )
                return xs

            for i in range(NLB):
                wq_sb = wp.tile([128, DC, D], BF16, tag="wq")
                nc.sync.dma_start(wq_sb[:], wq[i].rearrange("(c p) o -> p c o", p=128))
                wk_sb = wp.tile([128, DC, D], BF16, tag="wk")
                nc.sync.dma_start(wk_sb[:], wk[i].rearrange("(c p) o -> p c o", p=128))
                wv_sb = wp.tile([128, DC, D], BF16, tag="wv")
                nc.sync.dma_start(wv_sb[:], wv[i].rearrange("(c p) o -> p c o", p=128))
                wo_sb = wp.tile([128, DC, D], BF16, tag="wo")
                nc.sync.dma_start(wo_sb[:], wo[i].rearrange("(c p) o -> p c o", p=128))
                bqh_sb = wp.tile([HD, H], F32, tag="bqh")
                nc.sync.dma_start(bqh_sb[:], bqh[i])
                bkh_sb = wp.tile([HD, H], F32, tag="bkh")
                nc.sync.dma_start(bkh_sb[:], bkh[i])
                bvr_sb = wp.tile([1, D], F32R, tag="bvr")
                nc.sync.dma_start(bvr_sb[:], bvr[i].unsqueeze(0).bitcast(F32R))
                bor_sb = wp.tile([1, D], F32R, tag="bor")
                nc.sync.dma_start(bor_sb[:], bor[i].unsqueeze(0).bitcast(F32R))
                b2r_sb = wp.tile([1, D], F32R, tag="b2r")
                nc.sync.dma_start(b2r_sb[:], b2r[i].unsqueeze(0).bitcast(F32R))
                b1h_sb = wp.tile([128, FC], F32, tag="b1h")
                nc.sync.dma_start(b1h_sb[:], b1h[i])

                # ---------------- attention phase (all b) ----------------
                for b in range(BLOC):
                    x_b = load_x(b, i)
                    rs1 = stp.tile([128, L], F32, tag="rs1")
                    ln_stats(x_b, rs1[:])
                    xs = scaled(x_b, rs1[:])
                    if DEBUG and i == 0 and b == 0:
                        nc.sync.dma_start(dbg["d_xs"], xs[:])

                    # v projection, token-major, + ones column for denominators
                    vext = vep.tile([128, DC, H, HD + 1], F32R, tag="vext")
                    nc.vector.tensor_copy(
                        vext[:, :, :, HD:HD + 1],
                        ones_col[:].unsqueeze(1).unsqueeze(1).broadcast_to([128, DC, H, 1]),
                    )
                    for t in range(DC):
                        ps_v = ps_a.tile([128, D], F32, tag="seq")
                        for dc in range(DC):
                            nc.tensor.matmul(
                                ps_v[:], xs[:, dc, t * 128:(t + 1) * 128],
                                wv_sb[:, dc, :], start=(dc == 0), stop=False,
                            )
                        nc.tensor.matmul(ps_v[:], ones_row[0:1, 0:128], bvr_sb[:],
                                         start=False, stop=True)
                        nc.vector.tensor_copy(
                            vext[:, t, :, 0:HD],
                            ps_v[:].rearrange("p (h e) -> p h e", h=H),
                        )
                    if DEBUG and i == 0 and b == 0:
                        nc.sync.dma_start(dbg["d_v"], vext[:])

                    for pair in range(NP):
                        h0, h1 = 2 * pair, 2 * pair + 1
                        ps_q = ps_a.tile([128, L], F32, tag="seq")
                        ps_k = ps_a.tile([128, L], F32, tag="seq")
                        for dc in range(DC):
                            nc.tensor.matmul(
                                ps_q[:], wq_sb[:, dc, pair * 128:(pair + 1) * 128],
                                xs[:, dc, :], start=(dc == 0), stop=(dc == DC - 1),
                            )
                        for dc in range(DC):
                            nc.tensor.matmul(
                                ps_k[:], wk_sb[:, dc, pair * 128:(pair + 1) * 128],
                                xs[:, dc, :], start=(dc == 0), stop=(dc == DC - 1),
                            )
                        qx, kx = {}, {}
                        for hh in (h0, h1):
                            off = 64 * (hh % 2)
                            qx[hh] = qkp.tile([80, L], F32R, tag="qx", name=f"qx{hh}")
                            nc.vector.tensor_scalar_add(
                                out=qx[hh][0:HD, :], in0=ps_q[off:off + HD, :],
                                scalar1=bqh_sb[:, hh:hh + 1],
                            )
                            nc.sync.dma_start(qx[hh][HD:HD + 10, :], qe[b].bitcast(F32R))
                            kx[hh] = qkp.tile([80, L], F32R, tag="kx", name=f"kx{hh}")
                            nc.vector.tensor_scalar_add(
                                out=kx[hh][0:HD, :], in0=ps_k[off:off + HD, :],
                                scalar1=bkh_sb[:, hh:hh + 1],
                            )
                            nc.sync.dma_start(kx[hh][HD:HD + 10, :], ke[i, hh, b].bitcast(F32R))
                        if DEBUG and i == 0 and b == 0 and pair == 0:
                            nc.sync.dma_start(dbg["d_qx"], qx[h0][:])
                            nc.sync.dma_start(dbg["d_kx"], kx[h0][:])

                        attnT = atp.tile([128, L], BF16, tag="attnT")
                        for hh in (h0, h1):
                            wts = []
                            ps_o = ps_av.tile([HD + 1, L], F32, tag="av")
                            for cs in range(DC):
                                n0 = cs * 128
                                ps_s = ps_a.tile([128, L], F32, tag="seq")
                                nc.tensor.matmul(
                                    ps_s[:, 0:L - n0],
                                    kx[hh][0:74, n0:n0 + 128],
                                    qx[hh][0:74, n0:L],
                                    start=True, stop=True,
                                )
                                nc.vector.tensor_add(ps_s[:, 0:128], ps_s[:, 0:128],
                                                     tri_sb[:])
                                if DEBUG and i == 0 and b == 0 and hh == 0:
                                    _scd = stp.tile([128, L], F32, tag="scd", name=f"scd{cs}")
                                    nc.vector.tensor_copy(_scd[:, 0:L - n0], ps_s[:, 0:L - n0])
                                    nc.sync.dma_start(dbg["d_sc"][cs, :, 0:L - n0], _scd[:, 0:L - n0])
                                wt = wtp.tile([128, L], F32R, tag="wt", name=f"wt{cs}")
                                nc.scalar.activation(
                                    out=wt[:, 0:L - n0], in_=ps_s[:, 0:L - n0],
                                    func=AF.Exp, scale=1.0,
                                )
                                if DEBUG and i == 0 and b == 0 and hh == 0:
                                    nc.sync.dma_start(dbg["d_wt"][cs], wt[:])
                                wts.append(wt)
                            for cs in range(DC):
                                n0 = cs * 128
                                nc.tensor.matmul(
                                    ps_o[:, n0:L], vext[:, cs, hh, :],
                                    wts[cs][:, 0:L - n0],
                                    start=(cs == 0), stop=(cs == DC - 1),
                                )
                            if DEBUG and i == 0 and b == 0 and hh == 0:
                                _pod = atp.tile([HD + 1, L], F32, tag="pod")
                                nc.vector.tensor_copy(_pod[:], ps_o[:])
                                nc.sync.dma_start(dbg["d_po"], _pod[:])
                            den = st1.tile([1, L], F32, tag="den")
                            nc.vector.tensor_copy(den[:], ps_o[HD:HD + 1, :])
                            rcp = st1.tile([1, L], F32, tag="rcp")
                            rcs = st1.tile([1, L], F32, tag="rcs")
                            nc.vector.reciprocal_approx_accurate(
                                rcp[:], den[:], rcs[:]
                            )
                            rb = stp.tile([HD, L], F32, tag="rb")
                            nc.gpsimd.partition_broadcast(rb[:], rcp[:])
                            if DEBUG and i == 0 and b == 0 and hh == 0:
                                nc.sync.dma_start(dbg["d_rb"], rb[:])
                            off = 64 * (hh % 2)
                            nc.vector.tensor_mul(attnT[off:off + HD, :],
                                                 ps_o[0:HD, :], rb[:])
                        if DEBUG and i == 0 and b == 0 and pair == 0:
                            nc.sync.dma_start(dbg["d_at"], attnT[:])

                        for oc in range(DC):
                            if pair == 0:
                                ps_x = ps_acc.tile([128, L], F32, tag="acc", name=f"psx{oc}")
                                if oc == 0:
                                    ps_xs = []
                                ps_xs.append(ps_x)
                            nc.tensor.matmul(
                                ps_xs[oc][:], wo_sb[:, pair, oc * 128:(oc + 1) * 128],
                                attnT[:], start=(pair == 0), stop=False,
                            )
                    for oc in range(DC):
                        nc.tensor.matmul(
                            ps_xs[oc][:], bor_sb[0:1, oc * 128:(oc + 1) * 128],
                            ones_row[:], start=False, stop=True,
                        )
                        nc.vector.tensor_add(x_b[:, oc, :], ps_xs[oc][:], x_b[:, oc, :])

                    if DEBUG and i == 0 and b == 0:
                        nc.sync.dma_start(dbg["d_x1"], x_b[:].bitcast(F32))
                    # LN2 stats on post-attention x (rs kept for both ffn halves)
                    rsb = rs4p.tile([128, L], F32, tag="rsb")
                    ln_stats(x_b, rsb[:])
                    if b == 0:
                        rs_list = []
                    rs_list.append(rsb)
                    nc.sync.dma_start(xa[b], x_b[:].bitcast(F32))

                # ---------------- FFN phase (two streamed halves) ----------------
                for half in range(2):
                    w1h = wf.tile([128, DC, FH * 128], BF16, tag="w1h")
                    nc.sync.dma_start(
                        w1h[:],
                        w1[i, :, half * FH * 128:(half + 1) * FH * 128]
                        .rearrange("(c p) o -> p c o", p=128),
                    )
                    w2h = wf.tile([128, FH, D], BF16, tag="w2h")
                    nc.sync.dma_start(
                        w2h[:],
                        w2[i, half * FH * 128:(half + 1) * FH * 128, :]
                        .rearrange("(c p) o -> p c o", p=128),
                    )
                    for b in range(BLOC):
                        x_b = load_x(b, 1, src=xa)
                        xs2 = scaled(x_b, rs_list[b][:])
                        if half == 1:
                            x_b = load_x(b, 1, src=xw)
                        ps_f = [ps_acc.tile([128, L], F32, tag="acc", name=f"psf{_oc}")
                                for _oc in range(DC)]
                        for fc in range(FH):
                            gfc = half * FH + fc
                            ps_h = ps_a.tile([128, L], F32, tag="seq")
                            for dc in range(DC):
                                nc.tensor.matmul(
                                    ps_h[:], w1h[:, dc, fc * 128:(fc + 1) * 128],
                                    xs2[:, dc, :], start=(dc == 0), stop=(dc == DC - 1),
                                )
                            h1g = h1p.tile([128, L], BF16, tag="h1g")
                            nc.scalar.activation(
                                out=h1g[:], in_=ps_h[:], func=AF.Gelu,
                                bias=b1h_sb[:, gfc:gfc + 1], scale=1.0,
                            )
                            for oc in range(DC):
                                nc.tensor.matmul(
                                    ps_f[oc][:], w2h[:, fc, oc * 128:(oc + 1) * 128],
                                    h1g[:], start=(fc == 0),
                                    stop=(half == 0 and fc == FH - 1),
                                )
                        for oc in range(DC):
                            if half == 1:
                                nc.tensor.matmul(
                                    ps_f[oc][:],
                                    b2r_sb[0:1, oc * 128:(oc + 1) * 128],
                                    ones_row[:], start=False, stop=True,
                                )
                            nc.vector.tensor_add(x_b[:, oc, :], ps_f[oc][:],
                                                 x_b[:, oc, :])
                        if DEBUG and i == 0 and b == 0 and half == 1:
                            nc.sync.dma_start(dbg["d_x2"], x_b[:].bitcast(F32))
                        nc.sync.dma_start(xw[b], x_b[:].bitcast(F32))

            # ---------------- final layernorm + transpose ----------------
            for b in range(BLOC):
                x_b = load_x(b, NLB)
                rs1 = stp.tile([128, L], F32, tag="rs1")
                mu = stp.tile([128, L], F32, tag="mu")
                ln_stats(x_b, rs1[:], mu_out=mu[:])
                xc = xsp.tile([128, DC, L], F32, tag="xs")
                nc.vector.tensor_sub(
                    xc[:], x_b[:], mu[:].unsqueeze(1).broadcast_to([128, DC, L])
                )
                xf = sqp.tile([128, DC, L], F32, tag="xsq")
                nc.vector.tensor_mul(
                    xf[:], xc[:], rs1[:].unsqueeze(1).broadcast_to([128, DC, L])
                )
                for dc in range(DC):
                    nc.vector.tensor_scalar(
                        out=xf[:, dc, :], in0=xf[:, dc, :],
                        scalar1=fng_sb[:, dc:dc + 1], scalar2=fnb_sb[:, dc:dc + 1],
                        op0=AL.mult, op1=AL.add,
                    )
                for t in range(DC):
                    o_sb = osb.tile([128, D], F32, tag="osb")
                    for dc in range(DC):
                        ps_t = ps_a.tile([128, 128], F32, tag="seq")
                        nc.tensor.transpose(
                            ps_t[:], xf[:, dc, t * 128:(t + 1) * 128], ident[:]
                        )
                        nc.vector.tensor_copy(o_sb[:, dc * 128:(dc + 1) * 128], ps_t[:])
                    nc.sync.dma_start(out[b, t * 128:(t + 1) * 128, :], o_sb[:])

    nc.compile()
    return nc


def _center_cols(W):
    return W - W.mean(axis=0, keepdims=True)


def _preprocess(inputs):
    """Host-side folding; returns per-core in_maps."""
    f32 = np.float32
    g = {k: np.asarray(v) for k, v in inputs.items()}
    Wq, Wk, Wv, Wo = g["Wq"], g["Wk"], g["Wv"], g["Wo"]
    W1, W2 = g["W1"], g["W2"]
    g1, b1n = g["ln1_g"], g["ln1_b"]
    g2, b2n = g["ln2_g"], g["ln2_b"]

    wq_e = np.stack([_center_cols(g1[i][:, None] * Wq[i]) / SCALE for i in range(NL)]).astype(f32)
    bq_e = np.stack([(g["bq"][i] + b1n[i] @ Wq[i]) / SCALE for i in range(NL)]).astype(f32)
    wk_e = np.stack([_center_cols(g1[i][:, None] * Wk[i]) for i in range(NL)]).astype(f32)
    bk_e = np.stack([g["bk"][i] + b1n[i] @ Wk[i] for i in range(NL)]).astype(f32)
    wv_e = np.stack([_center_cols(g1[i][:, None] * Wv[i]) for i in range(NL)]).astype(f32)
    bv_e = np.stack([g["bv"][i] + b1n[i] @ Wv[i] for i in range(NL)]).astype(f32)
    w1_e = np.stack([_center_cols(g2[i][:, None] * W1[i]) for i in range(NL)]).astype(f32)
    b1_e = np.stack([g["b1"][i] + b2n[i] @ W1[i] for i in range(NL)]).astype(f32)

    ci = g["case_ids"].astype(np.int64)
    am = g["attention_mask"].astype(f32)
    verb = (ci == 8).astype(f32)
    qe = np.zeros((B, 10, L), f32)
    for c in range(C):
        qe[:, c, :] = (ci == c)
    qe[:, 9, :] = 1.0
    cb = g["case_bias"].astype(f32)
    vb = g["verb_bias"].astype(f32)
    ke = np.zeros((NL, H, B, 10, L), f32)
    for i in range(NL):
        for h in range(H):
            ke[i, h, :, 0:C, :] = np.transpose(cb[i, h][:, ci], (1, 0, 2))
            ke[i, h, :, 9, :] = vb[i, h] * verb - 10000.0 * (1.0 - am)

    tri = np.where(
        np.arange(128)[:, None] > np.arange(128)[None, :], f32(-10000.0), f32(0.0)
    ).astype(f32)

    import ml_dtypes
    bf16 = ml_dtypes.bfloat16
    common = {
        "wq": wq_e[:NLB].astype(bf16), "wk": wk_e[:NLB].astype(bf16),
        "wv": wv_e[:NLB].astype(bf16),
        "wo": np.ascontiguousarray(Wo.astype(f32)[:NLB]).astype(bf16),
        "w1": w1_e[:NLB].astype(bf16),
        "w2": np.ascontiguousarray(W2.astype(f32)[:NLB]).astype(bf16),
        "bqh": np.ascontiguousarray(bq_e.reshape(NL, H, HD).transpose(0, 2, 1))[:NLB],
        "bkh": np.ascontiguousarray(bk_e.reshape(NL, H, HD).transpose(0, 2, 1))[:NLB],
        "bvr": bv_e[:NLB], "bor": np.ascontiguousarray(g["bo"].astype(f32)[:NLB]),
        "b1h": np.ascontiguousarray(b1_e.reshape(NL, FC, 128).transpose(0, 2, 1))[:NLB],
        "b2r": np.ascontiguousarray(g["b2"].astype(f32)[:NLB]),
        "tri": tri,
        "fng": np.ascontiguousarray(g["fn_g"].astype(f32).reshape(DC, 128).T),
        "fnb": np.ascontiguousarray(g["fn_b"].astype(f32).reshape(DC, 128).T),
    }
    x = g["x"].astype(f32)
    in_maps = []
    for core in range(NCORES):
        sl = slice(core * BLOC, (core + 1) * BLOC)
        m = dict(common)
        m["xT"] = np.ascontiguousarray(x[sl].reshape(BLOC * L, D).T)
        m["qe"] = np.ascontiguousarray(qe[sl])
        m["ke"] = np.ascontiguousarray(ke[:NLB, :, sl])
        in_maps.append(m)
    return in_maps


def _get_program():
    global _PROGRAM
    if _PROGRAM is None:
        _PROGRAM = _build_program()
    return _PROGRAM


def kernel(**inputs) -> np.ndarray:
    from concourse.bass_utils import run_bass_kernel_spmd

    nc = _get_program()
    in_maps = _preprocess(inputs)
    res = run_bass_kernel_spmd(nc, in_maps, list(range(NCORES)))
    return np.concatenate(
        [res.results[c]["out"] for c in range(NCORES)], axis=0
    ).astype(np.float32)


# revision 28
# speedup vs baseline: 1.1338x; 1.0500x over previous
"""BrahmanTransformer Trainium2 kernel.

Sharding: data-parallel over batch (32 -> 4 sequences per core x 8 cores),
full 6-layer transformer per core in one Bass/Tile program; float32r matmuls.

Device dataflow (per core, d-major residual):
- LayerNorm: gamma/beta folded into weights host-side; mean subtraction folded
  via column-centered weights; on-device LN is xs = x * rsqrt(var+eps) only.
- Attention: case one-hot (9) + const row (1) appended to q, gathered
  case-bias columns + verb/pad row appended to k -> single K=74 score matmul.
  scoresT layout [key, query]; causality via partial-width matmuls; softmax
  without max-subtraction (scores are O(1)); denominators via a ones column
  appended to V (token-major) and the AV matmul.
- FFN: fc-halves streamed from HBM; psum-accumulated ffn2; gelu on ACT.
- All biases exact: ACT-copy bias (q/k), K=1 matmuls (v/o/ffn2), gelu bias.
"""
import os
import numpy as np

B, L, D, H, NL, F, C = 32, 512, 512, 8, 6, 2048, 9
HD = D // H
NCORES = 8
BLOC = B // NCORES            # 4 sequences per core
SCALE = float(np.sqrt(HD))
EPS = 1e-5
NLB = int(os.environ.get("KB_LAYERS", str(NL)))  # build-depth (debug aid)
DEBUG = bool(int(os.environ.get("KB_DEBUG", "0")))

DC = D // 128     # 4 d-chunks
FC = F // 128     # 16 f-chunks
FH = FC // 2      # 8 f-chunks per streamed half
NP = H // 2       # 4 head pairs

_PROGRAM = None


def _build_program(with_biases=True):
    import concourse.bacc as bacc
    import concourse.mybir as mybir
    import concourse.tile as tile
    from concourse.masks import make_identity

    F32 = mybir.dt.float32
    F32R = mybir.dt.float32r
    AF = mybir.ActivationFunctionType
    AL = mybir.AluOpType

    nc = bacc.Bacc("TRN2", target_bir_lowering=False, debug=False)

    BF16 = mybir.dt.bfloat16

    def inp(name, shape, dt=F32):
        return nc.dram_tensor(name, shape, dt, kind="ExternalInput").ap()

    xT = inp("xT", [D, BLOC * L])
    wq = inp("wq", [NLB, D, D], BF16)
    wk = inp("wk", [NLB, D, D], BF16)
    wv = inp("wv", [NLB, D, D], BF16)
    wo = inp("wo", [NLB, D, D], BF16)
    w1 = inp("w1", [NLB, D, F], BF16)
    w2 = inp("w2", [NLB, F, D], BF16)
    bqh = inp("bqh", [NLB, HD, H])
    bkh = inp("bkh", [NLB, HD, H])
    bvr = inp("bvr", [NLB, D])
    bor = inp("bor", [NLB, D])
    b1h = inp("b1h", [NLB, 128, FC])
    b2r = inp("b2r", [NLB, D])
    qe = inp("qe", [BLOC, 10, L])
    ke = inp("ke", [NLB, H, BLOC, 10, L])
    tri = inp("tri", [128, 128])
    fng = inp("fng", [128, DC])
    fnb = inp("fnb", [128, DC])
    out = nc.dram_tensor("out", [BLOC, L, D], F32, kind="ExternalOutput").ap()
    dbg = {}
    if DEBUG:
        BF16_ = mybir.dt.bfloat16
        for nm, shp, dt_ in [
            ("d_xs", [128, DC, L], BF16_), ("d_v", [128, DC, H, HD + 1], BF16_),
            ("d_qx", [80, L], BF16_), ("d_kx", [80, L], BF16_),
            ("d_wt", [DC, 128, L], BF16_),
            ("d_at", [128, L], BF16_), ("d_x1", [128, DC, L], F32),
            ("d_x2", [128, DC, L], F32),
            ("d_sc", [DC, 128, L], F32), ("d_po", [HD + 1, L], F32),
            ("d_rb", [HD, L], F32),
        ]:
            dbg[nm] = nc.dram_tensor(nm, shp, dt_, kind="ExternalOutput").ap()

    xT_r = xT.rearrange("(dc p) t -> p dc t", p=128)

    from contextlib import ExitStack

    with tile.TileContext(nc) as tc:
        with ExitStack() as _st:
            _e = _st.enter_context
            cp = _e(tc.tile_pool(name="const", bufs=1))
            wp = _e(tc.tile_pool(name="wpool", bufs=1))
            wf = _e(tc.tile_pool(name="wff", bufs=2))
            xbp = _e(tc.tile_pool(name="xb", bufs=2))
            sqp = _e(tc.tile_pool(name="sq", bufs=2))
            xsp = _e(tc.tile_pool(name="xsp", bufs=2))
            stp = _e(tc.tile_pool(name="stats", bufs=2))
            st1 = _e(tc.tile_pool(name="stats1", bufs=1))
            rs4p = _e(tc.tile_pool(name="rs4", bufs=4))
            qkp = _e(tc.tile_pool(name="qk", bufs=4))
            vep = _e(tc.tile_pool(name="vex", bufs=2))
            wtp = _e(tc.tile_pool(name="wtp", bufs=4))
            atp = _e(tc.tile_pool(name="atn", bufs=4))
            h1p = _e(tc.tile_pool(name="h1g", bufs=4))
            osb = _e(tc.tile_pool(name="osb", bufs=2))
            dram = _e(tc.tile_pool(name="dram", bufs=1, space="DRAM"))
            ps_a = _e(tc.tile_pool(name="ps_a", bufs=3, space="PSUM"))
            ps_av = _e(tc.tile_pool(name="ps_av", bufs=1, space="PSUM"))
            ps_acc = _e(tc.tile_pool(name="ps_acc", bufs=4, space="PSUM"))
            # ---- constants
            onesf = cp.tile([1, L], F32, tag="onesf")
            nc.vector.memset(onesf[:], 1.0)
            ones_row = cp.tile([1, L], F32R, tag="ones_row")
            nc.vector.tensor_copy(ones_row[:], onesf[:])
            redf = cp.tile([128, 128], F32, tag="redf")
            nc.vector.memset(redf[:], 1.0 / D)
            red = cp.tile([128, 128], F32R, tag="red")
            nc.vector.tensor_copy(red[:], redf[:])
            eps_t = cp.tile([128, 1], F32, tag="eps")
            nc.vector.memset(eps_t[:], EPS)
            ones_col = cp.tile([128, 1], F32, tag="ones_col")
            nc.vector.memset(ones_col[:], 1.0)
            ident = cp.tile([128, 128], F32, tag="ident")
            make_identity(nc, ident[:])
            tri_sb = cp.tile([128, 128], F32, tag="tri")
            nc.sync.dma_start(tri_sb[:], tri)
            fng_sb = cp.tile([128, DC], F32, tag="fng")
            nc.sync.dma_start(fng_sb[:], fng)
            fnb_sb = cp.tile([128, DC], F32, tag="fnb")
            nc.sync.dma_start(fnb_sb[:], fnb)

            # residual stream working copies in DRAM:
            # xw = running residual; xa = post-attention snapshot (LN2 input)
            xw = dram.tile([BLOC, 128, DC, L], F32, tag="xw")
            xa = dram.tile([BLOC, 128, DC, L], F32, tag="xa")

            def load_x(b, layer, src=None):
                x_b = xbp.tile([128, DC, L], F32R, tag="xb")
                if src is None and layer == 0:
                    nc.sync.dma_start(
                        x_b[:], xT_r[:, :, b * L:(b + 1) * L].bitcast(F32R)
                    )
                else:
                    nc.sync.dma_start(x_b[:], (xw if src is None else src)[b].bitcast(F32R))
                return x_b

            def ln_stats(x_b, rs_out, mu_out=None):
                """rs_out[:] = rsqrt(var(x)+eps) (+ mean into mu_out)."""
                xsq = sqp.tile([128, DC, L], F32R, tag="xsq")
                nc.scalar.activation(out=xsq[:], in_=x_b[:], func=AF.Square, scale=1.0)
                ps_mu = ps_a.tile([128, L], F32, tag="seq")
                ps_sq = ps_a.tile([128, L], F32, tag="seq")
                for dc in range(DC):
                    nc.tensor.matmul(ps_mu[:], red[:], x_b[:, dc, :],
                                     start=(dc == 0), stop=(dc == DC - 1))
                for dc in range(DC):
                    nc.tensor.matmul(ps_sq[:], red[:], xsq[:, dc, :],
                                     start=(dc == 0), stop=(dc == DC - 1))
                musq = stp.tile([128, L], F32, tag="musq")
                nc.scalar.activation(out=musq[:], in_=ps_mu[:], func=AF.Square, scale=1.0)
                nc.vector.tensor_sub(musq[:], ps_sq[:], musq[:])
                nc.scalar.activation(out=musq[:], in_=musq[:], func=AF.Sqrt,
                                     bias=eps_t[:], scale=1.0)
                scr = stp.tile([128, L], F32, tag="scr")
                nc.vector.reciprocal_approx_accurate(rs_out, musq[:], scr[:])
                if mu_out is not None:
                    nc.vector.tensor_copy(mu_out, ps_mu[:])

            def scaled(x_b, rs):
                xs = xsp.tile([128, DC, L], BF16, tag="xs")
                nc.vector.tensor_mul(
                    xs[:], x_b[:], rs.unsqueeze(1).broadcast_to([128, DC, L])
                )
                return xs

            for i in range(NLB):
                wq_sb = wp.tile([128, DC, D], BF16, tag="wq")
                nc.sync.dma_start(wq_sb[:], wq[i].rearrange("(c p) o -> p c o", p=128))
                wk_sb = wp.tile([128, DC, D], BF16, tag="wk")
                nc.sync.dma_start(wk_sb[:], wk[i].rearrange("(c p) o -> p c o", p=128))
                wv_sb = wp.tile([128, DC, D], BF16, tag="wv")
                nc.sync.dma_start(wv_sb[:], wv[i].rearrange("(c p) o -> p c o", p=128))
                wo_sb = wp.tile([128, DC, D], BF16, tag="wo")
                nc.sync.dma_start(wo_sb[:], wo[i].rearrange("(c p) o -> p c o", p=128))
                bqh_sb = wp.tile([HD, H], F32, tag="bqh")
                nc.sync.dma_start(bqh_sb[:], bqh[i])
                bkh_sb = wp.tile([HD, H], F32, tag="bkh")
                nc.sync.dma_start(bkh_sb[:], bkh[i])
                bvr_sb = wp.tile([1, D], F32R, tag="bvr")
                nc.sync.dma_start(bvr_sb[:], bvr[i].unsqueeze(0).bitcast(F32R))
                bor_sb = wp.tile([1, D], F32R, tag="bor")
                nc.sync.dma_start(bor_sb[:], bor[i].unsqueeze(0).bitcast(F32R))
                b2r_sb = wp.tile([1, D], F32R, tag="b2r")
                nc.sync.dma_start(b2r_sb[:], b2r[i].unsqueeze(0).bitcast(F32R))
                b1h_sb = wp.tile([128, FC], F32, tag="b1h")
                nc.sync.dma_start(b1h_sb[:], b1h[i])

                # ---------------- attention phase (all b) ----------------
                for b in range(BLOC):
                    x_b = load_x(b, i)
                    rs1 = stp.tile([128, L], F32, tag="rs1")
                    ln_stats(x_b, rs1[:])
                    xs = scaled(x_b, rs1[:])
                    if DEBUG and i == 0 and b == 0:
                        nc.sync.dma_start(dbg["d_xs"], xs[:])

                    # v projection, token-major, + ones column for denominators
                    vext = vep.tile([128, DC, H, HD + 1], F32R, tag="vext")
                    nc.vector.tensor_copy(
                        vext[:, :, :, HD:HD + 1],
                        ones_col[:].unsqueeze(1).unsqueeze(1).broadcast_to([128, DC, H, 1]),
                    )
                    for t in range(DC):
                        ps_v = ps_a.tile([128, D], F32, tag="seq")
                        for dc in range(DC):
                            nc.tensor.matmul(
                                ps_v[:], xs[:, dc, t * 128:(t + 1) * 128],
                                wv_sb[:, dc, :], start=(dc == 0),
                                stop=(not with_biases and dc == DC - 1),
                            )
                        if with_biases:
                            nc.tensor.matmul(ps_v[:], ones_row[0:1, 0:128], bvr_sb[:],
                                             start=False, stop=True)
                        nc.vector.tensor_copy(
                            vext[:, t, :, 0:HD],
                            ps_v[:].rearrange("p (h e) -> p h e", h=H),
                        )
                    if DEBUG and i == 0 and b == 0:
                        nc.sync.dma_start(dbg["d_v"], vext[:])

                    for pair in range(NP):
                        h0, h1 = 2 * pair, 2 * pair + 1
                        ps_q = ps_a.tile([128, L], F32, tag="seq")
                        ps_k = ps_a.tile([128, L], F32, tag="seq")
                        for dc in range(DC):
                            nc.tensor.matmul(
                                ps_q[:], wq_sb[:, dc, pair * 128:(pair + 1) * 128],
                                xs[:, dc, :], start=(dc == 0), stop=(dc == DC - 1),
                            )
                        for dc in range(DC):
                            nc.tensor.matmul(
                                ps_k[:], wk_sb[:, dc, pair * 128:(pair + 1) * 128],
                                xs[:, dc, :], start=(dc == 0), stop=(dc == DC - 1),
                            )
                        qx, kx = {}, {}
                        for hh in (h0, h1):
                            off = 64 * (hh % 2)
                            qx[hh] = qkp.tile([80, L], F32R, tag="qx", name=f"qx{hh}")
                            nc.vector.tensor_scalar_add(
                                out=qx[hh][0:HD, :], in0=ps_q[off:off + HD, :],
                                scalar1=bqh_sb[:, hh:hh + 1],
                            )
                            nc.sync.dma_start(qx[hh][HD:HD + 10, :], qe[b].bitcast(F32R))
                            kx[hh] = qkp.tile([80, L], F32R, tag="kx", name=f"kx{hh}")
                            nc.vector.tensor_scalar_add(
                                out=kx[hh][0:HD, :], in0=ps_k[off:off + HD, :],
                                scalar1=bkh_sb[:, hh:hh + 1],
                            )
                            nc.sync.dma_start(kx[hh][HD:HD + 10, :], ke[i, hh, b].bitcast(F32R))
                        if DEBUG and i == 0 and b == 0 and pair == 0:
                            nc.sync.dma_start(dbg["d_qx"], qx[h0][:])
                            nc.sync.dma_start(dbg["d_kx"], kx[h0][:])

                        attnT = atp.tile([128, L], BF16, tag="attnT")
                        for hh in (h0, h1):
                            wts = []
                            ps_o = ps_av.tile([HD + 1, L], F32, tag="av")
                            for cs in range(DC):
                                n0 = cs * 128
                                ps_s = ps_a.tile([128, L], F32, tag="seq")
                                nc.tensor.matmul(
                                    ps_s[:, 0:L - n0],
                                    kx[hh][0:74, n0:n0 + 128],
                                    qx[hh][0:74, n0:L],
                                    start=True, stop=True,
                                )
                                nc.vector.tensor_add(ps_s[:, 0:128], ps_s[:, 0:128],
                                                     tri_sb[:])
                                if DEBUG and i == 0 and b == 0 and hh == 0:
                                    _scd = stp.tile([128, L], F32, tag="scd", name=f"scd{cs}")
                                    nc.vector.tensor_copy(_scd[:, 0:L - n0], ps_s[:, 0:L - n0])
                                    nc.sync.dma_start(dbg["d_sc"][cs, :, 0:L - n0], _scd[:, 0:L - n0])
                                wt = wtp.tile([128, L], F32R, tag="wt", name=f"wt{cs}")
                                nc.scalar.activation(
                                    out=wt[:, 0:L - n0], in_=ps_s[:, 0:L - n0],
                                    func=AF.Exp, scale=1.0,
                                )
                                if DEBUG and i == 0 and b == 0 and hh == 0:
                                    nc.sync.dma_start(dbg["d_wt"][cs], wt[:])
                                wts.append(wt)
                            for cs in range(DC):
                                n0 = cs * 128
                                nc.tensor.matmul(
                                    ps_o[:, n0:L], vext[:, cs, hh, :],
                                    wts[cs][:, 0:L - n0],
                                    start=(cs == 0), stop=(cs == DC - 1),
                                )
                            if DEBUG and i == 0 and b == 0 and hh == 0:
                                _pod = atp.tile([HD + 1, L], F32, tag="pod")
                                nc.vector.tensor_copy(_pod[:], ps_o[:])
                                nc.sync.dma_start(dbg["d_po"], _pod[:])
                            den = st1.tile([1, L], F32, tag="den")
                            nc.vector.tensor_copy(den[:], ps_o[HD:HD + 1, :])
                            rcp = st1.tile([1, L], F32, tag="rcp")
                            rcs = st1.tile([1, L], F32, tag="rcs")
                            nc.vector.reciprocal_approx_accurate(
                                rcp[:], den[:], rcs[:]
                            )
                            rb = stp.tile([HD, L], F32, tag="rb")
                            nc.gpsimd.partition_broadcast(rb[:], rcp[:])
                            if DEBUG and i == 0 and b == 0 and hh == 0:
                                nc.sync.dma_start(dbg["d_rb"], rb[:])
                            off = 64 * (hh % 2)
                            nc.vector.tensor_mul(attnT[off:off + HD, :],
                                                 ps_o[0:HD, :], rb[:])
                        if DEBUG and i == 0 and b == 0 and pair == 0:
                            nc.sync.dma_start(dbg["d_at"], attnT[:])

                        for oc in range(DC):
                            if pair == 0:
                                ps_x = ps_acc.tile([128, L], F32, tag="acc", name=f"psx{oc}")
                                if oc == 0:
                                    ps_xs = []
                                ps_xs.append(ps_x)
                            nc.tensor.matmul(
                                ps_xs[oc][:], wo_sb[:, pair, oc * 128:(oc + 1) * 128],
                                attnT[:], start=(pair == 0),
                                stop=(not with_biases and pair == NP - 1),
                            )
                    for oc in range(DC):
                        if with_biases:
                            nc.tensor.matmul(
                                ps_xs[oc][:], bor_sb[0:1, oc * 128:(oc + 1) * 128],
                                ones_row[:], start=False, stop=True,
                            )
                        nc.vector.tensor_add(x_b[:, oc, :], ps_xs[oc][:], x_b[:, oc, :])

                    if DEBUG and i == 0 and b == 0:
                        nc.sync.dma_start(dbg["d_x1"], x_b[:].bitcast(F32))
                    # LN2 stats on post-attention x (rs kept for both ffn halves)
                    rsb = rs4p.tile([128, L], F32, tag="rsb")
                    ln_stats(x_b, rsb[:])
                    if b == 0:
                        rs_list = []
                    rs_list.append(rsb)
                    nc.sync.dma_start(xa[b], x_b[:].bitcast(F32))

                # ---------------- FFN phase (two streamed halves) ----------------
                for half in range(2):
                    w1h = wf.tile([128, DC, FH * 128], BF16, tag="w1h")
                    nc.sync.dma_start(
                        w1h[:],
                        w1[i, :, half * FH * 128:(half + 1) * FH * 128]
                        .rearrange("(c p) o -> p c o", p=128),
                    )
                    w2h = wf.tile([128, FH, D], BF16, tag="w2h")
                    nc.sync.dma_start(
                        w2h[:],
                        w2[i, half * FH * 128:(half + 1) * FH * 128, :]
                        .rearrange("(c p) o -> p c o", p=128),
                    )
                    for b in range(BLOC):
                        x_b = load_x(b, 1, src=xa)
                        xs2 = scaled(x_b, rs_list[b][:])
                        if half == 1:
                            x_b = load_x(b, 1, src=xw)
                        ps_f = [ps_acc.tile([128, L], F32, tag="acc", name=f"psf{_oc}")
                                for _oc in range(DC)]
                        for fc in range(FH):
                            gfc = half * FH + fc
                            ps_h = ps_a.tile([128, L], F32, tag="seq")
                            for dc in range(DC):
                                nc.tensor.matmul(
                                    ps_h[:], w1h[:, dc, fc * 128:(fc + 1) * 128],
                                    xs2[:, dc, :], start=(dc == 0), stop=(dc == DC - 1),
                                )
                            h1g = h1p.tile([128, L], BF16, tag="h1g")
                            nc.scalar.activation(
                                out=h1g[:], in_=ps_h[:], func=AF.Gelu,
                                bias=b1h_sb[:, gfc:gfc + 1], scale=1.0,
                            )
                            for oc in range(DC):
                                nc.tensor.matmul(
                                    ps_f[oc][:], w2h[:, fc, oc * 128:(oc + 1) * 128],
                                    h1g[:], start=(fc == 0),
                                    stop=(fc == FH - 1 and (half == 0 or not with_biases)),
                                )
                        for oc in range(DC):
                            if half == 1 and with_biases:
                                nc.tensor.matmul(
                                    ps_f[oc][:],
                                    b2r_sb[0:1, oc * 128:(oc + 1) * 128],
                                    ones_row[:], start=False, stop=True,
                                )
                            nc.vector.tensor_add(x_b[:, oc, :], ps_f[oc][:],
                                                 x_b[:, oc, :])
                        if DEBUG and i == 0 and b == 0 and half == 1:
                            nc.sync.dma_start(dbg["d_x2"], x_b[:].bitcast(F32))
                        nc.sync.dma_start(xw[b], x_b[:].bitcast(F32))

            # ---------------- final layernorm + transpose ----------------
            for b in range(BLOC):
                x_b = load_x(b, NLB)
                rs1 = stp.tile([128, L], F32, tag="rs1")
                mu = stp.tile([128, L], F32, tag="mu")
                ln_stats(x_b, rs1[:], mu_out=mu[:])
                xc = xsp.tile([128, DC, L], F32, tag="xs")
                nc.vector.tensor_sub(
                    xc[:], x_b[:], mu[:].unsqueeze(1).broadcast_to([128, DC, L])
                )
                xf = sqp.tile([128, DC, L], F32, tag="xsq")
                nc.vector.tensor_mul(
                    xf[:], xc[:], rs1[:].unsqueeze(1).broadcast_to([128, DC, L])
                )
                for dc in range(DC):
                    nc.vector.tensor_scalar(
                        out=xf[:, dc, :], in0=xf[:, dc, :],
                        scalar1=fng_sb[:, dc:dc + 1], scalar2=fnb_sb[:, dc:dc + 1],
                        op0=AL.mult, op1=AL.add,
                    )
                for t in range(DC):
                    o_sb = osb.tile([128, D], F32, tag="osb")
                    for dc in range(DC):
                        ps_t = ps_a.tile([128, 128], F32, tag="seq")
                        nc.tensor.transpose(
                            ps_t[:], xf[:, dc, t * 128:(t + 1) * 128], ident[:]
                        )
                        nc.vector.tensor_copy(o_sb[:, dc * 128:(dc + 1) * 128], ps_t[:])
                    nc.sync.dma_start(out[b, t * 128:(t + 1) * 128, :], o_sb[:])

    nc.compile()
    return nc


def _center_cols(W):
    return W - W.mean(axis=0, keepdims=True)


def _preprocess(inputs):
    """Host-side folding; returns per-core in_maps."""
    f32 = np.float32
    g = {k: np.asarray(v) for k, v in inputs.items()}
    Wq, Wk, Wv, Wo = g["Wq"], g["Wk"], g["Wv"], g["Wo"]
    W1, W2 = g["W1"], g["W2"]
    g1, b1n = g["ln1_g"], g["ln1_b"]
    g2, b2n = g["ln2_g"], g["ln2_b"]

    wq_e = np.stack([_center_cols(g1[i][:, None] * Wq[i]) / SCALE for i in range(NL)]).astype(f32)
    bq_e = np.stack([(g["bq"][i] + b1n[i] @ Wq[i]) / SCALE for i in range(NL)]).astype(f32)
    wk_e = np.stack([_center_cols(g1[i][:, None] * Wk[i]) for i in range(NL)]).astype(f32)
    bk_e = np.stack([g["bk"][i] + b1n[i] @ Wk[i] for i in range(NL)]).astype(f32)
    wv_e = np.stack([_center_cols(g1[i][:, None] * Wv[i]) for i in range(NL)]).astype(f32)
    bv_e = np.stack([g["bv"][i] + b1n[i] @ Wv[i] for i in range(NL)]).astype(f32)
    w1_e = np.stack([_center_cols(g2[i][:, None] * W1[i]) for i in range(NL)]).astype(f32)
    b1_e = np.stack([g["b1"][i] + b2n[i] @ W1[i] for i in range(NL)]).astype(f32)

    ci = g["case_ids"].astype(np.int64)
    am = g["attention_mask"].astype(f32)
    verb = (ci == 8).astype(f32)
    qe = np.zeros((B, 10, L), f32)
    for c in range(C):
        qe[:, c, :] = (ci == c)
    qe[:, 9, :] = 1.0
    cb = g["case_bias"].astype(f32)
    vb = g["verb_bias"].astype(f32)
    ke = np.zeros((NL, H, B, 10, L), f32)
    for i in range(NL):
        for h in range(H):
            ke[i, h, :, 0:C, :] = np.transpose(cb[i, h][:, ci], (1, 0, 2))
            ke[i, h, :, 9, :] = vb[i, h] * verb - 10000.0 * (1.0 - am)

    tri = np.where(
        np.arange(128)[:, None] > np.arange(128)[None, :], f32(-10000.0), f32(0.0)
    ).astype(f32)

    import ml_dtypes
    bf16 = ml_dtypes.bfloat16
    common = {
        "wq": wq_e[:NLB].astype(bf16), "wk": wk_e[:NLB].astype(bf16),
        "wv": wv_e[:NLB].astype(bf16),
        "wo": np.ascontiguousarray(Wo.astype(f32)[:NLB]).astype(bf16),
        "w1": w1_e[:NLB].astype(bf16),
        "w2": np.ascontiguousarray(W2.astype(f32)[:NLB]).astype(bf16),
        "bqh": np.ascontiguousarray(bq_e.reshape(NL, H, HD).transpose(0, 2, 1))[:NLB],
        "bkh": np.ascontiguousarray(bk_e.reshape(NL, H, HD).transpose(0, 2, 1))[:NLB],
        "bvr": bv_e[:NLB], "bor": np.ascontiguousarray(g["bo"].astype(f32)[:NLB]),
        "b1h": np.ascontiguousarray(b1_e.reshape(NL, FC, 128).transpose(0, 2, 1))[:NLB],
        "b2r": np.ascontiguousarray(g["b2"].astype(f32)[:NLB]),
        "tri": tri,
        "fng": np.ascontiguousarray(g["fn_g"].astype(f32).reshape(DC, 128).T),
        "fnb": np.ascontiguousarray(g["fn_b"].astype(f32).reshape(DC, 128).T),
    }
    x = g["x"].astype(f32)
    in_maps = []
    for core in range(NCORES):
        sl = slice(core * BLOC, (core + 1) * BLOC)
        m = dict(common)
        m["xT"] = np.ascontiguousarray(x[sl].reshape(BLOC * L, D).T)
        m["qe"] = np.ascontiguousarray(qe[sl])
        m["ke"] = np.ascontiguousarray(ke[:NLB, :, sl])
        in_maps.append(m)
    return in_maps


def _get_program(with_biases=None):
    global _PROGRAM
    if _PROGRAM is not None and (with_biases is None or _PROGRAM[1] == with_biases):
        return _PROGRAM[0]
    wb = True if with_biases is None else with_biases
    _PROGRAM = (_build_program(wb), wb)
    return _PROGRAM[0]


def kernel(**inputs) -> np.ndarray:
    from concourse.bass_utils import run_bass_kernel_spmd

    wb = any(
        np.any(np.asarray(inputs[k])) for k in ("bv", "bo", "b2")
    ) or np.any(np.asarray(inputs["ln1_b"])) or np.any(np.asarray(inputs["ln2_b"]))
    nc = _get_program(with_biases=wb)
    in_maps = _preprocess(inputs)
    res = run_bass_kernel_spmd(nc, in_maps, list(range(NCORES)))
    return np.concatenate(
        [res.results[c]["out"] for c in range(NCORES)], axis=0
    ).astype(np.float32)


# revision 30
# speedup vs baseline: 1.1401x; 1.0056x over previous
"""BrahmanTransformer Trainium2 kernel.

Sharding: data-parallel over batch (32 -> 4 sequences per core x 8 cores),
full 6-layer transformer per core in one Bass/Tile program; float32r matmuls.

Device dataflow (per core, d-major residual):
- LayerNorm: gamma/beta folded into weights host-side; mean subtraction folded
  via column-centered weights; on-device LN is xs = x * rsqrt(var+eps) only.
- Attention: case one-hot (9) + const row (1) appended to q, gathered
  case-bias columns + verb/pad row appended to k -> single K=74 score matmul.
  scoresT layout [key, query]; causality via partial-width matmuls; softmax
  without max-subtraction (scores are O(1)); denominators via a ones column
  appended to V (token-major) and the AV matmul.
- FFN: fc-halves streamed from HBM; psum-accumulated ffn2; gelu on ACT.
- All biases exact: ACT-copy bias (q/k), K=1 matmuls (v/o/ffn2), gelu bias.
"""
import os
import numpy as np

B, L, D, H, NL, F, C = 32, 512, 512, 8, 6, 2048, 9
HD = D // H
NCORES = 8
BLOC = B // NCORES            # 4 sequences per core
SCALE = float(np.sqrt(HD))
EPS = 1e-5
NLB = int(os.environ.get("KB_LAYERS", str(NL)))  # build-depth (debug aid)
DEBUG = bool(int(os.environ.get("KB_DEBUG", "0")))

DC = D // 128     # 4 d-chunks
FC = F // 128     # 16 f-chunks
FH = FC // 2      # 8 f-chunks per streamed half
NP = H // 2       # 4 head pairs

_PROGRAM = None


def _build_program(with_biases=True):
    import concourse.bacc as bacc
    import concourse.mybir as mybir
    import concourse.tile as tile
    from concourse.masks import make_identity

    F32 = mybir.dt.float32
    F32R = mybir.dt.float32r
    AF = mybir.ActivationFunctionType
    AL = mybir.AluOpType

    nc = bacc.Bacc("TRN2", target_bir_lowering=False, debug=False)

    BF16 = mybir.dt.bfloat16

    def inp(name, shape, dt=F32):
        return nc.dram_tensor(name, shape, dt, kind="ExternalInput").ap()

    xT = inp("xT", [D, BLOC * L])
    wq = inp("wq", [NLB, D, D], BF16)
    wk = inp("wk", [NLB, D, D], BF16)
    wv = inp("wv", [NLB, D, D], BF16)
    wo = inp("wo", [NLB, D, D], BF16)
    w1 = inp("w1", [NLB, D, F], BF16)
    w2 = inp("w2", [NLB, F, D], BF16)
    bqh = inp("bqh", [NLB, HD, H])
    bkh = inp("bkh", [NLB, HD, H])
    bvr = inp("bvr", [NLB, D])
    bor = inp("bor", [NLB, D])
    b1h = inp("b1h", [NLB, 128, FC])
    b2r = inp("b2r", [NLB, D])
    qe = inp("qe", [BLOC, 10, L])
    ke = inp("ke", [NLB, H, BLOC, 10, L])
    tri = inp("tri", [128, 128])
    fng = inp("fng", [128, DC])
    fnb = inp("fnb", [128, DC])
    out = nc.dram_tensor("out", [BLOC, L, D], F32, kind="ExternalOutput").ap()
    dbg = {}
    if DEBUG:
        BF16_ = mybir.dt.bfloat16
        for nm, shp, dt_ in [
            ("d_xs", [128, DC, L], BF16_), ("d_v", [128, DC, H, HD + 1], BF16_),
            ("d_qx", [80, L], BF16_), ("d_kx", [80, L], BF16_),
            ("d_wt", [DC, 128, L], BF16_),
            ("d_at", [128, L], BF16_), ("d_x1", [128, DC, L], F32),
            ("d_x2", [128, DC, L], F32),
            ("d_sc", [DC, 128, L], F32), ("d_po", [HD + 1, L], F32),
            ("d_rb", [HD, L], F32),
        ]:
            dbg[nm] = nc.dram_tensor(nm, shp, dt_, kind="ExternalOutput").ap()

    xT_r = xT.rearrange("(dc p) t -> p dc t", p=128)

    from contextlib import ExitStack

    with tile.TileContext(nc) as tc:
        with ExitStack() as _st:
            _e = _st.enter_context
            cp = _e(tc.tile_pool(name="const", bufs=1))
            wp = _e(tc.tile_pool(name="wpool", bufs=1))
            wf = _e(tc.tile_pool(name="wff", bufs=2))
            xbp = _e(tc.tile_pool(name="xb", bufs=2))
            sqp = _e(tc.tile_pool(name="sq", bufs=2))
            xsp = _e(tc.tile_pool(name="xsp", bufs=2))
            stp = _e(tc.tile_pool(name="stats", bufs=2))
            st1 = _e(tc.tile_pool(name="stats1", bufs=1))
            rs4p = _e(tc.tile_pool(name="rs4", bufs=4))
            qkp = _e(tc.tile_pool(name="qk", bufs=4))
            vep = _e(tc.tile_pool(name="vex", bufs=2))
            wtp = _e(tc.tile_pool(name="wtp", bufs=4))
            atp = _e(tc.tile_pool(name="atn", bufs=4))
            h1p = _e(tc.tile_pool(name="h1g", bufs=4))
            osb = _e(tc.tile_pool(name="osb", bufs=2))
            dram = _e(tc.tile_pool(name="dram", bufs=1, space="DRAM"))
            ps_a = _e(tc.tile_pool(name="ps_a", bufs=3, space="PSUM"))
            ps_av = _e(tc.tile_pool(name="ps_av", bufs=1, space="PSUM"))
            ps_acc = _e(tc.tile_pool(name="ps_acc", bufs=4, space="PSUM"))
            # ---- constants
            onesf = cp.tile([1, L], F32, tag="onesf")
            nc.vector.memset(onesf[:], 1.0)
            ones_row = cp.tile([1, L], F32R, tag="ones_row")
            nc.vector.tensor_copy(ones_row[:], onesf[:])
            redf = cp.tile([128, 128], F32, tag="redf")
            nc.vector.memset(redf[:], 1.0 / D)
            red = cp.tile([128, 128], F32R, tag="red")
            nc.vector.tensor_copy(red[:], redf[:])
            eps_t = cp.tile([128, 1], F32, tag="eps")
            nc.vector.memset(eps_t[:], EPS)
            ones_col = cp.tile([128, 1], F32, tag="ones_col")
            nc.vector.memset(ones_col[:], 1.0)
            ident = cp.tile([128, 128], F32, tag="ident")
            make_identity(nc, ident[:])
            tri_sb = cp.tile([128, 128], F32, tag="tri")
            nc.sync.dma_start(tri_sb[:], tri)
            fng_sb = cp.tile([128, DC], F32, tag="fng")
            nc.sync.dma_start(fng_sb[:], fng)
            fnb_sb = cp.tile([128, DC], F32, tag="fnb")
            nc.sync.dma_start(fnb_sb[:], fnb)

            # residual stream working copies in DRAM:
            # xw = running residual; xa = post-attention snapshot (LN2 input)
            xw = dram.tile([BLOC, 128, DC, L], F32, tag="xw")
            xa = dram.tile([BLOC, 128, DC, L], F32, tag="xa")

            def load_x(b, layer, src=None):
                x_b = xbp.tile([128, DC, L], F32R, tag="xb")
                if src is None and layer == 0:
                    nc.sync.dma_start(
                        x_b[:], xT_r[:, :, b * L:(b + 1) * L].bitcast(F32R)
                    )
                else:
                    nc.sync.dma_start(x_b[:], (xw if src is None else src)[b].bitcast(F32R))
                return x_b

            def ln_stats(x_b, rs_out, mu_out=None):
                """rs_out[:] = rsqrt(var(x)+eps) (+ mean into mu_out)."""
                xsq = sqp.tile([128, DC, L], F32R, tag="xsq")
                nc.scalar.activation(out=xsq[:], in_=x_b[:], func=AF.Square, scale=1.0)
                ps_mu = ps_a.tile([128, L], F32, tag="seq")
                ps_sq = ps_a.tile([128, L], F32, tag="seq")
                for dc in range(DC):
                    nc.tensor.matmul(ps_mu[:], red[:], x_b[:, dc, :],
                                     start=(dc == 0), stop=(dc == DC - 1))
                for dc in range(DC):
                    nc.tensor.matmul(ps_sq[:], red[:], xsq[:, dc, :],
                                     start=(dc == 0), stop=(dc == DC - 1))
                musq = stp.tile([128, L], F32, tag="musq")
                nc.scalar.activation(out=musq[:], in_=ps_mu[:], func=AF.Square, scale=1.0)
                nc.vector.tensor_sub(musq[:], ps_sq[:], musq[:])
                nc.scalar.activation(out=musq[:], in_=musq[:], func=AF.Sqrt,
                                     bias=eps_t[:], scale=1.0)
                scr = stp.tile([128, L], F32, tag="scr")
                nc.vector.reciprocal_approx_accurate(rs_out, musq[:], scr[:])
                if mu_out is not None:
                    nc.vector.tensor_copy(mu_out, ps_mu[:])

            def scaled(x_b, rs):
                xs = xsp.tile([128, DC, L], BF16, tag="xs")
                nc.vector.tensor_mul(
                    xs[:], x_b[:], rs.unsqueeze(1).broadcast_to([128, DC, L])
                )
                return xs

            for i in range(NLB):
                wq_sb = wp.tile([128, DC, D], BF16, tag="wq")
                nc.sync.dma_start(wq_sb[:], wq[i].rearrange("(c p) o -> p c o", p=128))
                wk_sb = wp.tile([128, DC, D], BF16, tag="wk")
                nc.sync.dma_start(wk_sb[:], wk[i].rearrange("(c p) o -> p c o", p=128))
                wv_sb = wp.tile([128, DC, D], BF16, tag="wv")
                nc.sync.dma_start(wv_sb[:], wv[i].rearrange("(c p) o -> p c o", p=128))
                wo_sb = wp.tile([128, DC, D], BF16, tag="wo")
                nc.sync.dma_start(wo_sb[:], wo[i].rearrange("(c p) o -> p c o", p=128))
                bqh_sb = wp.tile([HD, H], F32, tag="bqh")
                nc.sync.dma_start(bqh_sb[:], bqh[i])
                bkh_sb = wp.tile([HD, H], F32, tag="bkh")
                nc.sync.dma_start(bkh_sb[:], bkh[i])
                bvr_sb = wp.tile([1, D], F32R, tag="bvr")
                nc.sync.dma_start(bvr_sb[:], bvr[i].unsqueeze(0).bitcast(F32R))
                bor_sb = wp.tile([1, D], F32R, tag="bor")
                nc.sync.dma_start(bor_sb[:], bor[i].unsqueeze(0).bitcast(F32R))
                b2r_sb = wp.tile([1, D], F32R, tag="b2r")
                nc.sync.dma_start(b2r_sb[:], b2r[i].unsqueeze(0).bitcast(F32R))
                b1h_sb = wp.tile([128, FC], F32, tag="b1h")
                nc.sync.dma_start(b1h_sb[:], b1h[i])

                # ---------------- attention phase (all b) ----------------
                for b in range(BLOC):
                    x_b = load_x(b, i)
                    rs1 = stp.tile([128, L], F32, tag="rs1")
                    ln_stats(x_b, rs1[:])
                    xs = scaled(x_b, rs1[:])
                    if DEBUG and i == 0 and b == 0:
                        nc.sync.dma_start(dbg["d_xs"], xs[:])

                    # v projection, token-major, + ones column for denominators
                    vext = vep.tile([128, DC, H, HD + 1], F32R, tag="vext")
                    nc.vector.tensor_copy(
                        vext[:, :, :, HD:HD + 1],
                        ones_col[:].unsqueeze(1).unsqueeze(1).broadcast_to([128, DC, H, 1]),
                    )
                    for t in range(DC):
                        ps_v = ps_a.tile([128, D], F32, tag="seq")
                        for dc in range(DC):
                            nc.tensor.matmul(
                                ps_v[:], xs[:, dc, t * 128:(t + 1) * 128],
                                wv_sb[:, dc, :], start=(dc == 0),
                                stop=(not with_biases and dc == DC - 1),
                            )
                        if with_biases:
                            nc.tensor.matmul(ps_v[:], ones_row[0:1, 0:128], bvr_sb[:],
                                             start=False, stop=True)
                        nc.vector.tensor_copy(
                            vext[:, t, :, 0:HD],
                            ps_v[:].rearrange("p (h e) -> p h e", h=H),
                        )
                    if DEBUG and i == 0 and b == 0:
                        nc.sync.dma_start(dbg["d_v"], vext[:])

                    for pair in range(NP):
                        h0, h1 = 2 * pair, 2 * pair + 1
                        ps_q = ps_a.tile([128, L], F32, tag="seq")
                        ps_k = ps_a.tile([128, L], F32, tag="seq")
                        for dc in range(DC):
                            nc.tensor.matmul(
                                ps_q[:], wq_sb[:, dc, pair * 128:(pair + 1) * 128],
                                xs[:, dc, :], start=(dc == 0), stop=(dc == DC - 1),
                            )
                        for dc in range(DC):
                            nc.tensor.matmul(
                                ps_k[:], wk_sb[:, dc, pair * 128:(pair + 1) * 128],
                                xs[:, dc, :], start=(dc == 0), stop=(dc == DC - 1),
                            )
                        qx, kx = {}, {}
                        for hh in (h0, h1):
                            off = 64 * (hh % 2)
                            qx[hh] = qkp.tile([80, L], F32R, tag="qx", name=f"qx{hh}")
                            nc.vector.tensor_scalar_add(
                                out=qx[hh][0:HD, :], in0=ps_q[off:off + HD, :],
                                scalar1=bqh_sb[:, hh:hh + 1],
                            )
                            nc.sync.dma_start(qx[hh][HD:HD + 10, :], qe[b].bitcast(F32R))
                            kx[hh] = qkp.tile([80, L], F32R, tag="kx", name=f"kx{hh}")
                            nc.vector.tensor_scalar_add(
                                out=kx[hh][0:HD, :], in0=ps_k[off:off + HD, :],
                                scalar1=bkh_sb[:, hh:hh + 1],
                            )
                            nc.sync.dma_start(kx[hh][HD:HD + 10, :], ke[i, hh, b].bitcast(F32R))
                        if DEBUG and i == 0 and b == 0 and pair == 0:
                            nc.sync.dma_start(dbg["d_qx"], qx[h0][:])
                            nc.sync.dma_start(dbg["d_kx"], kx[h0][:])

                        attnT = atp.tile([128, L], BF16, tag="attnT")
                        for hh in (h0, h1):
                            wts = []
                            ps_o = ps_av.tile([HD + 1, L], F32, tag="av")
                            for cs in range(DC):
                                n0 = cs * 128
                                ps_s = ps_a.tile([128, L], F32, tag="seq")
                                nc.tensor.matmul(
                                    ps_s[:, 0:L - n0],
                                    kx[hh][0:74, n0:n0 + 128],
                                    qx[hh][0:74, n0:L],
                                    start=True, stop=True,
                                )
                                nc.vector.tensor_add(ps_s[:, 0:128], ps_s[:, 0:128],
                                                     tri_sb[:])
                                if DEBUG and i == 0 and b == 0 and hh == 0:
                                    _scd = stp.tile([128, L], F32, tag="scd", name=f"scd{cs}")
                                    nc.vector.tensor_copy(_scd[:, 0:L - n0], ps_s[:, 0:L - n0])
                                    nc.sync.dma_start(dbg["d_sc"][cs, :, 0:L - n0], _scd[:, 0:L - n0])
                                wt = wtp.tile([128, L], F32R, tag="wt", name=f"wt{cs}")
                                nc.scalar.activation(
                                    out=wt[:, 0:L - n0], in_=ps_s[:, 0:L - n0],
                                    func=AF.Exp, scale=1.0,
                                )
                                if DEBUG and i == 0 and b == 0 and hh == 0:
                                    nc.sync.dma_start(dbg["d_wt"][cs], wt[:])
                                wts.append(wt)
                            for cs in range(DC):
                                n0 = cs * 128
                                nc.tensor.matmul(
                                    ps_o[:, n0:L], vext[:, cs, hh, :],
                                    wts[cs][:, 0:L - n0],
                                    start=(cs == 0), stop=(cs == DC - 1),
                                )
                            if DEBUG and i == 0 and b == 0 and hh == 0:
                                _pod = atp.tile([HD + 1, L], F32, tag="pod")
                                nc.vector.tensor_copy(_pod[:], ps_o[:])
                                nc.sync.dma_start(dbg["d_po"], _pod[:])
                            den = st1.tile([1, L], F32, tag="den")
                            nc.vector.tensor_copy(den[:], ps_o[HD:HD + 1, :])
                            rcp = st1.tile([1, L], F32, tag="rcp")
                            rcs = st1.tile([1, L], F32, tag="rcs")
                            nc.vector.reciprocal_approx_accurate(
                                rcp[:], den[:], rcs[:]
                            )
                            rb = stp.tile([HD, L], F32, tag="rb")
                            nc.gpsimd.partition_broadcast(rb[:], rcp[:])
                            if DEBUG and i == 0 and b == 0 and hh == 0:
                                nc.sync.dma_start(dbg["d_rb"], rb[:])
                            off = 64 * (hh % 2)
                            nc.vector.tensor_mul(attnT[off:off + HD, :],
                                                 ps_o[0:HD, :], rb[:])
                        if DEBUG and i == 0 and b == 0 and pair == 0:
                            nc.sync.dma_start(dbg["d_at"], attnT[:])

                        for oc in range(DC):
                            if pair == 0:
                                ps_x = ps_acc.tile([128, L], F32, tag="acc", name=f"psx{oc}")
                                if oc == 0:
                                    ps_xs = []
                                ps_xs.append(ps_x)
                            nc.tensor.matmul(
                                ps_xs[oc][:], wo_sb[:, pair, oc * 128:(oc + 1) * 128],
                                attnT[:], start=(pair == 0),
                                stop=(not with_biases and pair == NP - 1),
                            )
                    for oc in range(DC):
                        if with_biases:
                            nc.tensor.matmul(
                                ps_xs[oc][:], bor_sb[0:1, oc * 128:(oc + 1) * 128],
                                ones_row[:], start=False, stop=True,
                            )
                        nc.vector.tensor_add(x_b[:, oc, :], ps_xs[oc][:], x_b[:, oc, :])

                    if DEBUG and i == 0 and b == 0:
                        nc.sync.dma_start(dbg["d_x1"], x_b[:].bitcast(F32))
                    # LN2 stats on post-attention x (rs kept for both ffn halves)
                    rsb = rs4p.tile([128, L], F32, tag="rsb")
                    ln_stats(x_b, rsb[:])
                    if b == 0:
                        rs_list = []
                    rs_list.append(rsb)
                    nc.sync.dma_start(xa[b], x_b[:].bitcast(F32))

                # ---------------- FFN phase (two streamed halves) ----------------
                for half in range(2):
                    w1h = wf.tile([128, DC, FH * 128], BF16, tag="w1h")
                    nc.sync.dma_start(
                        w1h[:],
                        w1[i, :, half * FH * 128:(half + 1) * FH * 128]
                        .rearrange("(c p) o -> p c o", p=128),
                    )
                    w2h = wf.tile([128, FH, D], BF16, tag="w2h")
                    nc.sync.dma_start(
                        w2h[:],
                        w2[i, half * FH * 128:(half + 1) * FH * 128, :]
                        .rearrange("(c p) o -> p c o", p=128),
                    )
                    for b in range(BLOC):
                        x_b = load_x(b, 1, src=xa)
                        xs2 = scaled(x_b, rs_list[b][:])
                        if half == 1:
                            x_b = load_x(b, 1, src=xw)
                        ps_f = [ps_acc.tile([128, L], F32, tag="acc", name=f"psf{_oc}")
                                for _oc in range(DC)]
                        for fc in range(FH):
                            gfc = half * FH + fc
                            ps_h = ps_a.tile([128, L], F32, tag="seq")
                            for dc in range(DC):
                                nc.tensor.matmul(
                                    ps_h[:], w1h[:, dc, fc * 128:(fc + 1) * 128],
                                    xs2[:, dc, :], start=(dc == 0), stop=(dc == DC - 1),
                                )
                            h1g = h1p.tile([128, L], BF16, tag="h1g")
                            nc.scalar.activation(
                                out=h1g[:], in_=ps_h[:], func=AF.Gelu,
                                bias=b1h_sb[:, gfc:gfc + 1], scale=1.0,
                            )
                            for oc in range(DC):
                                nc.tensor.matmul(
                                    ps_f[oc][:], w2h[:, fc, oc * 128:(oc + 1) * 128],
                                    h1g[:], start=(fc == 0),
                                    stop=(fc == FH - 1 and (half == 0 or not with_biases)),
                                )
                        for oc in range(DC):
                            if half == 1 and with_biases:
                                nc.tensor.matmul(
                                    ps_f[oc][:],
                                    b2r_sb[0:1, oc * 128:(oc + 1) * 128],
                                    ones_row[:], start=False, stop=True,
                                )
                            nc.vector.tensor_add(x_b[:, oc, :], ps_f[oc][:],
                                                 x_b[:, oc, :])
                        if DEBUG and i == 0 and b == 0 and half == 1:
                            nc.sync.dma_start(dbg["d_x2"], x_b[:].bitcast(F32))
                        nc.sync.dma_start(xw[b], x_b[:].bitcast(F32))

            # ---------------- final layernorm + transpose ----------------
            for b in range(BLOC):
                x_b = load_x(b, NLB)
                rs1 = stp.tile([128, L], F32, tag="rs1")
                mu = stp.tile([128, L], F32, tag="mu")
                ln_stats(x_b, rs1[:], mu_out=mu[:])
                xc = xsp.tile([128, DC, L], F32, tag="xs")
                nc.vector.tensor_sub(
                    xc[:], x_b[:], mu[:].unsqueeze(1).broadcast_to([128, DC, L])
                )
                xf = sqp.tile([128, DC, L], F32, tag="xsq")
                nc.vector.tensor_mul(
                    xf[:], xc[:], rs1[:].unsqueeze(1).broadcast_to([128, DC, L])
                )
                for dc in range(DC):
                    nc.vector.tensor_scalar(
                        out=xf[:, dc, :], in0=xf[:, dc, :],
                        scalar1=fng_sb[:, dc:dc + 1], scalar2=fnb_sb[:, dc:dc + 1],
                        op0=AL.mult, op1=AL.add,
                    )
                for t in range(DC):
                    o_sb = osb.tile([128, D], F32, tag="osb")
                    for dc in range(DC):
                        ps_t = ps_a.tile([128, 128], F32, tag="seq")
                        nc.tensor.transpose(
                            ps_t[:], xf[:, dc, t * 128:(t + 1) * 128], ident[:]
                        )
                        nc.vector.tensor_copy(o_sb[:, dc * 128:(dc + 1) * 128], ps_t[:])
                    nc.sync.dma_start(out[b, t * 128:(t + 1) * 128, :], o_sb[:])

    nc.compile()
    return nc


def _center_cols(W):
    return W - W.mean(axis=0, keepdims=True)


def _preprocess(inputs):
    """Host-side folding; returns per-core in_maps."""
    f32 = np.float32
    g = {k: np.asarray(v) for k, v in inputs.items()}
    Wq, Wk, Wv, Wo = g["Wq"], g["Wk"], g["Wv"], g["Wo"]
    W1, W2 = g["W1"], g["W2"]
    g1, b1n = g["ln1_g"], g["ln1_b"]
    g2, b2n = g["ln2_g"], g["ln2_b"]

    wq_e = np.stack([_center_cols(g1[i][:, None] * Wq[i]) / SCALE for i in range(NL)]).astype(f32)
    bq_e = np.stack([(g["bq"][i] + b1n[i] @ Wq[i]) / SCALE for i in range(NL)]).astype(f32)
    wk_e = np.stack([_center_cols(g1[i][:, None] * Wk[i]) for i in range(NL)]).astype(f32)
    bk_e = np.stack([g["bk"][i] + b1n[i] @ Wk[i] for i in range(NL)]).astype(f32)
    wv_e = np.stack([_center_cols(g1[i][:, None] * Wv[i]) for i in range(NL)]).astype(f32)
    bv_e = np.stack([g["bv"][i] + b1n[i] @ Wv[i] for i in range(NL)]).astype(f32)
    w1_e = np.stack([_center_cols(g2[i][:, None] * W1[i]) for i in range(NL)]).astype(f32)
    b1_e = np.stack([g["b1"][i] + b2n[i] @ W1[i] for i in range(NL)]).astype(f32)

    ci = g["case_ids"].astype(np.int64)
    am = g["attention_mask"].astype(f32)
    verb = (ci == 8).astype(f32)
    qe = np.zeros((B, 10, L), f32)
    for c in range(C):
        qe[:, c, :] = (ci == c)
    qe[:, 9, :] = 1.0
    cb = g["case_bias"].astype(f32)
    vb = g["verb_bias"].astype(f32)
    ke = np.zeros((NL, H, B, 10, L), f32)
    for i in range(NL):
        for h in range(H):
            ke[i, h, :, 0:C, :] = np.transpose(cb[i, h][:, ci], (1, 0, 2))
            ke[i, h, :, 9, :] = vb[i, h] * verb - 10000.0 * (1.0 - am)

    tri = np.where(
        np.arange(128)[:, None] > np.arange(128)[None, :], f32(-10000.0), f32(0.0)
    ).astype(f32)

    import ml_dtypes
    bf16 = ml_dtypes.bfloat16
    common = {
        "wq": wq_e[:NLB].astype(bf16), "wk": wk_e[:NLB].astype(bf16),
        "wv": wv_e[:NLB].astype(bf16),
        "wo": np.ascontiguousarray(Wo.astype(f32)[:NLB]).astype(bf16),
        "w1": w1_e[:NLB].astype(bf16),
        "w2": np.ascontiguousarray(W2.astype(f32)[:NLB]).astype(bf16),
        "bqh": np.ascontiguousarray(bq_e.reshape(NL, H, HD).transpose(0, 2, 1))[:NLB],
        "bkh": np.ascontiguousarray(bk_e.reshape(NL, H, HD).transpose(0, 2, 1))[:NLB],
        "bvr": bv_e[:NLB], "bor": np.ascontiguousarray(g["bo"].astype(f32)[:NLB]),
        "b1h": np.ascontiguousarray(b1_e.reshape(NL, FC, 128).transpose(0, 2, 1))[:NLB],
        "b2r": np.ascontiguousarray(g["b2"].astype(f32)[:NLB]),
        "tri": tri,
        "fng": np.ascontiguousarray(g["fn_g"].astype(f32).reshape(DC, 128).T),
        "fnb": np.ascontiguousarray(g["fn_b"].astype(f32).reshape(DC, 128).T),
    }
    x = g["x"].astype(f32)
    in_maps = []
    for core in range(NCORES):
        sl = slice(core * BLOC, (core + 1) * BLOC)
        m = dict(common)
        m["xT"] = np.ascontiguousarray(x[sl].reshape(BLOC * L, D).T)
        m["qe"] = np.ascontiguousarray(qe[sl])
        m["ke"] = np.ascontiguousarray(ke[:NLB, :, sl])
        in_maps.append(m)
    return in_maps


def _get_program(with_biases=None):
    global _PROGRAM
    if _PROGRAM is not None and (with_biases is None or _PROGRAM[1] == with_biases):
        return _PROGRAM[0]
    wb = True if with_biases is None else with_biases
    _PROGRAM = (_build_program(wb), wb)
    return _PROGRAM[0]


def kernel(**inputs) -> np.ndarray:
    from concourse.bass_utils import run_bass_kernel_spmd

    wb = any(
        np.any(np.asarray(inputs[k])) for k in ("bv", "bo", "b2")
    ) or np.any(np.asarray(inputs["ln1_b"])) or np.any(np.asarray(inputs["ln2_b"]))
    nc = _get_program(with_biases=wb)
    in_maps = _preprocess(inputs)
    res = run_bass_kernel_spmd(nc, in_maps, list(range(NCORES)))
    return np.concatenate(
        [res.results[c]["out"] for c in range(NCORES)], axis=0
    ).astype(np.float32)


# revision 31
# speedup vs baseline: 1.1681x; 1.0245x over previous
"""BrahmanTransformer Trainium2 kernel.

Sharding: data-parallel over batch (32 -> 4 sequences per core x 8 cores),
full 6-layer transformer per core in one Bass/Tile program; float32r matmuls.

Device dataflow (per core, d-major residual):
- LayerNorm: gamma/beta folded into weights host-side; mean subtraction folded
  via column-centered weights; on-device LN is xs = x * rsqrt(var+eps) only.
- Attention: case one-hot (9) + const row (1) appended to q, gathered
  case-bias columns + verb/pad row appended to k -> single K=74 score matmul.
  scoresT layout [key, query]; causality via partial-width matmuls; softmax
  without max-subtraction (scores are O(1)); denominators via a ones column
  appended to V (token-major) and the AV matmul.
- FFN: fc-halves streamed from HBM; psum-accumulated ffn2; gelu on ACT.
- All biases exact: ACT-copy bias (q/k), K=1 matmuls (v/o/ffn2), gelu bias.
"""
import os
import numpy as np

B, L, D, H, NL, F, C = 32, 512, 512, 8, 6, 2048, 9
HD = D // H
NCORES = 8
BLOC = B // NCORES            # 4 sequences per core
SCALE = float(np.sqrt(HD))
EPS = 1e-5
NLB = int(os.environ.get("KB_LAYERS", str(NL)))  # build-depth (debug aid)
DEBUG = bool(int(os.environ.get("KB_DEBUG", "0")))

DC = D // 128     # 4 d-chunks
FC = F // 128     # 16 f-chunks
FH = FC // 2      # 8 f-chunks per streamed half
NP = H // 2       # 4 head pairs

_PROGRAM = None


def _build_program(with_biases=True):
    import concourse.bacc as bacc
    import concourse.mybir as mybir
    import concourse.tile as tile
    from concourse.masks import make_identity

    F32 = mybir.dt.float32
    F32R = mybir.dt.float32r
    AF = mybir.ActivationFunctionType
    AL = mybir.AluOpType

    nc = bacc.Bacc("TRN2", target_bir_lowering=False, debug=False)

    BF16 = mybir.dt.bfloat16

    def inp(name, shape, dt=F32):
        return nc.dram_tensor(name, shape, dt, kind="ExternalInput").ap()

    xT = inp("xT", [D, BLOC * L])
    wq = inp("wq", [NLB, D, D], BF16)
    wk = inp("wk", [NLB, D, D], BF16)
    wv = inp("wv", [NLB, D, D], BF16)
    wo = inp("wo", [NLB, D, D], BF16)
    w1 = inp("w1", [NLB, D, F], BF16)
    w2 = inp("w2", [NLB, F, D], BF16)
    bqh = inp("bqh", [NLB, HD, H])
    bkh = inp("bkh", [NLB, HD, H])
    bvr = inp("bvr", [NLB, D])
    bor = inp("bor", [NLB, D])
    b1h = inp("b1h", [NLB, 128, FC])
    b2r = inp("b2r", [NLB, D])
    qe = inp("qe", [BLOC, 10, L])
    ke = inp("ke", [NLB, H, BLOC, 10, L])
    tri = inp("tri", [128, 128])
    fng = inp("fng", [128, DC])
    fnb = inp("fnb", [128, DC])
    out = nc.dram_tensor("out", [BLOC, L, D], F32, kind="ExternalOutput").ap()
    dbg = {}
    if DEBUG:
        BF16_ = mybir.dt.bfloat16
        for nm, shp, dt_ in [
            ("d_xs", [128, DC, L], BF16_), ("d_v", [128, DC, H, HD + 1], BF16_),
            ("d_qx", [80, L], BF16_), ("d_kx", [80, L], BF16_),
            ("d_wt", [DC, 128, L], BF16_),
            ("d_at", [128, L], BF16_), ("d_x1", [128, DC, L], F32),
            ("d_x2", [128, DC, L], F32),
            ("d_sc", [DC, 128, L], F32), ("d_po", [HD + 1, L], F32),
            ("d_rb", [HD, L], F32),
        ]:
            dbg[nm] = nc.dram_tensor(nm, shp, dt_, kind="ExternalOutput").ap()

    xT_r = xT.rearrange("(dc p) t -> p dc t", p=128)

    from contextlib import ExitStack

    with tile.TileContext(nc) as tc:
        with ExitStack() as _st:
            _e = _st.enter_context
            cp = _e(tc.tile_pool(name="const", bufs=1))
            wp = _e(tc.tile_pool(name="wpool", bufs=1))
            wf = _e(tc.tile_pool(name="wff", bufs=2))
            xbp = _e(tc.tile_pool(name="xb", bufs=2))
            sqp = _e(tc.tile_pool(name="sq", bufs=2))
            xsp = _e(tc.tile_pool(name="xsp", bufs=2))
            stp = _e(tc.tile_pool(name="stats", bufs=2))
            st1 = _e(tc.tile_pool(name="stats1", bufs=1))
            rs4p = _e(tc.tile_pool(name="rs4", bufs=4))
            qkp = _e(tc.tile_pool(name="qk", bufs=4))
            vep = _e(tc.tile_pool(name="vex", bufs=2))
            wtp = _e(tc.tile_pool(name="wtp", bufs=4))
            atp = _e(tc.tile_pool(name="atn", bufs=4))
            h1p = _e(tc.tile_pool(name="h1g", bufs=4))
            osb = _e(tc.tile_pool(name="osb", bufs=2))
            dram = _e(tc.tile_pool(name="dram", bufs=1, space="DRAM"))
            ps_a = _e(tc.tile_pool(name="ps_a", bufs=3, space="PSUM"))
            ps_av = _e(tc.tile_pool(name="ps_av", bufs=1, space="PSUM"))
            ps_acc = _e(tc.tile_pool(name="ps_acc", bufs=4, space="PSUM"))
            # ---- constants
            onesf = cp.tile([1, L], F32, tag="onesf")
            nc.vector.memset(onesf[:], 1.0)
            ones_row = cp.tile([1, L], F32R, tag="ones_row")
            nc.vector.tensor_copy(ones_row[:], onesf[:])
            redf = cp.tile([128, 128], F32, tag="redf")
            nc.vector.memset(redf[:], 1.0 / D)
            red = cp.tile([128, 128], F32R, tag="red")
            nc.vector.tensor_copy(red[:], redf[:])
            eps_t = cp.tile([128, 1], F32, tag="eps")
            nc.vector.memset(eps_t[:], EPS)
            ones_col = cp.tile([128, 1], F32, tag="ones_col")
            nc.vector.memset(ones_col[:], 1.0)
            ident = cp.tile([128, 128], F32, tag="ident")
            make_identity(nc, ident[:])
            tri_sb = cp.tile([128, 128], F32, tag="tri")
            nc.sync.dma_start(tri_sb[:], tri)
            fng_sb = cp.tile([128, DC], F32, tag="fng")
            nc.sync.dma_start(fng_sb[:], fng)
            fnb_sb = cp.tile([128, DC], F32, tag="fnb")
            nc.sync.dma_start(fnb_sb[:], fnb)

            # residual stream working copies in DRAM:
            # xw = running residual; xa = post-attention snapshot (LN2 input)
            xw = dram.tile([BLOC, 128, DC, L], F32, tag="xw")
            xa = dram.tile([BLOC, 128, DC, L], F32, tag="xa")

            def load_x(b, layer, src=None):
                x_b = xbp.tile([128, DC, L], F32R, tag="xb")
                if src is None and layer == 0:
                    nc.sync.dma_start(
                        x_b[:], xT_r[:, :, b * L:(b + 1) * L].bitcast(F32R)
                    )
                else:
                    nc.sync.dma_start(x_b[:], (xw if src is None else src)[b].bitcast(F32R))
                return x_b

            def ln_stats(x_b, rs_out, mu_out=None):
                """rs_out[:] = rsqrt(var(x)+eps) (+ mean into mu_out)."""
                xsq = sqp.tile([128, DC, L], F32R, tag="xsq")
                nc.scalar.activation(out=xsq[:], in_=x_b[:], func=AF.Square, scale=1.0)
                ps_mu = ps_a.tile([128, L], F32, tag="seq")
                ps_sq = ps_a.tile([128, L], F32, tag="seq")
                for dc in range(DC):
                    nc.tensor.matmul(ps_mu[:], red[:], x_b[:, dc, :],
                                     start=(dc == 0), stop=(dc == DC - 1))
                for dc in range(DC):
                    nc.tensor.matmul(ps_sq[:], red[:], xsq[:, dc, :],
                                     start=(dc == 0), stop=(dc == DC - 1))
                musq = stp.tile([128, L], F32, tag="musq")
                nc.scalar.activation(out=musq[:], in_=ps_mu[:], func=AF.Square, scale=1.0)
                nc.vector.tensor_sub(musq[:], ps_sq[:], musq[:])
                nc.scalar.activation(out=musq[:], in_=musq[:], func=AF.Sqrt,
                                     bias=eps_t[:], scale=1.0)
                scr = stp.tile([128, L], F32, tag="scr")
                nc.vector.reciprocal_approx_accurate(rs_out, musq[:], scr[:])
                if mu_out is not None:
                    nc.vector.tensor_copy(mu_out, ps_mu[:])

            def scaled(x_b, rs):
                xs = xsp.tile([128, DC, L], BF16, tag="xs")
                nc.vector.tensor_mul(
                    xs[:], x_b[:], rs.unsqueeze(1).broadcast_to([128, DC, L])
                )
                return xs

            for i in range(NLB):
                wq_sb = wp.tile([128, DC, D], BF16, tag="wq")
                nc.sync.dma_start(wq_sb[:], wq[i].rearrange("(c p) o -> p c o", p=128))
                wk_sb = wp.tile([128, DC, D], BF16, tag="wk")
                nc.sync.dma_start(wk_sb[:], wk[i].rearrange("(c p) o -> p c o", p=128))
                wv_sb = wp.tile([128, DC, D], BF16, tag="wv")
                nc.sync.dma_start(wv_sb[:], wv[i].rearrange("(c p) o -> p c o", p=128))
                wo_sb = wp.tile([128, DC, D], BF16, tag="wo")
                nc.sync.dma_start(wo_sb[:], wo[i].rearrange("(c p) o -> p c o", p=128))
                bqh_sb = wp.tile([HD, H], F32, tag="bqh")
                nc.sync.dma_start(bqh_sb[:], bqh[i])
                bkh_sb = wp.tile([HD, H], F32, tag="bkh")
                nc.sync.dma_start(bkh_sb[:], bkh[i])
                bvr_sb = wp.tile([1, D], F32R, tag="bvr")
                nc.sync.dma_start(bvr_sb[:], bvr[i].unsqueeze(0).bitcast(F32R))
                bor_sb = wp.tile([1, D], F32R, tag="bor")
                nc.sync.dma_start(bor_sb[:], bor[i].unsqueeze(0).bitcast(F32R))
                b2r_sb = wp.tile([1, D], F32R, tag="b2r")
                nc.sync.dma_start(b2r_sb[:], b2r[i].unsqueeze(0).bitcast(F32R))
                b1h_sb = wp.tile([128, FC], F32, tag="b1h")
                nc.sync.dma_start(b1h_sb[:], b1h[i])

                # ---------------- attention phase (all b) ----------------
                for b in range(BLOC):
                    x_b = load_x(b, i)
                    rs1 = stp.tile([128, L], F32, tag="rs1")
                    ln_stats(x_b, rs1[:])
                    xs = scaled(x_b, rs1[:])
                    if DEBUG and i == 0 and b == 0:
                        nc.sync.dma_start(dbg["d_xs"], xs[:])

                    # v projection, token-major, + ones column for denominators
                    vext = vep.tile([128, DC, H, HD + 1], F32R, tag="vext")
                    nc.vector.tensor_copy(
                        vext[:, :, :, HD:HD + 1],
                        ones_col[:].unsqueeze(1).unsqueeze(1).broadcast_to([128, DC, H, 1]),
                    )
                    for t in range(DC):
                        ps_v = ps_a.tile([128, D], F32, tag="seq")
                        for dc in range(DC):
                            nc.tensor.matmul(
                                ps_v[:], xs[:, dc, t * 128:(t + 1) * 128],
                                wv_sb[:, dc, :], start=(dc == 0),
                                stop=(not with_biases and dc == DC - 1),
                            )
                        if with_biases:
                            nc.tensor.matmul(ps_v[:], ones_row[0:1, 0:128], bvr_sb[:],
                                             start=False, stop=True)
                        nc.vector.tensor_copy(
                            vext[:, t, :, 0:HD],
                            ps_v[:].rearrange("p (h e) -> p h e", h=H),
                        )
                    if DEBUG and i == 0 and b == 0:
                        nc.sync.dma_start(dbg["d_v"], vext[:])

                    for pair in range(NP):
                        h0, h1 = 2 * pair, 2 * pair + 1
                        ps_q = ps_a.tile([128, L], F32, tag="seq")
                        ps_k = ps_a.tile([128, L], F32, tag="seq")
                        for dc in range(DC):
                            nc.tensor.matmul(
                                ps_q[:], wq_sb[:, dc, pair * 128:(pair + 1) * 128],
                                xs[:, dc, :], start=(dc == 0), stop=(dc == DC - 1),
                            )
                        for dc in range(DC):
                            nc.tensor.matmul(
                                ps_k[:], wk_sb[:, dc, pair * 128:(pair + 1) * 128],
                                xs[:, dc, :], start=(dc == 0), stop=(dc == DC - 1),
                            )
                        qx, kx = {}, {}
                        for hh in (h0, h1):
                            off = 64 * (hh % 2)
                            qx[hh] = qkp.tile([80, L], F32R, tag="qx", name=f"qx{hh}")
                            if off == 0:
                                nc.scalar.activation(
                                    out=qx[hh][0:HD, :], in_=ps_q[0:HD, :],
                                    func=AF.Identity, bias=bqh_sb[:, hh:hh + 1],
                                    scale=1.0,
                                )
                            else:
                                nc.vector.tensor_scalar_add(
                                    out=qx[hh][0:HD, :], in0=ps_q[off:off + HD, :],
                                    scalar1=bqh_sb[:, hh:hh + 1],
                                )
                            nc.sync.dma_start(qx[hh][HD:HD + 10, :], qe[b].bitcast(F32R))
                            kx[hh] = qkp.tile([80, L], F32R, tag="kx", name=f"kx{hh}")
                            if off == 0:
                                nc.scalar.activation(
                                    out=kx[hh][0:HD, :], in_=ps_k[0:HD, :],
                                    func=AF.Identity, bias=bkh_sb[:, hh:hh + 1],
                                    scale=1.0,
                                )
                            else:
                                nc.vector.tensor_scalar_add(
                                    out=kx[hh][0:HD, :], in0=ps_k[off:off + HD, :],
                                    scalar1=bkh_sb[:, hh:hh + 1],
                                )
                            nc.sync.dma_start(kx[hh][HD:HD + 10, :], ke[i, hh, b].bitcast(F32R))
                        if DEBUG and i == 0 and b == 0 and pair == 0:
                            nc.sync.dma_start(dbg["d_qx"], qx[h0][:])
                            nc.sync.dma_start(dbg["d_kx"], kx[h0][:])

                        attnT = atp.tile([128, L], BF16, tag="attnT")
                        for hh in (h0, h1):
                            wts = []
                            ps_o = ps_av.tile([HD + 1, L], F32, tag="av")
                            for cs in range(DC):
                                n0 = cs * 128
                                ps_s = ps_a.tile([128, L], F32, tag="seq")
                                nc.tensor.matmul(
                                    ps_s[:, 0:L - n0],
                                    kx[hh][0:74, n0:n0 + 128],
                                    qx[hh][0:74, n0:L],
                                    start=True, stop=True,
                                )
                                nc.vector.tensor_add(ps_s[:, 0:128], ps_s[:, 0:128],
                                                     tri_sb[:])
                                if DEBUG and i == 0 and b == 0 and hh == 0:
                                    _scd = stp.tile([128, L], F32, tag="scd", name=f"scd{cs}")
                                    nc.vector.tensor_copy(_scd[:, 0:L - n0], ps_s[:, 0:L - n0])
                                    nc.sync.dma_start(dbg["d_sc"][cs, :, 0:L - n0], _scd[:, 0:L - n0])
                                wt = wtp.tile([128, L], F32R, tag="wt", name=f"wt{cs}")
                                nc.scalar.activation(
                                    out=wt[:, 0:L - n0], in_=ps_s[:, 0:L - n0],
                                    func=AF.Exp, scale=1.0,
                                )
                                if DEBUG and i == 0 and b == 0 and hh == 0:
                                    nc.sync.dma_start(dbg["d_wt"][cs], wt[:])
                                wts.append(wt)
                            for cs in range(DC):
                                n0 = cs * 128
                                nc.tensor.matmul(
                                    ps_o[:, n0:L], vext[:, cs, hh, :],
                                    wts[cs][:, 0:L - n0],
                                    start=(cs == 0), stop=(cs == DC - 1),
                                )
                            if DEBUG and i == 0 and b == 0 and hh == 0:
                                _pod = atp.tile([HD + 1, L], F32, tag="pod")
                                nc.vector.tensor_copy(_pod[:], ps_o[:])
                                nc.sync.dma_start(dbg["d_po"], _pod[:])
                            den = st1.tile([1, L], F32, tag="den")
                            nc.vector.tensor_copy(den[:], ps_o[HD:HD + 1, :])
                            rcp = st1.tile([1, L], F32, tag="rcp")
                            rcs = st1.tile([1, L], F32, tag="rcs")
                            nc.vector.reciprocal_approx_accurate(
                                rcp[:], den[:], rcs[:]
                            )
                            rb = stp.tile([HD, L], F32, tag="rb")
                            nc.gpsimd.partition_broadcast(rb[:], rcp[:])
                            if DEBUG and i == 0 and b == 0 and hh == 0:
                                nc.sync.dma_start(dbg["d_rb"], rb[:])
                            off = 64 * (hh % 2)
                            nc.vector.tensor_mul(attnT[off:off + HD, :],
                                                 ps_o[0:HD, :], rb[:])
                        if DEBUG and i == 0 and b == 0 and pair == 0:
                            nc.sync.dma_start(dbg["d_at"], attnT[:])

                        for oc in range(DC):
                            if pair == 0:
                                ps_x = ps_acc.tile([128, L], F32, tag="acc", name=f"psx{oc}")
                                if oc == 0:
                                    ps_xs = []
                                ps_xs.append(ps_x)
                            nc.tensor.matmul(
                                ps_xs[oc][:], wo_sb[:, pair, oc * 128:(oc + 1) * 128],
                                attnT[:], start=(pair == 0),
                                stop=(not with_biases and pair == NP - 1),
                            )
                    for oc in range(DC):
                        if with_biases:
                            nc.tensor.matmul(
                                ps_xs[oc][:], bor_sb[0:1, oc * 128:(oc + 1) * 128],
                                ones_row[:], start=False, stop=True,
                            )
                        nc.vector.tensor_add(x_b[:, oc, :], ps_xs[oc][:], x_b[:, oc, :])

                    if DEBUG and i == 0 and b == 0:
                        nc.sync.dma_start(dbg["d_x1"], x_b[:].bitcast(F32))
                    # LN2 stats on post-attention x (rs kept for both ffn halves)
                    rsb = rs4p.tile([128, L], F32, tag="rsb")
                    ln_stats(x_b, rsb[:])
                    if b == 0:
                        rs_list = []
                    rs_list.append(rsb)
                    nc.sync.dma_start(xa[b], x_b[:].bitcast(F32))

                # ---------------- FFN phase (two streamed halves) ----------------
                for half in range(2):
                    w1h = wf.tile([128, DC, FH * 128], BF16, tag="w1h")
                    nc.sync.dma_start(
                        w1h[:],
                        w1[i, :, half * FH * 128:(half + 1) * FH * 128]
                        .rearrange("(c p) o -> p c o", p=128),
                    )
                    w2h = wf.tile([128, FH, D], BF16, tag="w2h")
                    nc.sync.dma_start(
                        w2h[:],
                        w2[i, half * FH * 128:(half + 1) * FH * 128, :]
                        .rearrange("(c p) o -> p c o", p=128),
                    )
                    for b in range(BLOC):
                        x_b = load_x(b, 1, src=xa)
                        xs2 = scaled(x_b, rs_list[b][:])
                        if half == 1:
                            x_b = load_x(b, 1, src=xw)
                        ps_f = [ps_acc.tile([128, L], F32, tag="acc", name=f"psf{_oc}")
                                for _oc in range(DC)]
                        for fc in range(FH):
                            gfc = half * FH + fc
                            ps_h = ps_a.tile([128, L], F32, tag="seq")
                            for dc in range(DC):
                                nc.tensor.matmul(
                                    ps_h[:], w1h[:, dc, fc * 128:(fc + 1) * 128],
                                    xs2[:, dc, :], start=(dc == 0), stop=(dc == DC - 1),
                                )
                            h1g = h1p.tile([128, L], BF16, tag="h1g")
                            nc.scalar.activation(
                                out=h1g[:], in_=ps_h[:], func=AF.Gelu,
                                bias=b1h_sb[:, gfc:gfc + 1], scale=1.0,
                            )
                            for oc in range(DC):
                                nc.tensor.matmul(
                                    ps_f[oc][:], w2h[:, fc, oc * 128:(oc + 1) * 128],
                                    h1g[:], start=(fc == 0),
                                    stop=(fc == FH - 1 and (half == 0 or not with_biases)),
                                )
                        for oc in range(DC):
                            if half == 1 and with_biases:
                                nc.tensor.matmul(
                                    ps_f[oc][:],
                                    b2r_sb[0:1, oc * 128:(oc + 1) * 128],
                                    ones_row[:], start=False, stop=True,
                                )
                            nc.vector.tensor_add(x_b[:, oc, :], ps_f[oc][:],
                                                 x_b[:, oc, :])
                        if DEBUG and i == 0 and b == 0 and half == 1:
                            nc.sync.dma_start(dbg["d_x2"], x_b[:].bitcast(F32))
                        nc.sync.dma_start(xw[b], x_b[:].bitcast(F32))

            # ---------------- final layernorm + transpose ----------------
            for b in range(BLOC):
                x_b = load_x(b, NLB)
                rs1 = stp.tile([128, L], F32, tag="rs1")
                mu = stp.tile([128, L], F32, tag="mu")
                ln_stats(x_b, rs1[:], mu_out=mu[:])
                xc = xsp.tile([128, DC, L], F32, tag="xs")
                nc.vector.tensor_sub(
                    xc[:], x_b[:], mu[:].unsqueeze(1).broadcast_to([128, DC, L])
                )
                xf = sqp.tile([128, DC, L], F32, tag="xsq")
                nc.vector.tensor_mul(
                    xf[:], xc[:], rs1[:].unsqueeze(1).broadcast_to([128, DC, L])
                )
                for dc in range(DC):
                    nc.vector.tensor_scalar(
                        out=xf[:, dc, :], in0=xf[:, dc, :],
                        scalar1=fng_sb[:, dc:dc + 1], scalar2=fnb_sb[:, dc:dc + 1],
                        op0=AL.mult, op1=AL.add,
                    )
                for t in range(DC):
                    o_sb = osb.tile([128, D], F32, tag="osb")
                    for dc in range(DC):
                        ps_t = ps_a.tile([128, 128], F32, tag="seq")
                        nc.tensor.transpose(
                            ps_t[:], xf[:, dc, t * 128:(t + 1) * 128], ident[:]
                        )
                        nc.vector.tensor_copy(o_sb[:, dc * 128:(dc + 1) * 128], ps_t[:])
                    nc.sync.dma_start(out[b, t * 128:(t + 1) * 128, :], o_sb[:])

    nc.compile()
    return nc


def _center_cols(W):
    return W - W.mean(axis=0, keepdims=True)


def _preprocess(inputs):
    """Host-side folding; returns per-core in_maps."""
    f32 = np.float32
    g = {k: np.asarray(v) for k, v in inputs.items()}
    Wq, Wk, Wv, Wo = g["Wq"], g["Wk"], g["Wv"], g["Wo"]
    W1, W2 = g["W1"], g["W2"]
    g1, b1n = g["ln1_g"], g["ln1_b"]
    g2, b2n = g["ln2_g"], g["ln2_b"]

    wq_e = np.stack([_center_cols(g1[i][:, None] * Wq[i]) / SCALE for i in range(NL)]).astype(f32)
    bq_e = np.stack([(g["bq"][i] + b1n[i] @ Wq[i]) / SCALE for i in range(NL)]).astype(f32)
    wk_e = np.stack([_center_cols(g1[i][:, None] * Wk[i]) for i in range(NL)]).astype(f32)
    bk_e = np.stack([g["bk"][i] + b1n[i] @ Wk[i] for i in range(NL)]).astype(f32)
    wv_e = np.stack([_center_cols(g1[i][:, None] * Wv[i]) for i in range(NL)]).astype(f32)
    bv_e = np.stack([g["bv"][i] + b1n[i] @ Wv[i] for i in range(NL)]).astype(f32)
    w1_e = np.stack([_center_cols(g2[i][:, None] * W1[i]) for i in range(NL)]).astype(f32)
    b1_e = np.stack([g["b1"][i] + b2n[i] @ W1[i] for i in range(NL)]).astype(f32)

    ci = g["case_ids"].astype(np.int64)
    am = g["attention_mask"].astype(f32)
    verb = (ci == 8).astype(f32)
    qe = np.zeros((B, 10, L), f32)
    for c in range(C):
        qe[:, c, :] = (ci == c)
    qe[:, 9, :] = 1.0
    cb = g["case_bias"].astype(f32)
    vb = g["verb_bias"].astype(f32)
    ke = np.zeros((NL, H, B, 10, L), f32)
    for i in range(NL):
        for h in range(H):
            ke[i, h, :, 0:C, :] = np.transpose(cb[i, h][:, ci], (1, 0, 2))
            ke[i, h, :, 9, :] = vb[i, h] * verb - 10000.0 * (1.0 - am)

    tri = np.where(
        np.arange(128)[:, None] > np.arange(128)[None, :], f32(-10000.0), f32(0.0)
    ).astype(f32)

    import ml_dtypes
    bf16 = ml_dtypes.bfloat16
    common = {
        "wq": wq_e[:NLB].astype(bf16), "wk": wk_e[:NLB].astype(bf16),
        "wv": wv_e[:NLB].astype(bf16),
        "wo": np.ascontiguousarray(Wo.astype(f32)[:NLB]).astype(bf16),
        "w1": w1_e[:NLB].astype(bf16),
        "w2": np.ascontiguousarray(W2.astype(f32)[:NLB]).astype(bf16),
        "bqh": np.ascontiguousarray(bq_e.reshape(NL, H, HD).transpose(0, 2, 1))[:NLB],
        "bkh": np.ascontiguousarray(bk_e.reshape(NL, H, HD).transpose(0, 2, 1))[:NLB],
        "bvr": bv_e[:NLB], "bor": np.ascontiguousarray(g["bo"].astype(f32)[:NLB]),
        "b1h": np.ascontiguousarray(b1_e.reshape(NL, FC, 128).transpose(0, 2, 1))[:NLB],
        "b2r": np.ascontiguousarray(g["b2"].astype(f32)[:NLB]),
        "tri": tri,
        "fng": np.ascontiguousarray(g["fn_g"].astype(f32).reshape(DC, 128).T),
        "fnb": np.ascontiguousarray(g["fn_b"].astype(f32).reshape(DC, 128).T),
    }
    x = g["x"].astype(f32)
    in_maps = []
    for core in range(NCORES):
        sl = slice(core * BLOC, (core + 1) * BLOC)
        m = dict(common)
        m["xT"] = np.ascontiguousarray(x[sl].reshape(BLOC * L, D).T)
        m["qe"] = np.ascontiguousarray(qe[sl])
        m["ke"] = np.ascontiguousarray(ke[:NLB, :, sl])
        in_maps.append(m)
    return in_maps


def _get_program(with_biases=None):
    global _PROGRAM
    if _PROGRAM is not None and (with_biases is None or _PROGRAM[1] == with_biases):
        return _PROGRAM[0]
    wb = True if with_biases is None else with_biases
    _PROGRAM = (_build_program(wb), wb)
    return _PROGRAM[0]


def kernel(**inputs) -> np.ndarray:
    from concourse.bass_utils import run_bass_kernel_spmd

    wb = any(
        np.any(np.asarray(inputs[k])) for k in ("bv", "bo", "b2")
    ) or np.any(np.asarray(inputs["ln1_b"])) or np.any(np.asarray(inputs["ln2_b"]))
    nc = _get_program(with_biases=wb)
    in_maps = _preprocess(inputs)
    res = run_bass_kernel_spmd(nc, in_maps, list(range(NCORES)))
    return np.concatenate(
        [res.results[c]["out"] for c in range(NCORES)], axis=0
    ).astype(np.float32)
